# revision 1
# baseline (speedup 1.0000x reference)
"""GNN message-passing (SE3-style graph attention) kernel for 8 Trainium2 cores.

Edge-parallel strategy:
- Nodes relabeled into 4 "subtables" x 8 cores so per-edge kv-gather indices
  fit int16 (dma_gather requirement). Each core owns 12500 dst nodes.
- Per layer: on-device q/k/v projections -> 4 AllGathers build global kv
  tables -> 4 passes over src subtables, each processing edges in node-major
  degree-sorted chunks: dma_gather kv rows, affine q broadcast, DVE
  scores/exp/messages, affine segment reduction into per-pass accumulators.
- Pass accumulators (degree-rank order) recombined into identity order by
  small dma_gathers, then divide / Wo project / residual / LayerNorm.
- Final: W_out, mean-pool via PE ones-matmul, AllReduce, FC head.
"""

import os
import sys
import types
import numpy as np

HEADS = 8
HEAD_DIM = 2
HIDDEN = 16
DIM = 4
DEPTH = 2
N_SUB = 4
KV_COLS = 64          # table row = 64 fp32 = 256B
SENT = 32767          # zeroed sentinel row in each kv subtable
SD_CAP = 128          # max S*D per chunk
S_CAP = 8

_RUN_CACHE = {}


# --------------------------------------------------------------------------
# harness shims (self-contained copies)
# --------------------------------------------------------------------------
def _split_excess_waits(nc, max_waits=1):
    """Walrus build allows 1 sync-wait per instruction; move extras to NOPs."""
    import concourse.mybir as mybir
    n = [0]
    for blk in nc.m.functions[0].blocks:
        new_insts = []
        for inst in blk.instructions:
            si = inst.sync_info
            if si is not None and len(si.on_wait) > max_waits:
                waits = list(si.on_wait)
                extra, keep = waits[:-max_waits], waits[-max_waits:]
                for i in range(0, len(extra), max_waits):
                    n[0] += 1
                    nop = mybir.InstNoOp(
                        name=f"IWS-{n[0]}", engine=inst.engine, ins=[], outs=[],
                        sync_info=mybir.SyncInfo(on_wait=extra[i:i + max_waits],
                                                 on_update=[]))
                    try:
                        nc.register_instruction(nop, overwrite=True)
                    except Exception:
                        pass
                    new_insts.append(nop)
                si.on_wait = keep
            new_insts.append(inst)
        blk.instructions[:] = new_insts


def _install_profhook():
    if 'antenv.axon_hooks' in sys.modules:
        return
    try:
        import antenv
        from trn_agent_boot.trn_boot import _ntff_profile_via_ctypes
        hook = _ntff_profile_via_ctypes('/opt/axon/libaxon_pjrt.so')
        mod = types.ModuleType('antenv.axon_hooks')
        state = {'hook': hook}
        mod.set_axon_ntff_profile_hook = lambda h: state.__setitem__('hook', h)
        mod.get_axon_ntff_profile_hook = lambda: state['hook']
        sys.modules['antenv.axon_hooks'] = mod
        antenv.axon_hooks = mod
    except Exception:
        pass


# --------------------------------------------------------------------------
# host-side planning
# --------------------------------------------------------------------------
def _cfg(n_nodes):
    n_cores = 8
    n_per_sub = n_nodes // N_SUB
    per_core_sub = n_per_sub // n_cores
    npc = N_SUB * per_core_sub
    npp = ((npc + 127) // 128) * 128
    return dict(n_nodes=n_nodes, n_cores=n_cores, n_per_sub=n_per_sub,
                per_core_sub=per_core_sub, npc=npc, npp=npp,
                n_tiles=npp // 128)


def _pack_gidx(idx_flat):
    """Gather feed position i lives at tile[i%16, i//16]; replicate x8 cores."""
    n = idx_flat.shape[0]
    assert n % 16 == 0
    tile16 = np.ascontiguousarray(
        idx_flat.reshape(n // 16, 16).T.astype(np.int16))
    return np.tile(tile16, (8, 1))


def _plan(edge_index, cfg):
    src = np.asarray(edge_index[0], dtype=np.int64)
    dst = np.asarray(edge_index[1], dtype=np.int64)
    nps, pcs = cfg["n_per_sub"], cfg["per_core_sub"]
    npp, n_cores = cfg["npp"], cfg["n_cores"]

    e_core = (dst % nps) // pcs
    e_l = (dst // nps) * pcs + (dst % pcs)
    e_t = src // nps
    e_row = src % nps

    passes = []
    for t in range(N_SUB):
        per_core = []
        for c in range(n_cores):
            m = (e_core == c) & (e_t == t)
            lt, rowt = e_l[m], e_row[m]
            deg = np.bincount(lt, minlength=npp).astype(np.int64)
            order = np.argsort(-deg, kind="stable")
            rank_of = np.empty(npp, dtype=np.int64)
            rank_of[order] = np.arange(npp)
            eorder = np.argsort(rank_of[lt], kind="stable")
            per_core.append(dict(deg=deg, order=order, rank_of=rank_of,
                                 lt=lt[eorder], rowt=rowt[eorder]))
        passes.append(per_core)

    schedule = []
    for t in range(N_SUB):
        chunks = []
        r0 = 0
        degs_sorted = [passes[t][c]["deg"][passes[t][c]["order"]]
                       for c in range(n_cores)]
        while r0 < npp:
            D = int(max(int(d[r0]) for d in degs_sorted))
            if D == 0:
                break
            S = max(1, min(S_CAP, SD_CAP // D, (npp - r0) // 128))
            chunks.append((r0, S, D))
            r0 += 128 * S
        schedule.append(chunks)

    plans = []
    for c in range(n_cores):
        gidx_cols, qidx_list, chunk_meta = [], [], []
        npad = np.zeros(npp, dtype=np.float64)
        gcol0 = qcol0 = 0
        for t in range(N_SUB):
            pc = passes[t][c]
            deg, order, rank_of = pc["deg"], pc["order"], pc["rank_of"]
            lt, rowt = pc["lt"], pc["rowt"]
            offs = np.zeros(npp + 1, dtype=np.int64)
            offs[1:] = np.cumsum(deg[order])
            ranks_e = rank_of[lt]
            j_e = np.arange(lt.shape[0]) - offs[ranks_e]
            for ci, (r0, S, D) in enumerate(schedule[t]):
                nrows = 128 * S * D
                idx_flat = np.full(nrows, SENT, dtype=np.int64)
                em = (ranks_e >= r0) & (ranks_e < r0 + 128 * S)
                q = ranks_e[em] - r0
                pos = ((q // 128) * D + j_e[em]) * 128 + (q % 128)
                idx_flat[pos] = rowt[em]
                gidx_cols.append(_pack_gidx(idx_flat))
                qidx_list.append(_pack_gidx(order[r0:r0 + 128 * S]))
                ch_nodes = order[r0:r0 + 128 * S]
                npad[ch_nodes] += D - deg[ch_nodes]
                chunk_meta.append((t, r0, S, D, gcol0, nrows // 16, qcol0))
                gcol0 += nrows // 16
                qcol0 += 8 * S
        gidx = (np.concatenate(gidx_cols, axis=1) if gidx_cols
                else np.zeros((128, 16), np.int16))
        qidx = (np.concatenate(qidx_list, axis=1) if qidx_list
                else np.zeros((128, 8), np.int16))
        cidx = np.stack([_pack_gidx(passes[t][c]["rank_of"][:npp])
                         for t in range(N_SUB)])
        npad_t = np.ascontiguousarray(
            npad.reshape(cfg["n_tiles"], 128).T.astype(np.float32))
        plans.append(dict(gidx=gidx, qidx=qidx, cidx=cidx, npad=npad_t,
                          chunk_meta=chunk_meta))
    return schedule, plans


def _permute_weights(W_in, Wq, Wk, Wv, Wo, W_out, fc_w, fc_b):
    perm = np.array([h * HEAD_DIM + d for d in range(HEAD_DIM)
                     for h in range(HEADS)], dtype=np.int64)
    return dict(W_in=np.ascontiguousarray(W_in),
                Wq=np.ascontiguousarray(Wq[:, :, perm]),
                Wk=np.ascontiguousarray(Wk[:, :, perm]),
                Wv=np.ascontiguousarray(Wv),
                Wo=np.ascontiguousarray(Wo),
                W_out=np.ascontiguousarray(W_out),
                fcw=np.ascontiguousarray(fc_w.T).reshape(1, 12).astype(np.float32),
                fcb=fc_b.reshape(1, 3).astype(np.float32))


# --------------------------------------------------------------------------
# device program
# --------------------------------------------------------------------------
def _build(meta, cfg):
    import concourse.bass as bass
    import concourse.mybir as mybir
    import concourse.tile as tile
    from concourse import library_config
    from concourse.masks import make_identity
    from concourse.library_overlay import lower_extended_insts

    dt = mybir.dt
    AX = mybir.AxisListType
    OP = mybir.AluOpType
    ACTF = mybir.ActivationFunctionType
    npp, n_tiles, npc = cfg["npp"], cfg["n_tiles"], cfg["npc"]
    pcs = cfg["per_core_sub"]
    g_cols_total, q_cols_total, chunk_meta = meta

    nc = bass.Bass(num_devices=cfg["n_cores"], num_swdge_queues=4)

    x_in = nc.dram_tensor("x_in", [npp, DIM], dt.float32, kind="ExternalInput")
    gidx_d = nc.dram_tensor("gidx", [128, g_cols_total], dt.int16, kind="ExternalInput")
    qidx_d = nc.dram_tensor("qidx", [128, q_cols_total], dt.int16, kind="ExternalInput")
    cidx_d = nc.dram_tensor("cidx", [N_SUB, 128, npp // 16], dt.int16, kind="ExternalInput")
    npad_d = nc.dram_tensor("npad", [128, n_tiles], dt.float32, kind="ExternalInput")
    w_in_d = nc.dram_tensor("w_in", [DIM, HIDDEN], dt.float32, kind="ExternalInput")
    wq_d = nc.dram_tensor("wq", [DEPTH, HIDDEN, HIDDEN], dt.float32, kind="ExternalInput")
    wk_d = nc.dram_tensor("wk", [DEPTH, HIDDEN, HIDDEN], dt.float32, kind="ExternalInput")
    wv_d = nc.dram_tensor("wv", [DEPTH, HIDDEN, HIDDEN], dt.float32, kind="ExternalInput")
    wo_d = nc.dram_tensor("wo", [DEPTH, HIDDEN, HIDDEN], dt.float32, kind="ExternalInput")
    wout_d = nc.dram_tensor("wout", [HIDDEN, DIM], dt.float32, kind="ExternalInput")
    fcw_d = nc.dram_tensor("fcw", [1, 12], dt.float32, kind="ExternalInput")
    fcb_d = nc.dram_tensor("fcb", [1, 3], dt.float32, kind="ExternalInput")
    y_out = nc.dram_tensor("y", [1, 3], dt.float32, kind="ExternalOutput")

    kv_tab = [nc.dram_tensor(f"kvtab{t}", [32768, KV_COLS], dt.float32,
                             kind="Internal", addr_space="Shared")
              for t in range(N_SUB)]
    q_tab = nc.dram_tensor("qtab", [npp, KV_COLS], dt.float32, kind="Internal")
    u_tab = [nc.dram_tensor(f"utab{t}", [npp, KV_COLS], dt.float32, kind="Internal")
             for t in range(N_SUB)]
    stage_d = nc.dram_tensor("stage", [npc, KV_COLS], dt.float32, kind="Internal")
    ar_in = nc.dram_tensor("ar_in", [1, 4], dt.float32, kind="Internal")
    ar_out = nc.dram_tensor("ar_out", [1, 4], dt.float32, kind="Internal",
                            addr_space="Shared")

    nc.gpsimd.load_library(library_config.attnmlp)
    gq = [0]
    _nregs = {}

    def gather(out_ap, in_ap, idx_ap, num_idxs):
        if num_idxs not in _nregs:
            _nregs[num_idxs] = nc.gpsimd.to_reg(num_idxs)
        nc.gpsimd.dma_gather(out_ap=out_ap, in_ap=in_ap, idxs_ap=idx_ap,
                             num_idxs=num_idxs, num_idxs_reg=_nregs[num_idxs],
                             elem_size=KV_COLS, single_packet=False,
                             queue_num=gq[0] % 4)
        gq[0] += 1

    def vap(base_ap, extra_off, dims):
        return bass.AP(base_ap.tensor, base_ap.offset + extra_off, dims)

    with tile.TileContext(nc) as tc:
        with (
            tc.tile_pool(name="const", bufs=1) as cpool,
            tc.tile_pool(name="res", bufs=1) as rpool,
            tc.tile_pool(name="work", bufs=2) as wpool,
            tc.tile_pool(name="gath", bufs=3) as gpool,
            tc.tile_pool(name="small", bufs=2) as spool,
            tc.tile_pool(name="psum", bufs=4, space="PSUM") as ppool,
        ):
            ident = cpool.tile([128, 128], dt.float32, name="ident")
            make_identity(nc, ident[:])
            w_in_t = cpool.tile([DIM, HIDDEN], dt.float32, name="w_in_t")
            nc.sync.dma_start(out=w_in_t[:], in_=w_in_d[:])
            wmat = {}
            for nm, dd in (("wq", wq_d), ("wk", wk_d), ("wv", wv_d), ("wo", wo_d)):
                for l in range(DEPTH):
                    w = cpool.tile([HIDDEN, HIDDEN], dt.float32, name=f"{nm}{l}")
                    nc.sync.dma_start(out=w[:], in_=dd[l])
                    wmat[(nm, l)] = w
            wout_t = cpool.tile([HIDDEN, DIM], dt.float32, name="wout_t")
            nc.sync.dma_start(out=wout_t[:], in_=wout_d[:])
            npad_t = cpool.tile([128, n_tiles], dt.float32, name="npad_t")
            nc.sync.dma_start(out=npad_t[:], in_=npad_d[:])

            zrow = cpool.tile([1, KV_COLS], dt.float32, name="zrow")
            nc.vector.memset(zrow[:], 0.0)
            for t in range(N_SUB):
                nc.sync.dma_start(out=kv_tab[t][SENT:SENT + 1, :], in_=zrow[:])

            h_all = rpool.tile([128, n_tiles, HIDDEN], dt.float32, name="h_all")
            u_tot = rpool.tile([128, n_tiles, 24], dt.float32, name="u_tot")
            u_acc2 = [rpool.tile([128, n_tiles, 24], dt.float32, name=f"u_acc{i}")
                      for i in range(2)]
            acc4 = rpool.tile([128, 4], dt.float32, name="acc4")

            # stage 0: h0 = x @ W_in
            xs = wpool.tile([128, n_tiles, DIM], dt.float32, tag="gt", name="xs")
            nc.sync.dma_start(out=xs[:],
                              in_=x_in[:].rearrange("(a p) f -> p a f", p=128))
            for k in range(n_tiles):
                tp = ppool.tile([DIM, 128], dt.float32, tag="tp", name="tp")
                nc.tensor.transpose(out=tp[:], in_=xs[:, k, :], identity=ident[:])
                tps = spool.tile([HIDDEN, 128], dt.float32, tag="hT", name="tps")
                nc.vector.tensor_copy(out=tps[0:DIM, :], in_=tp[:])
                hp = ppool.tile([128, HIDDEN], dt.float32, tag="mm", name="hp")
                nc.tensor.matmul(out=hp[:], lhsT=tps[0:DIM, :], rhs=w_in_t[:],
                                 start=True, stop=True)
                nc.vector.tensor_copy(out=h_all[:, k, :], in_=hp[:])

            scale = float(1.0 / np.sqrt(HEAD_DIM))

            for layer in range(DEPTH):
                # projections
                for k in range(n_tiles):
                    tp = ppool.tile([HIDDEN, 128], dt.float32, tag="tp", name="htp")
                    nc.tensor.transpose(out=tp[:], in_=h_all[:, k, :],
                                        identity=ident[:])
                    hT = spool.tile([HIDDEN, 128], dt.float32, tag="hT", name="hT")
                    nc.vector.tensor_copy(out=hT[:], in_=tp[:])
                    qp = ppool.tile([128, HIDDEN], dt.float32, tag="mm", name="qp")
                    nc.tensor.matmul(out=qp[:], lhsT=hT[:], rhs=wmat[("wq", layer)][:],
                                     start=True, stop=True)
                    qs = spool.tile([128, HIDDEN], dt.float32, tag="qs", name="qs")
                    nc.vector.tensor_copy(out=qs[:], in_=qp[:])
                    nc.sync.dma_start(out=q_tab[k * 128:(k + 1) * 128, 0:HIDDEN],
                                      in_=qs[:])
                    kp = ppool.tile([128, HIDDEN], dt.float32, tag="mm", name="kp")
                    nc.tensor.matmul(out=kp[:], lhsT=hT[:], rhs=wmat[("wk", layer)][:],
                                     start=True, stop=True)
                    kvs = spool.tile([128, 32], dt.float32, tag="kvs", name="kvs")
                    nc.vector.tensor_copy(out=kvs[:, 0:16], in_=kp[:])
                    vp = ppool.tile([128, HIDDEN], dt.float32, tag="mm", name="vp")
                    nc.tensor.matmul(out=vp[:], lhsT=hT[:], rhs=wmat[("wv", layer)][:],
                                     start=True, stop=True)
                    nc.vector.tensor_copy(out=kvs[:, 16:32], in_=vp[:])
                    lo, hi = k * 128, min((k + 1) * 128, npc)
                    if lo < npc:
                        nc.sync.dma_start(out=stage_d[lo:hi, 0:32],
                                          in_=kvs[0:hi - lo, :])
                for t in range(N_SUB):
                    nc.gpsimd.collective_compute(
                        "AllGather", mybir.AluOpType.bypass,
                        replica_groups=[list(range(cfg["n_cores"]))],
                        ins=[stage_d[t * pcs:(t + 1) * pcs, :]],
                        outs=[kv_tab[t][0:cfg["n_cores"] * pcs, :]])

                nc.vector.memset(u_tot[:], 0.0)
                for t in range(N_SUB):
                    u_acc = u_acc2[t % 2]
                    nc.vector.memset(u_acc[:], 0.0)
                    for (tt, r0, S, D, gc0, gcols, qc0) in chunk_meta:
                        if tt != t:
                            continue
                        sl = r0 // 128
                        gt = wpool.tile([128, 1024], dt.int16, tag="gt", name="gt")
                        nc.sync.dma_start(out=gt[:, 0:gcols],
                                          in_=gidx_d[:, gc0:gc0 + gcols])
                        qt = spool.tile([128, 64], dt.int16, tag="qt", name="qt")
                        nc.sync.dma_start(out=qt[:, 0:8 * S],
                                          in_=qidx_d[:, qc0:qc0 + 8 * S])
                        kvg = gpool.tile([128, SD_CAP, KV_COLS], dt.float32,
                                         tag="kvg", name="kvg")
                        nrow = S * D
                        done = 0
                        while done < nrow:
                            cnt = min(8, nrow - done)
                            gather(kvg[:, done:done + cnt, :], kv_tab[t][:],
                                   gt[:, done * 8:(done + cnt) * 8], cnt * 128)
                            done += cnt
                        qg = spool.tile([128, S_CAP, KV_COLS], dt.float32,
                                        tag="qg", name="qg")
                        gather(qg[:, 0:S, :], q_tab[:], qt[:, 0:8 * S], 128 * S)
                        kvga, qga = kvg[:], qg[:]
                        pk = kvga.ap[0][0]
                        pq = qga.ap[0][0]
                        prod = wpool.tile([128, S * HIDDEN, D],
                                          dt.float32, tag="prod", name="prod")
                        pp = prod[:].ap[0][0]
                        nc.vector.tensor_tensor(
                            out=vap(prod[:], 0,
                                    [[pp, 128], [HIDDEN * D, S], [D, HIDDEN], [1, D]]),
                            in0=vap(qga, 0,
                                    [[pq, 128], [KV_COLS, S], [1, HIDDEN], [0, D]]),
                            in1=vap(kvga, 0,
                                    [[pk, 128], [D * KV_COLS, S], [1, HIDDEN],
                                     [KV_COLS, D]]),
                            op=OP.mult)
                        wgt = wpool.tile([128, S * HEADS, D],
                                         dt.float32, tag="wgt", name="wgt")
                        pw = wgt[:].ap[0][0]
                        nc.vector.tensor_tensor(
                            out=vap(wgt[:], 0,
                                    [[pw, 128], [HEADS * D, S], [D, HEADS], [1, D]]),
                            in0=vap(prod[:], 0,
                                    [[pp, 128], [HIDDEN * D, S], [D, HEADS], [1, D]]),
                            in1=vap(prod[:], HEADS * D,
                                    [[pp, 128], [HIDDEN * D, S], [D, HEADS], [1, D]]),
                            op=OP.add)
                        nc.scalar.activation(
                            out=vap(wgt[:], 0, [[pw, 128], [1, S * HEADS * D]]),
                            in_=vap(wgt[:], 0, [[pw, 128], [1, S * HEADS * D]]),
                            func=ACTF.Exp, scale=scale)
                        nc.vector.tensor_reduce(
                            out=u_acc[:, sl:sl + S, 16:24],
                            in_=vap(wgt[:], 0,
                                    [[pw, 128], [D, S * HEADS], [1, D]]),
                            axis=AX.X, op=OP.add)
                        msg = wpool.tile([128, S * HIDDEN, D],
                                         dt.float32, tag="prod", name="msg")
                        pm = msg[:].ap[0][0]
                        nc.vector.tensor_tensor(
                            out=vap(msg[:], 0,
                                    [[pm, 128], [HIDDEN * D, S], [HEAD_DIM * D, HEADS],
                                     [D, HEAD_DIM], [1, D]]),
                            in0=vap(wgt[:], 0,
                                    [[pw, 128], [HEADS * D, S], [D, HEADS],
                                     [0, HEAD_DIM], [1, D]]),
                            in1=vap(kvga, 16,
                                    [[pk, 128], [D * KV_COLS, S], [HEAD_DIM, HEADS],
                                     [1, HEAD_DIM], [KV_COLS, D]]),
                            op=OP.mult)
                        nc.vector.tensor_reduce(
                            out=u_acc[:, sl:sl + S, 0:16],
                            in_=vap(msg[:], 0,
                                    [[pm, 128], [D, S * HIDDEN], [1, D]]),
                            axis=AX.X, op=OP.add)
                    # u_acc (rank order) -> DRAM -> gather into identity order
                    nc.sync.dma_start(
                        out=bass.AP(u_tab[t], 0,
                                    [[KV_COLS, 128], [128 * KV_COLS, n_tiles],
                                     [1, 24]]),
                        in_=u_acc[:])
                    ct = spool.tile([128, npp // 16], dt.int16, tag="ct", name="ct")
                    nc.sync.dma_start(out=ct[:], in_=cidx_d[t])
                    done = 0
                    while done < n_tiles:
                        cnt = min(8, n_tiles - done)
                        cg = spool.tile([128, 8, KV_COLS], dt.float32, tag="qg",
                                        name="cg")
                        gather(cg[:, 0:cnt, :], u_tab[t][:],
                               ct[:, done * 8:(done + cnt) * 8], cnt * 128)
                        nc.vector.tensor_tensor(
                            out=u_tot[:, done:done + cnt, :],
                            in0=u_tot[:, done:done + cnt, :],
                            in1=cg[:, 0:cnt, 0:24], op=OP.add)
                        done += cnt

                # epilogue
                dadj = spool.tile([128, n_tiles, HEADS], dt.float32, tag="dadj",
                                  name="dadj")
                pn = npad_t[:].ap[0][0]
                nc.vector.tensor_tensor(
                    out=dadj[:], in0=u_tot[:, :, 16:24],
                    in1=vap(npad_t[:], 0, [[pn, 128], [1, n_tiles], [0, HEADS]]),
                    op=OP.subtract)
                nc.vector.tensor_scalar_add(out=dadj[:], in0=dadj[:], scalar1=1e-9)
                rden = spool.tile([128, n_tiles, HEADS], dt.float32, tag="rden",
                                  name="rden")
                nc.vector.reciprocal(out=rden[:], in_=dadj[:])
                agg = wpool.tile([128, n_tiles, HIDDEN], dt.float32, tag="wgt",
                                 name="agg")
                pr_ = rden[:].ap[0][0]
                pa = agg[:].ap[0][0]
                nc.vector.tensor_tensor(
                    out=vap(agg[:], 0,
                            [[pa, 128], [HIDDEN, n_tiles], [HEAD_DIM, HEADS],
                             [1, HEAD_DIM]]),
                    in0=vap(u_tot[:], 0,
                            [[u_tot[:].ap[0][0], 128], [24, n_tiles],
                             [HEAD_DIM, HEADS], [1, HEAD_DIM]]),
                    in1=vap(rden[:], 0,
                            [[pr_, 128], [HEADS, n_tiles], [1, HEADS],
                             [0, HEAD_DIM]]),
                    op=OP.mult)
                hnew = wpool.tile([128, n_tiles, HIDDEN], dt.float32, tag="prod",
                                  name="hnew")
                for k in range(n_tiles):
                    tp = ppool.tile([HIDDEN, 128], dt.float32, tag="tp", name="atp")
                    nc.tensor.transpose(out=tp[:], in_=agg[:, k, :],
                                        identity=ident[:])
                    aT = spool.tile([HIDDEN, 128], dt.float32, tag="hT", name="aT")
                    nc.vector.tensor_copy(out=aT[:], in_=tp[:])
                    op_ = ppool.tile([128, HIDDEN], dt.float32, tag="mm", name="op_")
                    nc.tensor.matmul(out=op_[:], lhsT=aT[:],
                                     rhs=wmat[("wo", layer)][:],
                                     start=True, stop=True)
                    nc.vector.tensor_tensor(out=hnew[:, k, :], in0=op_[:],
                                            in1=h_all[:, k, :], op=OP.add)
                mu = spool.tile([128, n_tiles, 1], dt.float32, tag="mu", name="mu")
                nc.vector.tensor_reduce(out=mu[:], in_=hnew[:], axis=AX.X, op=OP.add)
                nc.vector.tensor_scalar_mul(out=mu[:], in0=mu[:], scalar1=1.0 / 16)
                cent = wpool.tile([128, n_tiles, HIDDEN], dt.float32, tag="wgt",
                                  name="cent")
                nc.vector.tensor_tensor(
                    out=cent[:], in0=hnew[:],
                    in1=vap(mu[:], 0, [[mu[:].ap[0][0], 128], [1, n_tiles],
                                       [0, HIDDEN]]),
                    op=OP.subtract)
                sq = wpool.tile([128, n_tiles, HIDDEN], dt.float32, tag="prod",
                                name="sq")
                nc.vector.tensor_tensor(out=sq[:], in0=cent[:], in1=cent[:],
                                        op=OP.mult)
                var = spool.tile([128, n_tiles, 1], dt.float32, tag="var", name="var")
                nc.vector.tensor_reduce(out=var[:], in_=sq[:], axis=AX.X, op=OP.add)
                nc.vector.tensor_scalar_mul(out=var[:], in0=var[:], scalar1=1.0 / 16)
                nc.vector.tensor_scalar_add(out=var[:], in0=var[:], scalar1=1e-5)
                rs = spool.tile([128, n_tiles, 1], dt.float32, tag="rs", name="rs")
                nc.vector.reciprocal(out=rs[:], in_=var[:])
                nc.scalar.activation(out=rs[:], in_=rs[:], func=ACTF.Sqrt)
                nc.vector.tensor_tensor(
                    out=h_all[:], in0=cent[:],
                    in1=vap(rs[:], 0, [[rs[:].ap[0][0], 128], [1, n_tiles],
                                       [0, HIDDEN]]),
                    op=OP.mult)

            # final head
            nc.vector.memset(acc4[:], 0.0)
            for k in range(n_tiles):
                tp = ppool.tile([HIDDEN, 128], dt.float32, tag="tp", name="ftp")
                nc.tensor.transpose(out=tp[:], in_=h_all[:, k, :], identity=ident[:])
                hT = spool.tile([HIDDEN, 128], dt.float32, tag="hT", name="fhT")
                nc.vector.tensor_copy(out=hT[:], in_=tp[:])
                gp = ppool.tile([128, DIM], dt.float32, tag="mm", name="gp")
                nc.tensor.matmul(out=gp[:], lhsT=hT[:], rhs=wout_t[:],
                                 start=True, stop=True)
                nc.vector.tensor_tensor(out=acc4[:], in0=acc4[:], in1=gp[:],
                                        op=OP.add)
            onesk = cpool.tile([128, 1], dt.float32, name="onesk")
            nc.vector.memset(onesk[:], 1.0 / cfg["n_nodes"])
            pooled_p = ppool.tile([1, 4], dt.float32, tag="mm", name="pooled_p")
            nc.tensor.matmul(out=pooled_p[:], lhsT=onesk[:], rhs=acc4[:],
                             start=True, stop=True)
            pooled_s = spool.tile([1, 4], dt.float32, tag="p4", name="pooled_s")
            nc.vector.tensor_copy(out=pooled_s[:], in_=pooled_p[:])
            nc.sync.dma_start(out=ar_in[:], in_=pooled_s[:])
            nc.gpsimd.collective_compute(
                "AllReduce", mybir.AluOpType.add,
                replica_groups=[list(range(cfg["n_cores"]))],
                ins=[ar_in[:]], outs=[ar_out[:]])
            pooled = spool.tile([1, 4], dt.float32, tag="p4b", name="pooled")
            nc.sync.dma_start(out=pooled[:], in_=ar_out[:])
            fcw_t = spool.tile([1, 12], dt.float32, tag="fcw", name="fcw_t")
            nc.sync.dma_start(out=fcw_t[:], in_=fcw_d[:])
            fcb_t = spool.tile([1, 3], dt.float32, tag="fcb", name="fcb_t")
            nc.sync.dma_start(out=fcb_t[:], in_=fcb_d[:])
            pr2 = spool.tile([1, 12], dt.float32, tag="pr2", name="pr2")
            nc.vector.tensor_tensor(
                out=pr2[:],
                in0=vap(pooled[:], 0, [[pooled[:].ap[0][0], 1], [0, 3], [1, 4]]),
                in1=vap(fcw_t[:], 0, [[fcw_t[:].ap[0][0], 1], [4, 3], [1, 4]]),
                op=OP.mult)
            y3 = spool.tile([1, 3], dt.float32, tag="y3", name="y3")
            nc.vector.tensor_reduce(
                out=y3[:],
                in_=vap(pr2[:], 0, [[pr2[:].ap[0][0], 1], [4, 3], [1, 4]]),
                axis=AX.X, op=OP.add)
            nc.vector.tensor_tensor(out=y3[:], in0=y3[:], in1=fcb_t[:], op=OP.add)
            nc.sync.dma_start(out=y_out[:], in_=y3[:])

    _split_excess_waits(nc, max_waits=1)
    lower_extended_insts(nc)
    return nc


def kernel(x, edge_index, W_in, Wq, Wk, Wv, Wo, W_out, fc_w, fc_b):
    x = np.asarray(x, dtype=np.float32)
    edge_index = np.asarray(edge_index)
    cfg = _cfg(x.shape[0])

    key = ("nc", x.shape[0], edge_index.shape[1])
    if key not in _RUN_CACHE:
        schedule, plans = _plan(edge_index, cfg)
        meta = (plans[0]["gidx"].shape[1], plans[0]["qidx"].shape[1],
                plans[0]["chunk_meta"])
        nc = _build(meta, cfg)
        _RUN_CACHE[key] = (nc, plans)
    nc, plans = _RUN_CACHE[key]

    wts = _permute_weights(
        np.asarray(W_in, np.float32), np.asarray(Wq, np.float32),
        np.asarray(Wk, np.float32), np.asarray(Wv, np.float32),
        np.asarray(Wo, np.float32), np.asarray(W_out, np.float32),
        np.asarray(fc_w, np.float32), np.asarray(fc_b, np.float32))

    nps, pcs, npp = cfg["n_per_sub"], cfg["per_core_sub"], cfg["npp"]
    old = np.arange(cfg["n_nodes"])
    c_of = (old % nps) // pcs
    l_of = (old // nps) * pcs + (old % pcs)
    in_maps = []
    for c in range(cfg["n_cores"]):
        xl = np.zeros((npp, DIM), dtype=np.float32)
        m = c_of == c
        xl[l_of[m]] = x[m]
        p = plans[c]
        in_maps.append(dict(
            x_in=xl, gidx=p["gidx"], qidx=p["qidx"], cidx=p["cidx"],
            npad=p["npad"], w_in=wts["W_in"], wq=wts["Wq"], wk=wts["Wk"],
            wv=wts["Wv"], wo=wts["Wo"], wout=wts["W_out"], fcw=wts["fcw"],
            fcb=wts["fcb"]))

    from concourse.bass_utils import run_bass_kernel_spmd
    trace = bool(os.environ.get("GNN_TRACE"))
    if trace:
        _install_profhook()
    res = run_bass_kernel_spmd(nc, in_maps, core_ids=list(range(cfg["n_cores"])),
                               trace=trace)
    if trace:
        _RUN_CACHE["last_result"] = res
    return np.asarray(res.results[0]["y"]).reshape(3).astype(np.float32)



# revision 8
# speedup vs baseline: 1.0477x; 1.0477x over previous
"""GNN message-passing (SE3-style graph attention) kernel for 8 Trainium2 cores.

Edge-parallel strategy:
- Nodes relabeled into 4 "subtables" x 8 cores so per-edge kv-gather indices
  fit int16 (dma_gather requirement). Each core owns 12500 dst nodes.
- Per layer: on-device q/k/v projections -> 4 AllGathers build global kv
  tables -> 4 passes over src subtables, each processing edges in node-major
  degree-sorted chunks: dma_gather kv rows, affine q broadcast, DVE
  scores/exp/messages, affine segment reduction into per-pass accumulators.
- Pass accumulators (degree-rank order) recombined into identity order by
  small dma_gathers, then divide / Wo project / residual / LayerNorm.
- Final: W_out, mean-pool via PE ones-matmul, AllReduce, FC head.
"""

import os
import sys
import types
import numpy as np

HEADS = 8
HEAD_DIM = 2
HIDDEN = 16
DIM = 4
DEPTH = 2
N_SUB = 4
KV_COLS = 64          # table row = 64 fp32 = 256B
SENT = 32767          # zeroed sentinel row in each kv subtable
SD_CAP = 128          # max S*D per chunk
S_CAP = 8

_RUN_CACHE = {}


# --------------------------------------------------------------------------
# harness shims (self-contained copies)
# --------------------------------------------------------------------------
def _split_excess_waits(nc, max_waits=1):
    """Walrus build allows 1 sync-wait per instruction; move extras to NOPs."""
    import concourse.mybir as mybir
    n = [0]
    for blk in nc.m.functions[0].blocks:
        new_insts = []
        for inst in blk.instructions:
            si = inst.sync_info
            if si is not None and len(si.on_wait) > max_waits:
                waits = list(si.on_wait)
                extra, keep = waits[:-max_waits], waits[-max_waits:]
                for i in range(0, len(extra), max_waits):
                    n[0] += 1
                    nop = mybir.InstNoOp(
                        name=f"IWS-{n[0]}", engine=inst.engine, ins=[], outs=[],
                        sync_info=mybir.SyncInfo(on_wait=extra[i:i + max_waits],
                                                 on_update=[]))
                    try:
                        nc.register_instruction(nop, overwrite=True)
                    except Exception:
                        pass
                    new_insts.append(nop)
                si.on_wait = keep
            new_insts.append(inst)
        blk.instructions[:] = new_insts


def _install_profhook():
    if 'antenv.axon_hooks' in sys.modules:
        return
    try:
        import antenv
        from trn_agent_boot.trn_boot import _ntff_profile_via_ctypes
        hook = _ntff_profile_via_ctypes('/opt/axon/libaxon_pjrt.so')
        mod = types.ModuleType('antenv.axon_hooks')
        state = {'hook': hook}
        mod.set_axon_ntff_profile_hook = lambda h: state.__setitem__('hook', h)
        mod.get_axon_ntff_profile_hook = lambda: state['hook']
        sys.modules['antenv.axon_hooks'] = mod
        antenv.axon_hooks = mod
    except Exception:
        pass


# --------------------------------------------------------------------------
# host-side planning
# --------------------------------------------------------------------------
def _cfg(n_nodes):
    n_cores = 8
    n_per_sub = n_nodes // N_SUB
    per_core_sub = n_per_sub // n_cores
    npc = N_SUB * per_core_sub
    npp = ((npc + 127) // 128) * 128
    return dict(n_nodes=n_nodes, n_cores=n_cores, n_per_sub=n_per_sub,
                per_core_sub=per_core_sub, npc=npc, npp=npp,
                n_tiles=npp // 128)


def _pack_gidx(idx_flat):
    """Gather feed position i lives at tile[i%16, i//16]; replicate x8 cores."""
    n = idx_flat.shape[0]
    assert n % 16 == 0
    tile16 = np.ascontiguousarray(
        idx_flat.reshape(n // 16, 16).T.astype(np.int16))
    return np.tile(tile16, (8, 1))


def _plan(edge_index, cfg):
    src = np.asarray(edge_index[0], dtype=np.int64)
    dst = np.asarray(edge_index[1], dtype=np.int64)
    nps, pcs = cfg["n_per_sub"], cfg["per_core_sub"]
    npp, n_cores = cfg["npp"], cfg["n_cores"]

    e_core = (dst % nps) // pcs
    e_l = (dst // nps) * pcs + (dst % pcs)
    e_t = src // nps
    e_row = src % nps

    passes = []
    for t in range(N_SUB):
        per_core = []
        for c in range(n_cores):
            m = (e_core == c) & (e_t == t)
            lt, rowt = e_l[m], e_row[m]
            deg = np.bincount(lt, minlength=npp).astype(np.int64)
            order = np.argsort(-deg, kind="stable")
            rank_of = np.empty(npp, dtype=np.int64)
            rank_of[order] = np.arange(npp)
            eorder = np.argsort(rank_of[lt], kind="stable")
            per_core.append(dict(deg=deg, order=order, rank_of=rank_of,
                                 lt=lt[eorder], rowt=rowt[eorder]))
        passes.append(per_core)

    schedule = []
    for t in range(N_SUB):
        chunks = []
        r0 = 0
        degs_sorted = [passes[t][c]["deg"][passes[t][c]["order"]]
                       for c in range(n_cores)]
        while r0 < npp:
            D = int(max(int(d[r0]) for d in degs_sorted))
            if D == 0:
                break
            S = max(1, min(S_CAP, SD_CAP // D, (npp - r0) // 128))
            chunks.append((r0, S, D))
            r0 += 128 * S
        schedule.append(chunks)

    plans = []
    for c in range(n_cores):
        gidx_cols, qidx_list, chunk_meta = [], [], []
        npad = np.zeros(npp, dtype=np.float64)
        gcol0 = qcol0 = 0
        for t in range(N_SUB):
            pc = passes[t][c]
            deg, order, rank_of = pc["deg"], pc["order"], pc["rank_of"]
            lt, rowt = pc["lt"], pc["rowt"]
            offs = np.zeros(npp + 1, dtype=np.int64)
            offs[1:] = np.cumsum(deg[order])
            ranks_e = rank_of[lt]
            j_e = np.arange(lt.shape[0]) - offs[ranks_e]
            for ci, (r0, S, D) in enumerate(schedule[t]):
                nrows = 128 * S * D
                idx_flat = np.full(nrows, SENT, dtype=np.int64)
                em = (ranks_e >= r0) & (ranks_e < r0 + 128 * S)
                q = ranks_e[em] - r0
                pos = ((q // 128) * D + j_e[em]) * 128 + (q % 128)
                idx_flat[pos] = rowt[em]
                gidx_cols.append(_pack_gidx(idx_flat))
                qidx_list.append(_pack_gidx(order[r0:r0 + 128 * S]))
                ch_nodes = order[r0:r0 + 128 * S]
                npad[ch_nodes] += D - deg[ch_nodes]
                chunk_meta.append((t, r0, S, D, gcol0, nrows // 16, qcol0))
                gcol0 += nrows // 16
                qcol0 += 8 * S
        gidx = (np.concatenate(gidx_cols, axis=1) if gidx_cols
                else np.zeros((128, 16), np.int16))
        qidx = (np.concatenate(qidx_list, axis=1) if qidx_list
                else np.zeros((128, 8), np.int16))
        cidx = np.stack([_pack_gidx(passes[t][c]["rank_of"][:npp])
                         for t in range(N_SUB)])
        npad_t = np.ascontiguousarray(
            npad.reshape(cfg["n_tiles"], 128).T.astype(np.float32))
        plans.append(dict(gidx=gidx, qidx=qidx, cidx=cidx, npad=npad_t,
                          chunk_meta=chunk_meta))
    return schedule, plans


def _permute_weights(W_in, Wq, Wk, Wv, Wo, W_out, fc_w, fc_b):
    perm = np.array([h * HEAD_DIM + d for d in range(HEAD_DIM)
                     for h in range(HEADS)], dtype=np.int64)
    return dict(W_in=np.ascontiguousarray(W_in),
                Wq=np.ascontiguousarray(Wq[:, :, perm]),
                Wk=np.ascontiguousarray(Wk[:, :, perm]),
                Wv=np.ascontiguousarray(Wv),
                Wo=np.ascontiguousarray(Wo),
                W_out=np.ascontiguousarray(W_out),
                fcw=np.ascontiguousarray(fc_w.T).reshape(1, 12).astype(np.float32),
                fcb=fc_b.reshape(1, 3).astype(np.float32))


# --------------------------------------------------------------------------
# device program
# --------------------------------------------------------------------------
def _build(meta, cfg):
    import concourse.bass as bass
    import concourse.mybir as mybir
    import concourse.tile as tile
    from concourse import library_config
    from concourse.masks import make_identity
    from concourse.library_overlay import lower_extended_insts

    dt = mybir.dt
    AX = mybir.AxisListType
    OP = mybir.AluOpType
    ACTF = mybir.ActivationFunctionType
    npp, n_tiles, npc = cfg["npp"], cfg["n_tiles"], cfg["npc"]
    pcs = cfg["per_core_sub"]
    g_cols_total, q_cols_total, chunk_meta = meta

    nc = bass.Bass(num_devices=cfg["n_cores"], num_swdge_queues=4)

    x_in = nc.dram_tensor("x_in", [npp, DIM], dt.float32, kind="ExternalInput")
    gidx_d = nc.dram_tensor("gidx", [128, g_cols_total], dt.int16, kind="ExternalInput")
    qidx_d = nc.dram_tensor("qidx", [128, q_cols_total], dt.int16, kind="ExternalInput")
    cidx_d = nc.dram_tensor("cidx", [N_SUB, 128, npp // 16], dt.int16, kind="ExternalInput")
    npad_d = nc.dram_tensor("npad", [128, n_tiles], dt.float32, kind="ExternalInput")
    w_in_d = nc.dram_tensor("w_in", [DIM, HIDDEN], dt.float32, kind="ExternalInput")
    wq_d = nc.dram_tensor("wq", [DEPTH, HIDDEN, HIDDEN], dt.float32, kind="ExternalInput")
    wk_d = nc.dram_tensor("wk", [DEPTH, HIDDEN, HIDDEN], dt.float32, kind="ExternalInput")
    wv_d = nc.dram_tensor("wv", [DEPTH, HIDDEN, HIDDEN], dt.float32, kind="ExternalInput")
    wo_d = nc.dram_tensor("wo", [DEPTH, HIDDEN, HIDDEN], dt.float32, kind="ExternalInput")
    wout_d = nc.dram_tensor("wout", [HIDDEN, DIM], dt.float32, kind="ExternalInput")
    fcw_d = nc.dram_tensor("fcw", [1, 12], dt.float32, kind="ExternalInput")
    fcb_d = nc.dram_tensor("fcb", [1, 3], dt.float32, kind="ExternalInput")
    y_out = nc.dram_tensor("y", [1, 3], dt.float32, kind="ExternalOutput")

    kv_tab = [nc.dram_tensor(f"kvtab{t}", [32768, KV_COLS], dt.float32,
                             kind="Internal", addr_space="Shared")
              for t in range(N_SUB)]
    q_tab = nc.dram_tensor("qtab", [npp, KV_COLS], dt.float32, kind="Internal")
    u_tab = [nc.dram_tensor(f"utab{t}", [npp, KV_COLS], dt.float32, kind="Internal")
             for t in range(N_SUB)]
    stage_d = nc.dram_tensor("stage", [npc, KV_COLS], dt.float32, kind="Internal")
    ar_in = nc.dram_tensor("ar_in", [1, 4], dt.float32, kind="Internal")
    ar_out = nc.dram_tensor("ar_out", [1, 4], dt.float32, kind="Internal",
                            addr_space="Shared")

    nc.gpsimd.load_library(library_config.attnmlp)
    gq = [0]
    _nregs = {}

    def gather(out_ap, in_tensor, nrows, idx_ap, num_idxs, elem_size):
        """Raw InstDMAGatherAnt reading elem_size f32 from 256B-pitch rows.

        Bypasses bass's elem_size_bytes%256 assert: the ucode only requires
        the row *pitch* (elem_step bytes) to be a 256B multiple."""
        if num_idxs not in _nregs:
            _nregs[num_idxs] = nc.gpsimd.to_reg(num_idxs)
        g = nc.gpsimd
        in_ap = bass.AP(in_tensor, 0, [[KV_COLS, nrows], [1, elem_size]])
        _in_ap = g.lower_ap_dma(in_ap, for_custom_bir_dma=True)
        _idxs_ap = g.lower_ap(idx_ap)
        _out_ap = g.lower_ap(out_ap)
        g.add_instruction(mybir.InstDMAGatherAnt(
            name=nc.get_next_instruction_name(),
            ins=[*_in_ap, _idxs_ap, g.lower_val_access(_nregs[num_idxs])],
            outs=[_out_ap],
            transpose=False, num_idxs=num_idxs, elem_size=elem_size,
            stride_bytes_256=1, gen_mode=0, single_packet=False,
            queue_num=gq[0] % 4,
            sbuf_tokens_per_rank=0, sbuf_free_dim_per_rank=0,
            sbuf_free_dim_pad_per_rank=0, sbuf_byte_offset=0))
        gq[0] += 1

    def vap(base_ap, extra_off, dims):
        return bass.AP(base_ap.tensor, base_ap.offset + extra_off, dims)

    with tile.TileContext(nc) as tc:
        with (
            tc.tile_pool(name="const", bufs=1) as cpool,
            tc.tile_pool(name="res", bufs=1) as rpool,
            tc.tile_pool(name="work", bufs=2) as wpool,
            tc.tile_pool(name="gath", bufs=4) as gpool,
            tc.tile_pool(name="small", bufs=2) as spool,
            tc.tile_pool(name="psum", bufs=4, space="PSUM") as ppool,
        ):
            ident = cpool.tile([128, 128], dt.float32, name="ident")
            make_identity(nc, ident[:])
            w_in_t = cpool.tile([DIM, HIDDEN], dt.float32, name="w_in_t")
            nc.sync.dma_start(out=w_in_t[:], in_=w_in_d[:])
            wmat = {}
            for nm, dd in (("wq", wq_d), ("wk", wk_d), ("wv", wv_d), ("wo", wo_d)):
                for l in range(DEPTH):
                    w = cpool.tile([HIDDEN, HIDDEN], dt.float32, name=f"{nm}{l}")
                    nc.sync.dma_start(out=w[:], in_=dd[l])
                    wmat[(nm, l)] = w
            wout_t = cpool.tile([HIDDEN, DIM], dt.float32, name="wout_t")
            nc.sync.dma_start(out=wout_t[:], in_=wout_d[:])
            npad_t = cpool.tile([128, n_tiles], dt.float32, name="npad_t")
            nc.sync.dma_start(out=npad_t[:], in_=npad_d[:])

            zrow = cpool.tile([1, KV_COLS], dt.float32, name="zrow")
            nc.vector.memset(zrow[:], 0.0)
            for t in range(N_SUB):
                nc.sync.dma_start(out=kv_tab[t][SENT:SENT + 1, :], in_=zrow[:])

            h_all = rpool.tile([128, n_tiles, HIDDEN], dt.float32, name="h_all")
            u_tot = rpool.tile([128, n_tiles, 24], dt.float32, name="u_tot")
            u_acc2 = [rpool.tile([128, n_tiles, 24], dt.float32, name=f"u_acc{i}")
                      for i in range(2)]
            acc4 = rpool.tile([128, 4], dt.float32, name="acc4")

            # stage 0: h0 = x @ W_in
            xs = wpool.tile([128, n_tiles, DIM], dt.float32, tag="gt", name="xs")
            nc.sync.dma_start(out=xs[:],
                              in_=x_in[:].rearrange("(a p) f -> p a f", p=128))
            for k in range(n_tiles):
                tp = ppool.tile([DIM, 128], dt.float32, tag="tp", name="tp")
                nc.tensor.transpose(out=tp[:], in_=xs[:, k, :], identity=ident[:])
                tps = spool.tile([HIDDEN, 128], dt.float32, tag="hT", name="tps")
                nc.vector.tensor_copy(out=tps[0:DIM, :], in_=tp[:])
                hp = ppool.tile([128, HIDDEN], dt.float32, tag="mm", name="hp")
                nc.tensor.matmul(out=hp[:], lhsT=tps[0:DIM, :], rhs=w_in_t[:],
                                 start=True, stop=True)
                nc.vector.tensor_copy(out=h_all[:, k, :], in_=hp[:])

            scale = float(1.0 / np.sqrt(HEAD_DIM))

            for layer in range(DEPTH):
                # k/v projections + stage; AllGather[t] issued as soon as
                # subtable t's stage rows are complete
                ag_after = {}
                for t in range(N_SUB):
                    ag_after[(((t + 1) * pcs + 127) // 128) - 1] = t
                for k in range(n_tiles):
                    tp = ppool.tile([HIDDEN, 128], dt.float32, tag="tp", name="htp")
                    nc.tensor.transpose(out=tp[:], in_=h_all[:, k, :],
                                        identity=ident[:])
                    hT = spool.tile([HIDDEN, 128], dt.float32, tag="hT", name="hT")
                    nc.vector.tensor_copy(out=hT[:], in_=tp[:])
                    kp = ppool.tile([128, HIDDEN], dt.float32, tag="mm", name="kp")
                    nc.tensor.matmul(out=kp[:], lhsT=hT[:], rhs=wmat[("wk", layer)][:],
                                     start=True, stop=True)
                    kvs = spool.tile([128, 32], dt.float32, tag="kvs", name="kvs")
                    nc.vector.tensor_copy(out=kvs[:, 0:16], in_=kp[:])
                    vp = ppool.tile([128, HIDDEN], dt.float32, tag="mm", name="vp")
                    nc.tensor.matmul(out=vp[:], lhsT=hT[:], rhs=wmat[("wv", layer)][:],
                                     start=True, stop=True)
                    nc.vector.tensor_copy(out=kvs[:, 16:32], in_=vp[:])
                    lo, hi = k * 128, min((k + 1) * 128, npc)
                    if lo < npc:
                        nc.sync.dma_start(out=stage_d[lo:hi, 0:32],
                                          in_=kvs[0:hi - lo, :])
                    t = ag_after.get(k)
                    if t is not None:
                        nc.gpsimd.collective_compute(
                            "AllGather", mybir.AluOpType.bypass,
                            replica_groups=[list(range(cfg["n_cores"]))],
                            ins=[stage_d[t * pcs:(t + 1) * pcs, :]],
                            outs=[kv_tab[t][0:cfg["n_cores"] * pcs, :]])
                # q projection overlaps the AllGather flight
                for k in range(n_tiles):
                    tp = ppool.tile([HIDDEN, 128], dt.float32, tag="tp", name="qtp")
                    nc.tensor.transpose(out=tp[:], in_=h_all[:, k, :],
                                        identity=ident[:])
                    hT = spool.tile([HIDDEN, 128], dt.float32, tag="hT", name="qhT")
                    nc.vector.tensor_copy(out=hT[:], in_=tp[:])
                    qp = ppool.tile([128, HIDDEN], dt.float32, tag="mm", name="qp")
                    nc.tensor.matmul(out=qp[:], lhsT=hT[:], rhs=wmat[("wq", layer)][:],
                                     start=True, stop=True)
                    qs = spool.tile([128, HIDDEN], dt.float32, tag="qs", name="qs")
                    nc.vector.tensor_copy(out=qs[:], in_=qp[:])
                    nc.sync.dma_start(out=q_tab[k * 128:(k + 1) * 128, 0:HIDDEN],
                                      in_=qs[:])

                nc.vector.memset(u_tot[:], 0.0)
                for t in range(N_SUB):
                    u_acc = u_acc2[t % 2]
                    nc.vector.memset(u_acc[:], 0.0)
                    for (tt, r0, S, D, gc0, gcols, qc0) in chunk_meta:
                        if tt != t:
                            continue
                        sl = r0 // 128
                        gt = wpool.tile([128, 1024], dt.int16, tag="gt", name="gt")
                        nc.sync.dma_start(out=gt[:, 0:gcols],
                                          in_=gidx_d[:, gc0:gc0 + gcols])
                        qt = spool.tile([128, 64], dt.int16, tag="qt", name="qt")
                        nc.sync.dma_start(out=qt[:, 0:8 * S],
                                          in_=qidx_d[:, qc0:qc0 + 8 * S])
                        kvg = gpool.tile([128, SD_CAP, 32], dt.float32,
                                         tag="kvg", name="kvg")
                        nrow = S * D
                        done = 0
                        while done < nrow:
                            cnt = min(8, nrow - done)
                            gather(kvg[:, done:done + cnt, :], kv_tab[t], 32768,
                                   gt[:, done * 8:(done + cnt) * 8], cnt * 128, 32)
                            done += cnt
                        qg = spool.tile([128, S_CAP, HIDDEN], dt.float32,
                                        tag="qg", name="qg")
                        gather(qg[:, 0:S, :], q_tab, npp,
                               qt[:, 0:8 * S], 128 * S, HIDDEN)
                        kvga, qga = kvg[:], qg[:]
                        pk = kvga.ap[0][0]
                        pq = qga.ap[0][0]
                        prod = wpool.tile([128, S * HIDDEN, D],
                                          dt.float32, tag="prod", name="prod")
                        pp = prod[:].ap[0][0]
                        nc.vector.tensor_tensor(
                            out=vap(prod[:], 0,
                                    [[pp, 128], [HIDDEN * D, S], [D, HIDDEN], [1, D]]),
                            in0=vap(qga, 0,
                                    [[pq, 128], [HIDDEN, S], [1, HIDDEN], [0, D]]),
                            in1=vap(kvga, 0,
                                    [[pk, 128], [D * 32, S], [1, HIDDEN],
                                     [32, D]]),
                            op=OP.mult)
                        wgt = wpool.tile([128, S * HEADS, D],
                                         dt.float32, tag="wgt", name="wgt")
                        pw = wgt[:].ap[0][0]
                        nc.vector.tensor_tensor(
                            out=vap(wgt[:], 0,
                                    [[pw, 128], [HEADS * D, S], [D, HEADS], [1, D]]),
                            in0=vap(prod[:], 0,
                                    [[pp, 128], [HIDDEN * D, S], [D, HEADS], [1, D]]),
                            in1=vap(prod[:], HEADS * D,
                                    [[pp, 128], [HIDDEN * D, S], [D, HEADS], [1, D]]),
                            op=OP.add)
                        nc.scalar.activation(
                            out=vap(wgt[:], 0, [[pw, 128], [1, S * HEADS * D]]),
                            in_=vap(wgt[:], 0, [[pw, 128], [1, S * HEADS * D]]),
                            func=ACTF.Exp, scale=scale)
                        nc.vector.tensor_reduce(
                            out=u_acc[:, sl:sl + S, 16:24],
                            in_=vap(wgt[:], 0,
                                    [[pw, 128], [D, S * HEADS], [1, D]]),
                            axis=AX.X, op=OP.add)
                        msg = wpool.tile([128, S * HIDDEN, D],
                                         dt.float32, tag="prod", name="msg")
                        pm = msg[:].ap[0][0]
                        nc.vector.tensor_tensor(
                            out=vap(msg[:], 0,
                                    [[pm, 128], [HIDDEN * D, S], [HEAD_DIM * D, HEADS],
                                     [D, HEAD_DIM], [1, D]]),
                            in0=vap(wgt[:], 0,
                                    [[pw, 128], [HEADS * D, S], [D, HEADS],
                                     [0, HEAD_DIM], [1, D]]),
                            in1=vap(kvga, 16,
                                    [[pk, 128], [D * 32, S], [HEAD_DIM, HEADS],
                                     [1, HEAD_DIM], [32, D]]),
                            op=OP.mult)
                        nc.vector.tensor_reduce(
                            out=u_acc[:, sl:sl + S, 0:16],
                            in_=vap(msg[:], 0,
                                    [[pm, 128], [D, S * HIDDEN], [1, D]]),
                            axis=AX.X, op=OP.add)
                    # u_acc (rank order) -> DRAM -> gather into identity order
                    nc.sync.dma_start(
                        out=bass.AP(u_tab[t], 0,
                                    [[KV_COLS, 128], [128 * KV_COLS, n_tiles],
                                     [1, 24]]),
                        in_=u_acc[:])
                    ct = spool.tile([128, npp // 16], dt.int16, tag="ct", name="ct")
                    nc.sync.dma_start(out=ct[:], in_=cidx_d[t])
                    done = 0
                    while done < n_tiles:
                        cnt = min(8, n_tiles - done)
                        cg = spool.tile([128, 8, 24], dt.float32, tag="cgt",
                                        name="cg")
                        gather(cg[:, 0:cnt, :], u_tab[t], npp,
                               ct[:, done * 8:(done + cnt) * 8], cnt * 128, 24)
                        nc.vector.tensor_tensor(
                            out=u_tot[:, done:done + cnt, :],
                            in0=u_tot[:, done:done + cnt, :],
                            in1=cg[:, 0:cnt, :], op=OP.add)
                        done += cnt

                # epilogue
                dadj = spool.tile([128, n_tiles, HEADS], dt.float32, tag="dadj",
                                  name="dadj")
                pn = npad_t[:].ap[0][0]
                nc.vector.tensor_tensor(
                    out=dadj[:], in0=u_tot[:, :, 16:24],
                    in1=vap(npad_t[:], 0, [[pn, 128], [1, n_tiles], [0, HEADS]]),
                    op=OP.subtract)
                nc.vector.tensor_scalar_add(out=dadj[:], in0=dadj[:], scalar1=1e-9)
                rden = spool.tile([128, n_tiles, HEADS], dt.float32, tag="rden",
                                  name="rden")
                nc.vector.reciprocal(out=rden[:], in_=dadj[:])
                agg = wpool.tile([128, n_tiles, HIDDEN], dt.float32, tag="wgt",
                                 name="agg")
                pr_ = rden[:].ap[0][0]
                pa = agg[:].ap[0][0]
                nc.vector.tensor_tensor(
                    out=vap(agg[:], 0,
                            [[pa, 128], [HIDDEN, n_tiles], [HEAD_DIM, HEADS],
                             [1, HEAD_DIM]]),
                    in0=vap(u_tot[:], 0,
                            [[u_tot[:].ap[0][0], 128], [24, n_tiles],
                             [HEAD_DIM, HEADS], [1, HEAD_DIM]]),
                    in1=vap(rden[:], 0,
                            [[pr_, 128], [HEADS, n_tiles], [1, HEADS],
                             [0, HEAD_DIM]]),
                    op=OP.mult)
                hnew = wpool.tile([128, n_tiles, HIDDEN], dt.float32, tag="prod",
                                  name="hnew")
                for k in range(n_tiles):
                    tp = ppool.tile([HIDDEN, 128], dt.float32, tag="tp", name="atp")
                    nc.tensor.transpose(out=tp[:], in_=agg[:, k, :],
                                        identity=ident[:])
                    aT = spool.tile([HIDDEN, 128], dt.float32, tag="hT", name="aT")
                    nc.vector.tensor_copy(out=aT[:], in_=tp[:])
                    op_ = ppool.tile([128, HIDDEN], dt.float32, tag="mm", name="op_")
                    nc.tensor.matmul(out=op_[:], lhsT=aT[:],
                                     rhs=wmat[("wo", layer)][:],
                                     start=True, stop=True)
                    nc.vector.tensor_tensor(out=hnew[:, k, :], in0=op_[:],
                                            in1=h_all[:, k, :], op=OP.add)
                mu = spool.tile([128, n_tiles, 1], dt.float32, tag="mu", name="mu")
                nc.vector.tensor_reduce(out=mu[:], in_=hnew[:], axis=AX.X, op=OP.add)
                nc.vector.tensor_scalar_mul(out=mu[:], in0=mu[:], scalar1=1.0 / 16)
                cent = wpool.tile([128, n_tiles, HIDDEN], dt.float32, tag="wgt",
                                  name="cent")
                nc.vector.tensor_tensor(
                    out=cent[:], in0=hnew[:],
                    in1=vap(mu[:], 0, [[mu[:].ap[0][0], 128], [1, n_tiles],
                                       [0, HIDDEN]]),
                    op=OP.subtract)
                sq = wpool.tile([128, n_tiles, HIDDEN], dt.float32, tag="prod",
                                name="sq")
                nc.vector.tensor_tensor(out=sq[:], in0=cent[:], in1=cent[:],
                                        op=OP.mult)
                var = spool.tile([128, n_tiles, 1], dt.float32, tag="var", name="var")
                nc.vector.tensor_reduce(out=var[:], in_=sq[:], axis=AX.X, op=OP.add)
                nc.vector.tensor_scalar_mul(out=var[:], in0=var[:], scalar1=1.0 / 16)
                nc.vector.tensor_scalar_add(out=var[:], in0=var[:], scalar1=1e-5)
                rs = spool.tile([128, n_tiles, 1], dt.float32, tag="rs", name="rs")
                nc.vector.reciprocal(out=rs[:], in_=var[:])
                nc.scalar.activation(out=rs[:], in_=rs[:], func=ACTF.Sqrt)
                nc.vector.tensor_tensor(
                    out=h_all[:], in0=cent[:],
                    in1=vap(rs[:], 0, [[rs[:].ap[0][0], 128], [1, n_tiles],
                                       [0, HIDDEN]]),
                    op=OP.mult)

            # final head
            nc.vector.memset(acc4[:], 0.0)
            for k in range(n_tiles):
                tp = ppool.tile([HIDDEN, 128], dt.float32, tag="tp", name="ftp")
                nc.tensor.transpose(out=tp[:], in_=h_all[:, k, :], identity=ident[:])
                hT = spool.tile([HIDDEN, 128], dt.float32, tag="hT", name="fhT")
                nc.vector.tensor_copy(out=hT[:], in_=tp[:])
                gp = ppool.tile([128, DIM], dt.float32, tag="mm", name="gp")
                nc.tensor.matmul(out=gp[:], lhsT=hT[:], rhs=wout_t[:],
                                 start=True, stop=True)
                nc.vector.tensor_tensor(out=acc4[:], in0=acc4[:], in1=gp[:],
                                        op=OP.add)
            onesk = cpool.tile([128, 1], dt.float32, name="onesk")
            nc.vector.memset(onesk[:], 1.0 / cfg["n_nodes"])
            pooled_p = ppool.tile([1, 4], dt.float32, tag="mm", name="pooled_p")
            nc.tensor.matmul(out=pooled_p[:], lhsT=onesk[:], rhs=acc4[:],
                             start=True, stop=True)
            pooled_s = spool.tile([1, 4], dt.float32, tag="p4", name="pooled_s")
            nc.vector.tensor_copy(out=pooled_s[:], in_=pooled_p[:])
            nc.sync.dma_start(out=ar_in[:], in_=pooled_s[:])
            nc.gpsimd.collective_compute(
                "AllReduce", mybir.AluOpType.add,
                replica_groups=[list(range(cfg["n_cores"]))],
                ins=[ar_in[:]], outs=[ar_out[:]])
            pooled = spool.tile([1, 4], dt.float32, tag="p4b", name="pooled")
            nc.sync.dma_start(out=pooled[:], in_=ar_out[:])
            fcw_t = spool.tile([1, 12], dt.float32, tag="fcw", name="fcw_t")
            nc.sync.dma_start(out=fcw_t[:], in_=fcw_d[:])
            fcb_t = spool.tile([1, 3], dt.float32, tag="fcb", name="fcb_t")
            nc.sync.dma_start(out=fcb_t[:], in_=fcb_d[:])
            pr2 = spool.tile([1, 12], dt.float32, tag="pr2", name="pr2")
            nc.vector.tensor_tensor(
                out=pr2[:],
                in0=vap(pooled[:], 0, [[pooled[:].ap[0][0], 1], [0, 3], [1, 4]]),
                in1=vap(fcw_t[:], 0, [[fcw_t[:].ap[0][0], 1], [4, 3], [1, 4]]),
                op=OP.mult)
            y3 = spool.tile([1, 3], dt.float32, tag="y3", name="y3")
            nc.vector.tensor_reduce(
                out=y3[:],
                in_=vap(pr2[:], 0, [[pr2[:].ap[0][0], 1], [4, 3], [1, 4]]),
                axis=AX.X, op=OP.add)
            nc.vector.tensor_tensor(out=y3[:], in0=y3[:], in1=fcb_t[:], op=OP.add)
            nc.sync.dma_start(out=y_out[:], in_=y3[:])

    _split_excess_waits(nc, max_waits=1)
    lower_extended_insts(nc)
    return nc


def kernel(x, edge_index, W_in, Wq, Wk, Wv, Wo, W_out, fc_w, fc_b):
    x = np.asarray(x, dtype=np.float32)
    edge_index = np.asarray(edge_index)
    cfg = _cfg(x.shape[0])

    key = ("nc", x.shape[0], edge_index.shape[1])
    if key not in _RUN_CACHE:
        schedule, plans = _plan(edge_index, cfg)
        meta = (plans[0]["gidx"].shape[1], plans[0]["qidx"].shape[1],
                plans[0]["chunk_meta"])
        nc = _build(meta, cfg)
        _RUN_CACHE[key] = (nc, plans)
    nc, plans = _RUN_CACHE[key]

    wts = _permute_weights(
        np.asarray(W_in, np.float32), np.asarray(Wq, np.float32),
        np.asarray(Wk, np.float32), np.asarray(Wv, np.float32),
        np.asarray(Wo, np.float32), np.asarray(W_out, np.float32),
        np.asarray(fc_w, np.float32), np.asarray(fc_b, np.float32))

    nps, pcs, npp = cfg["n_per_sub"], cfg["per_core_sub"], cfg["npp"]
    old = np.arange(cfg["n_nodes"])
    c_of = (old % nps) // pcs
    l_of = (old // nps) * pcs + (old % pcs)
    in_maps = []
    for c in range(cfg["n_cores"]):
        xl = np.zeros((npp, DIM), dtype=np.float32)
        m = c_of == c
        xl[l_of[m]] = x[m]
        p = plans[c]
        in_maps.append(dict(
            x_in=xl, gidx=p["gidx"], qidx=p["qidx"], cidx=p["cidx"],
            npad=p["npad"], w_in=wts["W_in"], wq=wts["Wq"], wk=wts["Wk"],
            wv=wts["Wv"], wo=wts["Wo"], wout=wts["W_out"], fcw=wts["fcw"],
            fcb=wts["fcb"]))

    from concourse.bass_utils import run_bass_kernel_spmd
    trace = bool(os.environ.get("GNN_TRACE"))
    if trace:
        _install_profhook()
    res = run_bass_kernel_spmd(nc, in_maps, core_ids=list(range(cfg["n_cores"])),
                               trace=trace)
    if trace:
        _RUN_CACHE["last_result"] = res
    return np.asarray(res.results[0]["y"]).reshape(3).astype(np.float32)



# revision 12
# speedup vs baseline: 1.0765x; 1.0275x over previous
"""GNN message-passing (SE3-style graph attention) kernel for 8 Trainium2 cores.

Edge-parallel strategy:
- Nodes relabeled into 4 "subtables" x 8 cores so per-edge kv-gather indices
  fit int16 (dma_gather requirement). Each core owns 12500 dst nodes.
- Per layer: on-device q/k/v projections -> 4 AllGathers build global kv
  tables -> 4 passes over src subtables, each processing edges in node-major
  degree-sorted chunks: dma_gather kv rows, affine q broadcast, DVE
  scores/exp/messages, affine segment reduction into per-pass accumulators.
- Pass accumulators (degree-rank order) recombined into identity order by
  small dma_gathers, then divide / Wo project / residual / LayerNorm.
- Final: W_out, mean-pool via PE ones-matmul, AllReduce, FC head.
"""

import os
import sys
import types
import numpy as np

HEADS = 8
HEAD_DIM = 2
HIDDEN = 16
DIM = 4
DEPTH = 2
N_SUB = 4
KV_COLS = 64          # table row = 64 fp32 = 256B
SENT = 32767          # zeroed sentinel row in each kv subtable
SD_CAP = 128          # max S*D per chunk
S_CAP = 8

_RUN_CACHE = {}


# --------------------------------------------------------------------------
# harness shims (self-contained copies)
# --------------------------------------------------------------------------
def _split_excess_waits(nc, max_waits=1):
    """Walrus build allows 1 sync-wait per instruction; move extras to NOPs."""
    import concourse.mybir as mybir
    n = [0]
    for blk in nc.m.functions[0].blocks:
        new_insts = []
        for inst in blk.instructions:
            si = inst.sync_info
            if si is not None and len(si.on_wait) > max_waits:
                waits = list(si.on_wait)
                extra, keep = waits[:-max_waits], waits[-max_waits:]
                for i in range(0, len(extra), max_waits):
                    n[0] += 1
                    nop = mybir.InstNoOp(
                        name=f"IWS-{n[0]}", engine=inst.engine, ins=[], outs=[],
                        sync_info=mybir.SyncInfo(on_wait=extra[i:i + max_waits],
                                                 on_update=[]))
                    try:
                        nc.register_instruction(nop, overwrite=True)
                    except Exception:
                        pass
                    new_insts.append(nop)
                si.on_wait = keep
            new_insts.append(inst)
        blk.instructions[:] = new_insts


def _install_profhook():
    if 'antenv.axon_hooks' in sys.modules:
        return
    try:
        import antenv
        from trn_agent_boot.trn_boot import _ntff_profile_via_ctypes
        hook = _ntff_profile_via_ctypes('/opt/axon/libaxon_pjrt.so')
        mod = types.ModuleType('antenv.axon_hooks')
        state = {'hook': hook}
        mod.set_axon_ntff_profile_hook = lambda h: state.__setitem__('hook', h)
        mod.get_axon_ntff_profile_hook = lambda: state['hook']
        sys.modules['antenv.axon_hooks'] = mod
        antenv.axon_hooks = mod
    except Exception:
        pass


# --------------------------------------------------------------------------
# host-side planning
# --------------------------------------------------------------------------
def _cfg(n_nodes):
    n_cores = 8
    n_per_sub = n_nodes // N_SUB
    per_core_sub = n_per_sub // n_cores
    npc = N_SUB * per_core_sub
    npp = ((npc + 127) // 128) * 128
    return dict(n_nodes=n_nodes, n_cores=n_cores, n_per_sub=n_per_sub,
                per_core_sub=per_core_sub, npc=npc, npp=npp,
                n_tiles=npp // 128)


def _pack_gidx(idx_flat):
    """Gather feed position i lives at tile[i%16, i//16]; replicate x8 cores."""
    n = idx_flat.shape[0]
    assert n % 16 == 0
    tile16 = np.ascontiguousarray(
        idx_flat.reshape(n // 16, 16).T.astype(np.int16))
    return np.tile(tile16, (8, 1))


def _plan(edge_index, cfg):
    src = np.asarray(edge_index[0], dtype=np.int64)
    dst = np.asarray(edge_index[1], dtype=np.int64)
    nps, pcs = cfg["n_per_sub"], cfg["per_core_sub"]
    npp, n_cores = cfg["npp"], cfg["n_cores"]

    e_core = (dst % nps) // pcs
    e_l = (dst // nps) * pcs + (dst % pcs)
    e_t = src // nps
    e_row = src % nps

    passes = []
    for t in range(N_SUB):
        per_core = []
        for c in range(n_cores):
            m = (e_core == c) & (e_t == t)
            lt, rowt = e_l[m], e_row[m]
            deg = np.bincount(lt, minlength=npp).astype(np.int64)
            order = np.argsort(-deg, kind="stable")
            rank_of = np.empty(npp, dtype=np.int64)
            rank_of[order] = np.arange(npp)
            eorder = np.argsort(rank_of[lt], kind="stable")
            per_core.append(dict(deg=deg, order=order, rank_of=rank_of,
                                 lt=lt[eorder], rowt=rowt[eorder]))
        passes.append(per_core)

    schedule = []
    for t in range(N_SUB):
        chunks = []
        r0 = 0
        degs_sorted = [passes[t][c]["deg"][passes[t][c]["order"]]
                       for c in range(n_cores)]
        while r0 < npp:
            D = int(max(int(d[r0]) for d in degs_sorted))
            if D == 0:
                break
            S = max(1, min(S_CAP, SD_CAP // D, (npp - r0) // 128))
            chunks.append((r0, S, D))
            r0 += 128 * S
        schedule.append(chunks)

    plans = []
    for c in range(n_cores):
        gidx_cols, qidx_list, chunk_meta = [], [], []
        npad = np.zeros(npp, dtype=np.float64)
        gcol0 = qcol0 = 0
        for t in range(N_SUB):
            pc = passes[t][c]
            deg, order, rank_of = pc["deg"], pc["order"], pc["rank_of"]
            lt, rowt = pc["lt"], pc["rowt"]
            offs = np.zeros(npp + 1, dtype=np.int64)
            offs[1:] = np.cumsum(deg[order])
            ranks_e = rank_of[lt]
            j_e = np.arange(lt.shape[0]) - offs[ranks_e]
            for ci, (r0, S, D) in enumerate(schedule[t]):
                nrows = 128 * S * D
                idx_flat = np.full(nrows, SENT, dtype=np.int64)
                em = (ranks_e >= r0) & (ranks_e < r0 + 128 * S)
                q = ranks_e[em] - r0
                pos = ((q // 128) * D + j_e[em]) * 128 + (q % 128)
                idx_flat[pos] = rowt[em]
                gidx_cols.append(_pack_gidx(idx_flat))
                qidx_list.append(_pack_gidx(order[r0:r0 + 128 * S]))
                ch_nodes = order[r0:r0 + 128 * S]
                npad[ch_nodes] += D - deg[ch_nodes]
                chunk_meta.append((t, r0, S, D, gcol0, nrows // 16, qcol0))
                gcol0 += nrows // 16
                qcol0 += 8 * S
        gidx = (np.concatenate(gidx_cols, axis=1) if gidx_cols
                else np.zeros((128, 16), np.int16))
        qidx = (np.concatenate(qidx_list, axis=1) if qidx_list
                else np.zeros((128, 8), np.int16))
        cidx = np.stack([_pack_gidx(passes[t][c]["rank_of"][:npp])
                         for t in range(N_SUB)])
        npad_t = np.ascontiguousarray(
            npad.reshape(cfg["n_tiles"], 128).T.astype(np.float32))
        plans.append(dict(gidx=gidx, qidx=qidx, cidx=cidx, npad=npad_t,
                          chunk_meta=chunk_meta))
    return schedule, plans


def _permute_weights(W_in, Wq, Wk, Wv, Wo, W_out, fc_w, fc_b):
    perm = np.array([h * HEAD_DIM + d for d in range(HEAD_DIM)
                     for h in range(HEADS)], dtype=np.int64)
    return dict(W_in=np.ascontiguousarray(W_in),
                Wq=np.ascontiguousarray(Wq[:, :, perm]),
                Wk=np.ascontiguousarray(Wk[:, :, perm]),
                Wv=np.ascontiguousarray(Wv),
                Wo=np.ascontiguousarray(Wo),
                W_out=np.ascontiguousarray(W_out),
                fcw=np.ascontiguousarray(fc_w.T).reshape(1, 12).astype(np.float32),
                fcb=fc_b.reshape(1, 3).astype(np.float32))


# --------------------------------------------------------------------------
# device program
# --------------------------------------------------------------------------
def _build(meta, cfg):
    import concourse.bass as bass
    import concourse.mybir as mybir
    import concourse.tile as tile
    from concourse import library_config
    from concourse.masks import make_identity
    from concourse.library_overlay import lower_extended_insts

    dt = mybir.dt
    AX = mybir.AxisListType
    OP = mybir.AluOpType
    ACTF = mybir.ActivationFunctionType
    npp, n_tiles, npc = cfg["npp"], cfg["n_tiles"], cfg["npc"]
    pcs = cfg["per_core_sub"]
    g_cols_total, q_cols_total, chunk_meta = meta

    nc = bass.Bass(num_devices=cfg["n_cores"], num_swdge_queues=4)

    x_in = nc.dram_tensor("x_in", [npp, DIM], dt.float32, kind="ExternalInput")
    gidx_d = nc.dram_tensor("gidx", [128, g_cols_total], dt.int16, kind="ExternalInput")
    qidx_d = nc.dram_tensor("qidx", [128, q_cols_total], dt.int16, kind="ExternalInput")
    cidx_d = nc.dram_tensor("cidx", [N_SUB, 128, npp // 16], dt.int16, kind="ExternalInput")
    npad_d = nc.dram_tensor("npad", [128, n_tiles], dt.float32, kind="ExternalInput")
    w_in_d = nc.dram_tensor("w_in", [DIM, HIDDEN], dt.float32, kind="ExternalInput")
    wq_d = nc.dram_tensor("wq", [DEPTH, HIDDEN, HIDDEN], dt.float32, kind="ExternalInput")
    wk_d = nc.dram_tensor("wk", [DEPTH, HIDDEN, HIDDEN], dt.float32, kind="ExternalInput")
    wv_d = nc.dram_tensor("wv", [DEPTH, HIDDEN, HIDDEN], dt.float32, kind="ExternalInput")
    wo_d = nc.dram_tensor("wo", [DEPTH, HIDDEN, HIDDEN], dt.float32, kind="ExternalInput")
    wout_d = nc.dram_tensor("wout", [HIDDEN, DIM], dt.float32, kind="ExternalInput")
    fcw_d = nc.dram_tensor("fcw", [1, 12], dt.float32, kind="ExternalInput")
    fcb_d = nc.dram_tensor("fcb", [1, 3], dt.float32, kind="ExternalInput")
    y_out = nc.dram_tensor("y", [1, 3], dt.float32, kind="ExternalOutput")

    kv_tab = [nc.dram_tensor(f"kvtab{t}", [32768, KV_COLS], dt.float32,
                             kind="Internal", addr_space="Shared")
              for t in range(N_SUB)]
    q_tab = nc.dram_tensor("qtab", [npp, KV_COLS], dt.float32, kind="Internal")
    u_tab = [nc.dram_tensor(f"utab{t}", [npp, KV_COLS], dt.float32, kind="Internal")
             for t in range(N_SUB)]
    stage_t = [nc.dram_tensor(f"stage{t}", [npc // N_SUB, KV_COLS], dt.float32,
                              kind="Internal") for t in range(N_SUB)]
    ar_in = nc.dram_tensor("ar_in", [1, 4], dt.float32, kind="Internal")
    ar_out = nc.dram_tensor("ar_out", [1, 4], dt.float32, kind="Internal",
                            addr_space="Shared")

    nc.gpsimd.load_library(library_config.attnmlp)
    gq = [0]
    _nregs = {}

    def gather(out_ap, in_tensor, nrows, idx_ap, num_idxs, elem_size):
        """Raw InstDMAGatherAnt reading elem_size f32 from 256B-pitch rows.

        Bypasses bass's elem_size_bytes%256 assert: the ucode only requires
        the row *pitch* (elem_step bytes) to be a 256B multiple."""
        if num_idxs not in _nregs:
            _nregs[num_idxs] = nc.gpsimd.to_reg(num_idxs)
        g = nc.gpsimd
        in_ap = bass.AP(in_tensor, 0, [[KV_COLS, nrows], [1, elem_size]])
        _in_ap = g.lower_ap_dma(in_ap, for_custom_bir_dma=True)
        _idxs_ap = g.lower_ap(idx_ap)
        _out_ap = g.lower_ap(out_ap)
        g.add_instruction(mybir.InstDMAGatherAnt(
            name=nc.get_next_instruction_name(),
            ins=[*_in_ap, _idxs_ap, g.lower_val_access(_nregs[num_idxs])],
            outs=[_out_ap],
            transpose=False, num_idxs=num_idxs, elem_size=elem_size,
            stride_bytes_256=1, gen_mode=0, single_packet=False,
            queue_num=gq[0] % 4,
            sbuf_tokens_per_rank=0, sbuf_free_dim_per_rank=0,
            sbuf_free_dim_pad_per_rank=0, sbuf_byte_offset=0))
        gq[0] += 1

    def vap(base_ap, extra_off, dims):
        return bass.AP(base_ap.tensor, base_ap.offset + extra_off, dims)

    with tile.TileContext(nc) as tc:
        with (
            tc.tile_pool(name="const", bufs=1) as cpool,
            tc.tile_pool(name="res", bufs=1) as rpool,
            tc.tile_pool(name="work", bufs=2) as wpool,
            tc.tile_pool(name="gath", bufs=4) as gpool,
            tc.tile_pool(name="small", bufs=2) as spool,
            tc.tile_pool(name="psum", bufs=4, space="PSUM") as ppool,
        ):
            ident = cpool.tile([128, 128], dt.float32, name="ident")
            make_identity(nc, ident[:])
            w_in_t = cpool.tile([DIM, HIDDEN], dt.float32, name="w_in_t")
            nc.sync.dma_start(out=w_in_t[:], in_=w_in_d[:])
            wmat = {}
            for nm, dd in (("wq", wq_d), ("wk", wk_d), ("wv", wv_d), ("wo", wo_d)):
                for l in range(DEPTH):
                    w = cpool.tile([HIDDEN, HIDDEN], dt.float32, name=f"{nm}{l}")
                    nc.sync.dma_start(out=w[:], in_=dd[l])
                    wmat[(nm, l)] = w
            wout_t = cpool.tile([HIDDEN, DIM], dt.float32, name="wout_t")
            nc.sync.dma_start(out=wout_t[:], in_=wout_d[:])
            npad_t = cpool.tile([128, n_tiles], dt.float32, name="npad_t")
            nc.sync.dma_start(out=npad_t[:], in_=npad_d[:])

            zrow = cpool.tile([1, KV_COLS], dt.float32, name="zrow")
            nc.vector.memset(zrow[:], 0.0)
            for t in range(N_SUB):
                nc.sync.dma_start(out=kv_tab[t][SENT:SENT + 1, :], in_=zrow[:])

            h_all = rpool.tile([128, n_tiles, HIDDEN], dt.float32, name="h_all")
            u_tot = rpool.tile([128, n_tiles, 24], dt.float32, name="u_tot")
            u_accs = [rpool.tile([128, n_tiles, 24], dt.float32, name=f"u_acc{i}")
                      for i in range(N_SUB)]
            acc4 = rpool.tile([128, 4], dt.float32, name="acc4")

            # stage 0: h0 = x @ W_in
            xs = wpool.tile([128, n_tiles, DIM], dt.float32, tag="gt", name="xs")
            nc.sync.dma_start(out=xs[:],
                              in_=x_in[:].rearrange("(a p) f -> p a f", p=128))
            for k in range(n_tiles):
                tp = ppool.tile([DIM, 128], dt.float32, tag="tp", name="tp")
                nc.tensor.transpose(out=tp[:], in_=xs[:, k, :], identity=ident[:])
                tps = spool.tile([HIDDEN, 128], dt.float32, tag="hT", name="tps")
                nc.vector.tensor_copy(out=tps[0:DIM, :], in_=tp[:])
                hp = ppool.tile([128, HIDDEN], dt.float32, tag="mm", name="hp")
                nc.tensor.matmul(out=hp[:], lhsT=tps[0:DIM, :], rhs=w_in_t[:],
                                 start=True, stop=True)
                nc.vector.tensor_copy(out=h_all[:, k, :], in_=hp[:])

            scale = float(1.0 / np.sqrt(HEAD_DIM))

            def issue_ag(t):
                nc.gpsimd.collective_compute(
                    "AllGather", mybir.AluOpType.bypass,
                    replica_groups=[list(range(cfg["n_cores"]))],
                    ins=[stage_t[t][:]],
                    outs=[kv_tab[t][0:cfg["n_cores"] * pcs, :]])

            for layer in range(DEPTH):
                # q/k/v projections; stage written per-subtable so each
                # AllGather depends only on its own slab
                for k in range(n_tiles):
                    tp = ppool.tile([HIDDEN, 128], dt.float32, tag="tp", name="htp")
                    nc.tensor.transpose(out=tp[:], in_=h_all[:, k, :],
                                        identity=ident[:])
                    hT = spool.tile([HIDDEN, 128], dt.float32, tag="hT", name="hT")
                    nc.vector.tensor_copy(out=hT[:], in_=tp[:])
                    qp = ppool.tile([128, HIDDEN], dt.float32, tag="mm", name="qp")
                    nc.tensor.matmul(out=qp[:], lhsT=hT[:], rhs=wmat[("wq", layer)][:],
                                     start=True, stop=True)
                    qs = spool.tile([128, HIDDEN], dt.float32, tag="qs", name="qs")
                    nc.vector.tensor_copy(out=qs[:], in_=qp[:])
                    nc.sync.dma_start(out=q_tab[k * 128:(k + 1) * 128, 0:HIDDEN],
                                      in_=qs[:])
                    kp = ppool.tile([128, HIDDEN], dt.float32, tag="mm", name="kp")
                    nc.tensor.matmul(out=kp[:], lhsT=hT[:], rhs=wmat[("wk", layer)][:],
                                     start=True, stop=True)
                    kvs = spool.tile([128, 32], dt.float32, tag="kvs", name="kvs")
                    nc.vector.tensor_copy(out=kvs[:, 0:16], in_=kp[:])
                    vp = ppool.tile([128, HIDDEN], dt.float32, tag="mm", name="vp")
                    nc.tensor.matmul(out=vp[:], lhsT=hT[:], rhs=wmat[("wv", layer)][:],
                                     start=True, stop=True)
                    nc.vector.tensor_copy(out=kvs[:, 16:32], in_=vp[:])
                    lo, hi = k * 128, min((k + 1) * 128, npc)
                    r = lo
                    while r < hi:
                        t = r // pcs
                        e = min(hi, (t + 1) * pcs)
                        nc.sync.dma_start(
                            out=stage_t[t][r - t * pcs:e - t * pcs, 0:32],
                            in_=kvs[r - lo:e - lo, :])
                        r = e
                issue_ag(0)
                issue_ag(1)

                for t in range(N_SUB):
                    u_acc = u_accs[t]
                    nc.vector.memset(u_acc[:], 0.0)
                    for (tt, r0, S, D, gc0, gcols, qc0) in chunk_meta:
                        if tt != t:
                            continue
                        sl = r0 // 128
                        gt = wpool.tile([128, 1024], dt.int16, tag="gt", name="gt")
                        nc.sync.dma_start(out=gt[:, 0:gcols],
                                          in_=gidx_d[:, gc0:gc0 + gcols])
                        qt = spool.tile([128, 64], dt.int16, tag="qt", name="qt")
                        nc.sync.dma_start(out=qt[:, 0:8 * S],
                                          in_=qidx_d[:, qc0:qc0 + 8 * S])
                        kvg = gpool.tile([128, SD_CAP, 32], dt.float32,
                                         tag="kvg", name="kvg")
                        nrow = S * D
                        done = 0
                        while done < nrow:
                            cnt = min(8, nrow - done)
                            gather(kvg[:, done:done + cnt, :], kv_tab[t], 32768,
                                   gt[:, done * 8:(done + cnt) * 8], cnt * 128, 32)
                            done += cnt
                        qg = spool.tile([128, S_CAP, HIDDEN], dt.float32,
                                        tag="qg", name="qg")
                        gather(qg[:, 0:S, :], q_tab, npp,
                               qt[:, 0:8 * S], 128 * S, HIDDEN)
                        kvga, qga = kvg[:], qg[:]
                        pk = kvga.ap[0][0]
                        pq = qga.ap[0][0]
                        prod = wpool.tile([128, S * HIDDEN, D],
                                          dt.float32, tag="prod", name="prod")
                        pp = prod[:].ap[0][0]
                        nc.vector.tensor_tensor(
                            out=vap(prod[:], 0,
                                    [[pp, 128], [HIDDEN * D, S], [D, HIDDEN], [1, D]]),
                            in0=vap(qga, 0,
                                    [[pq, 128], [HIDDEN, S], [1, HIDDEN], [0, D]]),
                            in1=vap(kvga, 0,
                                    [[pk, 128], [D * 32, S], [1, HIDDEN],
                                     [32, D]]),
                            op=OP.mult)
                        wgt = wpool.tile([128, S * HEADS, D],
                                         dt.float32, tag="wgt", name="wgt")
                        pw = wgt[:].ap[0][0]
                        nc.vector.tensor_tensor(
                            out=vap(wgt[:], 0,
                                    [[pw, 128], [HEADS * D, S], [D, HEADS], [1, D]]),
                            in0=vap(prod[:], 0,
                                    [[pp, 128], [HIDDEN * D, S], [D, HEADS], [1, D]]),
                            in1=vap(prod[:], HEADS * D,
                                    [[pp, 128], [HIDDEN * D, S], [D, HEADS], [1, D]]),
                            op=OP.add)
                        nc.scalar.activation(
                            out=vap(wgt[:], 0, [[pw, 128], [1, S * HEADS * D]]),
                            in_=vap(wgt[:], 0, [[pw, 128], [1, S * HEADS * D]]),
                            func=ACTF.Exp, scale=scale)
                        nc.vector.tensor_reduce(
                            out=u_acc[:, sl:sl + S, 16:24],
                            in_=vap(wgt[:], 0,
                                    [[pw, 128], [D, S * HEADS], [1, D]]),
                            axis=AX.X, op=OP.add)
                        msg = wpool.tile([128, S * HIDDEN, D],
                                         dt.float32, tag="prod", name="msg")
                        pm = msg[:].ap[0][0]
                        nc.vector.tensor_tensor(
                            out=vap(msg[:], 0,
                                    [[pm, 128], [HIDDEN * D, S], [HEAD_DIM * D, HEADS],
                                     [D, HEAD_DIM], [1, D]]),
                            in0=vap(wgt[:], 0,
                                    [[pw, 128], [HEADS * D, S], [D, HEADS],
                                     [0, HEAD_DIM], [1, D]]),
                            in1=vap(kvga, 16,
                                    [[pk, 128], [D * 32, S], [HEAD_DIM, HEADS],
                                     [1, HEAD_DIM], [32, D]]),
                            op=OP.mult)
                        nc.vector.tensor_reduce(
                            out=u_acc[:, sl:sl + S, 0:16],
                            in_=vap(msg[:], 0,
                                    [[pm, 128], [D, S * HIDDEN], [1, D]]),
                            axis=AX.X, op=OP.add)
                    # u_acc (rank order) -> DRAM; recombination deferred to
                    # layer end so pass t+1 gathers are never blocked
                    nc.sync.dma_start(
                        out=bass.AP(u_tab[t], 0,
                                    [[KV_COLS, 128], [128 * KV_COLS, n_tiles],
                                     [1, 24]]),
                        in_=u_acc[:])
                    if t + 2 < N_SUB:
                        issue_ag(t + 2)

                # recombine all passes into identity order
                nc.vector.memset(u_tot[:], 0.0)
                for t in range(N_SUB):
                    ct = spool.tile([128, npp // 16], dt.int16, tag="ct", name="ct")
                    nc.sync.dma_start(out=ct[:], in_=cidx_d[t])
                    done = 0
                    while done < n_tiles:
                        cnt = min(8, n_tiles - done)
                        cg = spool.tile([128, 8, 24], dt.float32, tag="cgt",
                                        name="cg")
                        gather(cg[:, 0:cnt, :], u_tab[t], npp,
                               ct[:, done * 8:(done + cnt) * 8], cnt * 128, 24)
                        nc.vector.tensor_tensor(
                            out=u_tot[:, done:done + cnt, :],
                            in0=u_tot[:, done:done + cnt, :],
                            in1=cg[:, 0:cnt, :], op=OP.add)
                        done += cnt

                # epilogue
                dadj = spool.tile([128, n_tiles, HEADS], dt.float32, tag="dadj",
                                  name="dadj")
                pn = npad_t[:].ap[0][0]
                nc.vector.tensor_tensor(
                    out=dadj[:], in0=u_tot[:, :, 16:24],
                    in1=vap(npad_t[:], 0, [[pn, 128], [1, n_tiles], [0, HEADS]]),
                    op=OP.subtract)
                nc.vector.tensor_scalar_add(out=dadj[:], in0=dadj[:], scalar1=1e-9)
                rden = spool.tile([128, n_tiles, HEADS], dt.float32, tag="rden",
                                  name="rden")
                nc.vector.reciprocal(out=rden[:], in_=dadj[:])
                agg = wpool.tile([128, n_tiles, HIDDEN], dt.float32, tag="wgt",
                                 name="agg")
                pr_ = rden[:].ap[0][0]
                pa = agg[:].ap[0][0]
                nc.vector.tensor_tensor(
                    out=vap(agg[:], 0,
                            [[pa, 128], [HIDDEN, n_tiles], [HEAD_DIM, HEADS],
                             [1, HEAD_DIM]]),
                    in0=vap(u_tot[:], 0,
                            [[u_tot[:].ap[0][0], 128], [24, n_tiles],
                             [HEAD_DIM, HEADS], [1, HEAD_DIM]]),
                    in1=vap(rden[:], 0,
                            [[pr_, 128], [HEADS, n_tiles], [1, HEADS],
                             [0, HEAD_DIM]]),
                    op=OP.mult)
                hnew = wpool.tile([128, n_tiles, HIDDEN], dt.float32, tag="prod",
                                  name="hnew")
                for k in range(n_tiles):
                    tp = ppool.tile([HIDDEN, 128], dt.float32, tag="tp", name="atp")
                    nc.tensor.transpose(out=tp[:], in_=agg[:, k, :],
                                        identity=ident[:])
                    aT = spool.tile([HIDDEN, 128], dt.float32, tag="hT", name="aT")
                    nc.vector.tensor_copy(out=aT[:], in_=tp[:])
                    op_ = ppool.tile([128, HIDDEN], dt.float32, tag="mm", name="op_")
                    nc.tensor.matmul(out=op_[:], lhsT=aT[:],
                                     rhs=wmat[("wo", layer)][:],
                                     start=True, stop=True)
                    nc.vector.tensor_tensor(out=hnew[:, k, :], in0=op_[:],
                                            in1=h_all[:, k, :], op=OP.add)
                mu = spool.tile([128, n_tiles, 1], dt.float32, tag="mu", name="mu")
                nc.vector.tensor_reduce(out=mu[:], in_=hnew[:], axis=AX.X, op=OP.add)
                nc.vector.tensor_scalar_mul(out=mu[:], in0=mu[:], scalar1=1.0 / 16)
                cent = wpool.tile([128, n_tiles, HIDDEN], dt.float32, tag="wgt",
                                  name="cent")
                nc.vector.tensor_tensor(
                    out=cent[:], in0=hnew[:],
                    in1=vap(mu[:], 0, [[mu[:].ap[0][0], 128], [1, n_tiles],
                                       [0, HIDDEN]]),
                    op=OP.subtract)
                sq = wpool.tile([128, n_tiles, HIDDEN], dt.float32, tag="prod",
                                name="sq")
                nc.vector.tensor_tensor(out=sq[:], in0=cent[:], in1=cent[:],
                                        op=OP.mult)
                var = spool.tile([128, n_tiles, 1], dt.float32, tag="var", name="var")
                nc.vector.tensor_reduce(out=var[:], in_=sq[:], axis=AX.X, op=OP.add)
                nc.vector.tensor_scalar_mul(out=var[:], in0=var[:], scalar1=1.0 / 16)
                nc.vector.tensor_scalar_add(out=var[:], in0=var[:], scalar1=1e-5)
                rs = spool.tile([128, n_tiles, 1], dt.float32, tag="rs", name="rs")
                nc.vector.reciprocal(out=rs[:], in_=var[:])
                nc.scalar.activation(out=rs[:], in_=rs[:], func=ACTF.Sqrt)
                nc.vector.tensor_tensor(
                    out=h_all[:], in0=cent[:],
                    in1=vap(rs[:], 0, [[rs[:].ap[0][0], 128], [1, n_tiles],
                                       [0, HIDDEN]]),
                    op=OP.mult)

            # final head
            nc.vector.memset(acc4[:], 0.0)
            for k in range(n_tiles):
                tp = ppool.tile([HIDDEN, 128], dt.float32, tag="tp", name="ftp")
                nc.tensor.transpose(out=tp[:], in_=h_all[:, k, :], identity=ident[:])
                hT = spool.tile([HIDDEN, 128], dt.float32, tag="hT", name="fhT")
                nc.vector.tensor_copy(out=hT[:], in_=tp[:])
                gp = ppool.tile([128, DIM], dt.float32, tag="mm", name="gp")
                nc.tensor.matmul(out=gp[:], lhsT=hT[:], rhs=wout_t[:],
                                 start=True, stop=True)
                nc.vector.tensor_tensor(out=acc4[:], in0=acc4[:], in1=gp[:],
                                        op=OP.add)
            onesk = cpool.tile([128, 1], dt.float32, name="onesk")
            nc.vector.memset(onesk[:], 1.0 / cfg["n_nodes"])
            pooled_p = ppool.tile([1, 4], dt.float32, tag="mm", name="pooled_p")
            nc.tensor.matmul(out=pooled_p[:], lhsT=onesk[:], rhs=acc4[:],
                             start=True, stop=True)
            pooled_s = spool.tile([1, 4], dt.float32, tag="p4", name="pooled_s")
            nc.vector.tensor_copy(out=pooled_s[:], in_=pooled_p[:])
            nc.sync.dma_start(out=ar_in[:], in_=pooled_s[:])
            nc.gpsimd.collective_compute(
                "AllReduce", mybir.AluOpType.add,
                replica_groups=[list(range(cfg["n_cores"]))],
                ins=[ar_in[:]], outs=[ar_out[:]])
            pooled = spool.tile([1, 4], dt.float32, tag="p4b", name="pooled")
            nc.sync.dma_start(out=pooled[:], in_=ar_out[:])
            fcw_t = spool.tile([1, 12], dt.float32, tag="fcw", name="fcw_t")
            nc.sync.dma_start(out=fcw_t[:], in_=fcw_d[:])
            fcb_t = spool.tile([1, 3], dt.float32, tag="fcb", name="fcb_t")
            nc.sync.dma_start(out=fcb_t[:], in_=fcb_d[:])
            pr2 = spool.tile([1, 12], dt.float32, tag="pr2", name="pr2")
            nc.vector.tensor_tensor(
                out=pr2[:],
                in0=vap(pooled[:], 0, [[pooled[:].ap[0][0], 1], [0, 3], [1, 4]]),
                in1=vap(fcw_t[:], 0, [[fcw_t[:].ap[0][0], 1], [4, 3], [1, 4]]),
                op=OP.mult)
            y3 = spool.tile([1, 3], dt.float32, tag="y3", name="y3")
            nc.vector.tensor_reduce(
                out=y3[:],
                in_=vap(pr2[:], 0, [[pr2[:].ap[0][0], 1], [4, 3], [1, 4]]),
                axis=AX.X, op=OP.add)
            nc.vector.tensor_tensor(out=y3[:], in0=y3[:], in1=fcb_t[:], op=OP.add)
            nc.sync.dma_start(out=y_out[:], in_=y3[:])

    _split_excess_waits(nc, max_waits=1)
    lower_extended_insts(nc)
    return nc


def kernel(x, edge_index, W_in, Wq, Wk, Wv, Wo, W_out, fc_w, fc_b):
    x = np.asarray(x, dtype=np.float32)
    edge_index = np.asarray(edge_index)
    cfg = _cfg(x.shape[0])

    key = ("nc", x.shape[0], edge_index.shape[1])
    if key not in _RUN_CACHE:
        schedule, plans = _plan(edge_index, cfg)
        meta = (plans[0]["gidx"].shape[1], plans[0]["qidx"].shape[1],
                plans[0]["chunk_meta"])
        nc = _build(meta, cfg)
        _RUN_CACHE[key] = (nc, plans)
    nc, plans = _RUN_CACHE[key]

    wts = _permute_weights(
        np.asarray(W_in, np.float32), np.asarray(Wq, np.float32),
        np.asarray(Wk, np.float32), np.asarray(Wv, np.float32),
        np.asarray(Wo, np.float32), np.asarray(W_out, np.float32),
        np.asarray(fc_w, np.float32), np.asarray(fc_b, np.float32))

    nps, pcs, npp = cfg["n_per_sub"], cfg["per_core_sub"], cfg["npp"]
    old = np.arange(cfg["n_nodes"])
    c_of = (old % nps) // pcs
    l_of = (old // nps) * pcs + (old % pcs)
    in_maps = []
    for c in range(cfg["n_cores"]):
        xl = np.zeros((npp, DIM), dtype=np.float32)
        m = c_of == c
        xl[l_of[m]] = x[m]
        p = plans[c]
        in_maps.append(dict(
            x_in=xl, gidx=p["gidx"], qidx=p["qidx"], cidx=p["cidx"],
            npad=p["npad"], w_in=wts["W_in"], wq=wts["Wq"], wk=wts["Wk"],
            wv=wts["Wv"], wo=wts["Wo"], wout=wts["W_out"], fcw=wts["fcw"],
            fcb=wts["fcb"]))

    from concourse.bass_utils import run_bass_kernel_spmd
    trace = bool(os.environ.get("GNN_TRACE"))
    if trace:
        _install_profhook()
    res = run_bass_kernel_spmd(nc, in_maps, core_ids=list(range(cfg["n_cores"])),
                               trace=trace)
    if trace:
        _RUN_CACHE["last_result"] = res
    return np.asarray(res.results[0]["y"]).reshape(3).astype(np.float32)



# revision 15
# speedup vs baseline: 1.0950x; 1.0172x over previous
"""GNN message-passing (SE3-style graph attention) kernel for 8 Trainium2 cores.

Edge-parallel strategy:
- Nodes relabeled into 4 "subtables" x 8 cores so per-edge kv-gather indices
  fit int16 (dma_gather requirement). Each core owns 12500 dst nodes.
- Per layer: on-device q/k/v projections -> 4 AllGathers build global kv
  tables -> 4 passes over src subtables, each processing edges in node-major
  degree-sorted chunks: dma_gather kv rows, affine q broadcast, DVE
  scores/exp/messages, affine segment reduction into per-pass accumulators.
- Pass accumulators (degree-rank order) recombined into identity order by
  small dma_gathers, then divide / Wo project / residual / LayerNorm.
- Final: W_out, mean-pool via PE ones-matmul, AllReduce, FC head.
"""

import os
import sys
import types
import numpy as np

HEADS = 8
HEAD_DIM = 2
HIDDEN = 16
DIM = 4
DEPTH = 2
N_SUB = 4
KV_COLS = 64          # table row = 64 fp32 = 256B
SENT = 32767          # zeroed sentinel row in each kv subtable
SD_CAP = 128          # max S*D per chunk
S_CAP = 8

_RUN_CACHE = {}


# --------------------------------------------------------------------------
# harness shims (self-contained copies)
# --------------------------------------------------------------------------
def _split_excess_waits(nc, max_waits=1):
    """Walrus build allows 1 sync-wait per instruction; move extras to NOPs."""
    import concourse.mybir as mybir
    n = [0]
    for blk in nc.m.functions[0].blocks:
        new_insts = []
        for inst in blk.instructions:
            si = inst.sync_info
            if si is not None and len(si.on_wait) > max_waits:
                waits = list(si.on_wait)
                extra, keep = waits[:-max_waits], waits[-max_waits:]
                for i in range(0, len(extra), max_waits):
                    n[0] += 1
                    nop = mybir.InstNoOp(
                        name=f"IWS-{n[0]}", engine=inst.engine, ins=[], outs=[],
                        sync_info=mybir.SyncInfo(on_wait=extra[i:i + max_waits],
                                                 on_update=[]))
                    try:
                        nc.register_instruction(nop, overwrite=True)
                    except Exception:
                        pass
                    new_insts.append(nop)
                si.on_wait = keep
            new_insts.append(inst)
        blk.instructions[:] = new_insts


def _install_profhook():
    if 'antenv.axon_hooks' in sys.modules:
        return
    try:
        import antenv
        from trn_agent_boot.trn_boot import _ntff_profile_via_ctypes
        hook = _ntff_profile_via_ctypes('/opt/axon/libaxon_pjrt.so')
        mod = types.ModuleType('antenv.axon_hooks')
        state = {'hook': hook}
        mod.set_axon_ntff_profile_hook = lambda h: state.__setitem__('hook', h)
        mod.get_axon_ntff_profile_hook = lambda: state['hook']
        sys.modules['antenv.axon_hooks'] = mod
        antenv.axon_hooks = mod
    except Exception:
        pass


# --------------------------------------------------------------------------
# host-side planning
# --------------------------------------------------------------------------
def _cfg(n_nodes):
    n_cores = 8
    n_per_sub = n_nodes // N_SUB
    per_core_sub = n_per_sub // n_cores
    npc = N_SUB * per_core_sub
    npp = ((npc + 127) // 128) * 128
    return dict(n_nodes=n_nodes, n_cores=n_cores, n_per_sub=n_per_sub,
                per_core_sub=per_core_sub, npc=npc, npp=npp,
                n_tiles=npp // 128)


def _pack_gidx(idx_flat):
    """Gather feed position i lives at tile[i%16, i//16]; replicate x8 cores."""
    n = idx_flat.shape[0]
    assert n % 16 == 0
    tile16 = np.ascontiguousarray(
        idx_flat.reshape(n // 16, 16).T.astype(np.int16))
    return np.tile(tile16, (8, 1))


def _plan(edge_index, cfg):
    src = np.asarray(edge_index[0], dtype=np.int64)
    dst = np.asarray(edge_index[1], dtype=np.int64)
    nps, pcs = cfg["n_per_sub"], cfg["per_core_sub"]
    npp, n_cores = cfg["npp"], cfg["n_cores"]

    e_core = (dst % nps) // pcs
    e_l = (dst // nps) * pcs + (dst % pcs)
    e_t = src // nps
    e_row = src % nps

    passes = []
    for t in range(N_SUB):
        per_core = []
        for c in range(n_cores):
            m = (e_core == c) & (e_t == t)
            lt, rowt = e_l[m], e_row[m]
            deg = np.bincount(lt, minlength=npp).astype(np.int64)
            order = np.argsort(-deg, kind="stable")
            rank_of = np.empty(npp, dtype=np.int64)
            rank_of[order] = np.arange(npp)
            eorder = np.argsort(rank_of[lt], kind="stable")
            per_core.append(dict(deg=deg, order=order, rank_of=rank_of,
                                 lt=lt[eorder], rowt=rowt[eorder]))
        passes.append(per_core)

    schedule = []
    for t in range(N_SUB):
        chunks = []
        r0 = 0
        degs_sorted = [passes[t][c]["deg"][passes[t][c]["order"]]
                       for c in range(n_cores)]
        while r0 < npp:
            D = int(max(int(d[r0]) for d in degs_sorted))
            if D == 0:
                break
            S = max(1, min(S_CAP, SD_CAP // D, (npp - r0) // 128))
            chunks.append((r0, S, D))
            r0 += 128 * S
        schedule.append(chunks)

    plans = []
    for c in range(n_cores):
        gidx_cols, qidx_list, chunk_meta = [], [], []
        npad = np.zeros(npp, dtype=np.float64)
        gcol0 = 0
        for t in range(N_SUB):
            pc = passes[t][c]
            deg, order, rank_of = pc["deg"], pc["order"], pc["rank_of"]
            lt, rowt = pc["lt"], pc["rowt"]
            offs = np.zeros(npp + 1, dtype=np.int64)
            offs[1:] = np.cumsum(deg[order])
            ranks_e = rank_of[lt]
            j_e = np.arange(lt.shape[0]) - offs[ranks_e]
            # one pass-wide q index block: full rank order
            qidx_list.append(_pack_gidx(order[0:npp]))
            for ci, (r0, S, D) in enumerate(schedule[t]):
                nrows = 128 * S * D
                idx_flat = np.full(nrows, SENT, dtype=np.int64)
                em = (ranks_e >= r0) & (ranks_e < r0 + 128 * S)
                q = ranks_e[em] - r0
                pos = ((q // 128) * D + j_e[em]) * 128 + (q % 128)
                idx_flat[pos] = rowt[em]
                gidx_cols.append(_pack_gidx(idx_flat))
                ch_nodes = order[r0:r0 + 128 * S]
                npad[ch_nodes] += D - deg[ch_nodes]
                chunk_meta.append((t, r0, S, D, gcol0, nrows // 16))
                gcol0 += nrows // 16
        gidx = (np.concatenate(gidx_cols, axis=1) if gidx_cols
                else np.zeros((128, 16), np.int16))
        qidx = (np.concatenate(qidx_list, axis=1) if qidx_list
                else np.zeros((128, 8), np.int16))
        cidx = np.stack([_pack_gidx(passes[t][c]["rank_of"][:npp])
                         for t in range(N_SUB)])
        npad_t = np.ascontiguousarray(
            npad.reshape(cfg["n_tiles"], 128).T.astype(np.float32))
        plans.append(dict(gidx=gidx, qidx=qidx, cidx=cidx, npad=npad_t,
                          chunk_meta=chunk_meta))
    return schedule, plans


def _permute_weights(W_in, Wq, Wk, Wv, Wo, W_out, fc_w, fc_b):
    perm = np.array([h * HEAD_DIM + d for d in range(HEAD_DIM)
                     for h in range(HEADS)], dtype=np.int64)
    return dict(W_in=np.ascontiguousarray(W_in),
                Wq=np.ascontiguousarray(Wq[:, :, perm]),
                Wk=np.ascontiguousarray(Wk[:, :, perm]),
                Wv=np.ascontiguousarray(Wv),
                Wo=np.ascontiguousarray(Wo),
                W_out=np.ascontiguousarray(W_out),
                fcw=np.ascontiguousarray(fc_w.T).reshape(1, 12).astype(np.float32),
                fcb=fc_b.reshape(1, 3).astype(np.float32))


# --------------------------------------------------------------------------
# device program
# --------------------------------------------------------------------------
def _build(meta, cfg):
    import concourse.bass as bass
    import concourse.mybir as mybir
    import concourse.tile as tile
    from concourse import library_config
    from concourse.masks import make_identity
    from concourse.library_overlay import lower_extended_insts

    dt = mybir.dt
    AX = mybir.AxisListType
    OP = mybir.AluOpType
    ACTF = mybir.ActivationFunctionType
    npp, n_tiles, npc = cfg["npp"], cfg["n_tiles"], cfg["npc"]
    pcs = cfg["per_core_sub"]
    g_cols_total, q_cols_total, chunk_meta = meta

    nc = bass.Bass(num_devices=cfg["n_cores"], num_swdge_queues=4)

    x_in = nc.dram_tensor("x_in", [npp, DIM], dt.float32, kind="ExternalInput")
    gidx_d = nc.dram_tensor("gidx", [128, g_cols_total], dt.int16, kind="ExternalInput")
    qidx_d = nc.dram_tensor("qidx", [128, q_cols_total], dt.int16, kind="ExternalInput")
    cidx_d = nc.dram_tensor("cidx", [N_SUB, 128, npp // 16], dt.int16, kind="ExternalInput")
    npad_d = nc.dram_tensor("npad", [128, n_tiles], dt.float32, kind="ExternalInput")
    w_in_d = nc.dram_tensor("w_in", [DIM, HIDDEN], dt.float32, kind="ExternalInput")
    wq_d = nc.dram_tensor("wq", [DEPTH, HIDDEN, HIDDEN], dt.float32, kind="ExternalInput")
    wk_d = nc.dram_tensor("wk", [DEPTH, HIDDEN, HIDDEN], dt.float32, kind="ExternalInput")
    wv_d = nc.dram_tensor("wv", [DEPTH, HIDDEN, HIDDEN], dt.float32, kind="ExternalInput")
    wo_d = nc.dram_tensor("wo", [DEPTH, HIDDEN, HIDDEN], dt.float32, kind="ExternalInput")
    wout_d = nc.dram_tensor("wout", [HIDDEN, DIM], dt.float32, kind="ExternalInput")
    fcw_d = nc.dram_tensor("fcw", [1, 12], dt.float32, kind="ExternalInput")
    fcb_d = nc.dram_tensor("fcb", [1, 3], dt.float32, kind="ExternalInput")
    y_out = nc.dram_tensor("y", [1, 3], dt.float32, kind="ExternalOutput")

    kv_tab = [nc.dram_tensor(f"kvtab{t}", [32768, KV_COLS], dt.float32,
                             kind="Internal", addr_space="Shared")
              for t in range(N_SUB)]
    q_tab = nc.dram_tensor("qtab", [npp, KV_COLS], dt.float32, kind="Internal")
    u_tab = [nc.dram_tensor(f"utab{t}", [npp, KV_COLS], dt.float32, kind="Internal")
             for t in range(N_SUB)]
    stage_t = [nc.dram_tensor(f"stage{t}", [npc // N_SUB, KV_COLS], dt.float32,
                              kind="Internal") for t in range(N_SUB)]
    ar_in = nc.dram_tensor("ar_in", [1, 4], dt.float32, kind="Internal")
    ar_out = nc.dram_tensor("ar_out", [1, 4], dt.float32, kind="Internal",
                            addr_space="Shared")

    nc.gpsimd.load_library(library_config.attnmlp)
    gq = [0]
    _nregs = {}

    def gather(out_ap, in_tensor, nrows, idx_ap, num_idxs, elem_size):
        """Raw InstDMAGatherAnt reading elem_size f32 from 256B-pitch rows.

        Bypasses bass's elem_size_bytes%256 assert: the ucode only requires
        the row *pitch* (elem_step bytes) to be a 256B multiple."""
        if num_idxs not in _nregs:
            _nregs[num_idxs] = nc.gpsimd.to_reg(num_idxs)
        g = nc.gpsimd
        in_ap = bass.AP(in_tensor, 0, [[KV_COLS, nrows], [1, elem_size]])
        _in_ap = g.lower_ap_dma(in_ap, for_custom_bir_dma=True)
        _idxs_ap = g.lower_ap(idx_ap)
        _out_ap = g.lower_ap(out_ap)
        g.add_instruction(mybir.InstDMAGatherAnt(
            name=nc.get_next_instruction_name(),
            ins=[*_in_ap, _idxs_ap, g.lower_val_access(_nregs[num_idxs])],
            outs=[_out_ap],
            transpose=False, num_idxs=num_idxs, elem_size=elem_size,
            stride_bytes_256=1, gen_mode=0, single_packet=False,
            queue_num=gq[0] % 4,
            sbuf_tokens_per_rank=0, sbuf_free_dim_per_rank=0,
            sbuf_free_dim_pad_per_rank=0, sbuf_byte_offset=0))
        gq[0] += 1

    def vap(base_ap, extra_off, dims):
        return bass.AP(base_ap.tensor, base_ap.offset + extra_off, dims)

    with tile.TileContext(nc) as tc:
        with (
            tc.tile_pool(name="const", bufs=1) as cpool,
            tc.tile_pool(name="res", bufs=1) as rpool,
            tc.tile_pool(name="work", bufs=2) as wpool,
            tc.tile_pool(name="gath", bufs=4) as gpool,
            tc.tile_pool(name="small", bufs=2) as spool,
            tc.tile_pool(name="psum", bufs=4, space="PSUM") as ppool,
        ):
            ident = cpool.tile([128, 128], dt.float32, name="ident")
            make_identity(nc, ident[:])
            w_in_t = cpool.tile([DIM, HIDDEN], dt.float32, name="w_in_t")
            nc.sync.dma_start(out=w_in_t[:], in_=w_in_d[:])
            wmat = {}
            for nm, dd in (("wq", wq_d), ("wk", wk_d), ("wv", wv_d), ("wo", wo_d)):
                for l in range(DEPTH):
                    w = cpool.tile([HIDDEN, HIDDEN], dt.float32, name=f"{nm}{l}")
                    nc.sync.dma_start(out=w[:], in_=dd[l])
                    wmat[(nm, l)] = w
            wout_t = cpool.tile([HIDDEN, DIM], dt.float32, name="wout_t")
            nc.sync.dma_start(out=wout_t[:], in_=wout_d[:])
            npad_t = cpool.tile([128, n_tiles], dt.float32, name="npad_t")
            nc.sync.dma_start(out=npad_t[:], in_=npad_d[:])

            zrow = cpool.tile([1, KV_COLS], dt.float32, name="zrow")
            nc.vector.memset(zrow[:], 0.0)
            for t in range(N_SUB):
                nc.sync.dma_start(out=kv_tab[t][SENT:SENT + 1, :], in_=zrow[:])

            h_all = rpool.tile([128, n_tiles, HIDDEN], dt.float32, name="h_all")
            u_tot = rpool.tile([128, n_tiles, 24], dt.float32, name="u_tot")
            u_accs = [rpool.tile([128, n_tiles, 24], dt.float32, name=f"u_acc{i}")
                      for i in range(N_SUB)]
            acc4 = rpool.tile([128, 4], dt.float32, name="acc4")

            # stage 0: h0 = x @ W_in
            xs = wpool.tile([128, n_tiles, DIM], dt.float32, tag="gt", name="xs")
            nc.sync.dma_start(out=xs[:],
                              in_=x_in[:].rearrange("(a p) f -> p a f", p=128))
            for k in range(n_tiles):
                tp = ppool.tile([DIM, 128], dt.float32, tag="tp", name="tp")
                nc.tensor.transpose(out=tp[:], in_=xs[:, k, :], identity=ident[:])
                tps = spool.tile([HIDDEN, 128], dt.float32, tag="hT", name="tps")
                nc.vector.tensor_copy(out=tps[0:DIM, :], in_=tp[:])
                hp = ppool.tile([128, HIDDEN], dt.float32, tag="mm", name="hp")
                nc.tensor.matmul(out=hp[:], lhsT=tps[0:DIM, :], rhs=w_in_t[:],
                                 start=True, stop=True)
                nc.vector.tensor_copy(out=h_all[:, k, :], in_=hp[:])

            scale = float(1.0 / np.sqrt(HEAD_DIM))

            def issue_ag(t):
                nc.gpsimd.collective_compute(
                    "AllGather", mybir.AluOpType.bypass,
                    replica_groups=[list(range(cfg["n_cores"]))],
                    ins=[stage_t[t][:]],
                    outs=[kv_tab[t][0:cfg["n_cores"] * pcs, :]])

            for layer in range(DEPTH):
                # q/k/v projections; stage written per-subtable so each
                # AllGather depends only on its own slab
                for k in range(n_tiles):
                    tp = ppool.tile([HIDDEN, 128], dt.float32, tag="tp", name="htp")
                    nc.tensor.transpose(out=tp[:], in_=h_all[:, k, :],
                                        identity=ident[:])
                    hT = spool.tile([HIDDEN, 128], dt.float32, tag="hT", name="hT")
                    nc.vector.tensor_copy(out=hT[:], in_=tp[:])
                    qp = ppool.tile([128, HIDDEN], dt.float32, tag="mm", name="qp")
                    nc.tensor.matmul(out=qp[:], lhsT=hT[:], rhs=wmat[("wq", layer)][:],
                                     start=True, stop=True)
                    qs = spool.tile([128, HIDDEN], dt.float32, tag="qs", name="qs")
                    nc.vector.tensor_copy(out=qs[:], in_=qp[:])
                    nc.sync.dma_start(out=q_tab[k * 128:(k + 1) * 128, 0:HIDDEN],
                                      in_=qs[:])
                    kp = ppool.tile([128, HIDDEN], dt.float32, tag="mm", name="kp")
                    nc.tensor.matmul(out=kp[:], lhsT=hT[:], rhs=wmat[("wk", layer)][:],
                                     start=True, stop=True)
                    kvs = spool.tile([128, 32], dt.float32, tag="kvs", name="kvs")
                    nc.vector.tensor_copy(out=kvs[:, 0:16], in_=kp[:])
                    vp = ppool.tile([128, HIDDEN], dt.float32, tag="mm", name="vp")
                    nc.tensor.matmul(out=vp[:], lhsT=hT[:], rhs=wmat[("wv", layer)][:],
                                     start=True, stop=True)
                    nc.vector.tensor_copy(out=kvs[:, 16:32], in_=vp[:])
                    lo, hi = k * 128, min((k + 1) * 128, npc)
                    r = lo
                    while r < hi:
                        t = r // pcs
                        e = min(hi, (t + 1) * pcs)
                        nc.sync.dma_start(
                            out=stage_t[t][r - t * pcs:e - t * pcs, 0:32],
                            in_=kvs[r - lo:e - lo, :])
                        r = e
                issue_ag(0)
                issue_ag(1)

                for t in range(N_SUB):
                    u_acc = u_accs[t]
                    nc.vector.memset(u_acc[:], 0.0)
                    # pass-wide q gather: q_tab permuted into rank order
                    qt = spool.tile([128, npp // 16], dt.int16, tag="qt", name="qt")
                    nc.sync.dma_start(
                        out=qt[:], in_=qidx_d[:, t * (npp // 16):(t + 1) * (npp // 16)])
                    qg = wpool.tile([128, n_tiles, HIDDEN], dt.float32,
                                    tag="qgp", name="qgp")
                    done = 0
                    while done < n_tiles:
                        cnt = min(32, n_tiles - done)
                        gather(qg[:, done:done + cnt, :], q_tab, npp,
                               qt[:, done * 8:(done + cnt) * 8], cnt * 128, HIDDEN)
                        done += cnt
                    qgp = qg[:]
                    pq = qgp.ap[0][0]
                    for (tt, r0, S, D, gc0, gcols) in chunk_meta:
                        if tt != t:
                            continue
                        sl = r0 // 128
                        gt = wpool.tile([128, 1024], dt.int16, tag="gt", name="gt")
                        nc.sync.dma_start(out=gt[:, 0:gcols],
                                          in_=gidx_d[:, gc0:gc0 + gcols])
                        kvg = gpool.tile([128, SD_CAP, 32], dt.float32,
                                         tag="kvg", name="kvg")
                        nrow = S * D
                        done = 0
                        while done < nrow:
                            cnt = min(32, nrow - done)
                            gather(kvg[:, done:done + cnt, :], kv_tab[t], 32768,
                                   gt[:, done * 8:(done + cnt) * 8], cnt * 128, 32)
                            done += cnt
                        kvga = kvg[:]
                        qga = vap(qgp, sl * HIDDEN,
                                  [[pq, 128], [HIDDEN, S], [1, HIDDEN]])
                        pk = kvga.ap[0][0]
                        prod = wpool.tile([128, S * HIDDEN, D],
                                          dt.float32, tag="prod", name="prod")
                        pp = prod[:].ap[0][0]
                        nc.vector.tensor_tensor(
                            out=vap(prod[:], 0,
                                    [[pp, 128], [HIDDEN * D, S], [D, HIDDEN], [1, D]]),
                            in0=vap(qga, 0,
                                    [[pq, 128], [HIDDEN, S], [1, HIDDEN], [0, D]]),
                            in1=vap(kvga, 0,
                                    [[pk, 128], [D * 32, S], [1, HIDDEN],
                                     [32, D]]),
                            op=OP.mult)
                        wgt = wpool.tile([128, S * HEADS, D],
                                         dt.float32, tag="wgt", name="wgt")
                        pw = wgt[:].ap[0][0]
                        nc.vector.tensor_tensor(
                            out=vap(wgt[:], 0,
                                    [[pw, 128], [HEADS * D, S], [D, HEADS], [1, D]]),
                            in0=vap(prod[:], 0,
                                    [[pp, 128], [HIDDEN * D, S], [D, HEADS], [1, D]]),
                            in1=vap(prod[:], HEADS * D,
                                    [[pp, 128], [HIDDEN * D, S], [D, HEADS], [1, D]]),
                            op=OP.add)
                        nc.scalar.activation(
                            out=vap(wgt[:], 0, [[pw, 128], [1, S * HEADS * D]]),
                            in_=vap(wgt[:], 0, [[pw, 128], [1, S * HEADS * D]]),
                            func=ACTF.Exp, scale=scale)
                        nc.vector.tensor_reduce(
                            out=u_acc[:, sl:sl + S, 16:24],
                            in_=vap(wgt[:], 0,
                                    [[pw, 128], [D, S * HEADS], [1, D]]),
                            axis=AX.X, op=OP.add)
                        msg = wpool.tile([128, S * HIDDEN, D],
                                         dt.float32, tag="prod", name="msg")
                        pm = msg[:].ap[0][0]
                        nc.vector.tensor_tensor(
                            out=vap(msg[:], 0,
                                    [[pm, 128], [HIDDEN * D, S], [HEAD_DIM * D, HEADS],
                                     [D, HEAD_DIM], [1, D]]),
                            in0=vap(wgt[:], 0,
                                    [[pw, 128], [HEADS * D, S], [D, HEADS],
                                     [0, HEAD_DIM], [1, D]]),
                            in1=vap(kvga, 16,
                                    [[pk, 128], [D * 32, S], [HEAD_DIM, HEADS],
                                     [1, HEAD_DIM], [32, D]]),
                            op=OP.mult)
                        nc.vector.tensor_reduce(
                            out=u_acc[:, sl:sl + S, 0:16],
                            in_=vap(msg[:], 0,
                                    [[pm, 128], [D, S * HIDDEN], [1, D]]),
                            axis=AX.X, op=OP.add)
                    # u_acc (rank order) -> DRAM; recombination deferred to
                    # layer end so pass t+1 gathers are never blocked
                    nc.sync.dma_start(
                        out=bass.AP(u_tab[t], 0,
                                    [[KV_COLS, 128], [128 * KV_COLS, n_tiles],
                                     [1, 24]]),
                        in_=u_acc[:])
                    if t + 2 < N_SUB:
                        issue_ag(t + 2)

                # recombine all passes into identity order
                nc.vector.memset(u_tot[:], 0.0)
                for t in range(N_SUB):
                    ct = spool.tile([128, npp // 16], dt.int16, tag="ct", name="ct")
                    nc.sync.dma_start(out=ct[:], in_=cidx_d[t])
                    done = 0
                    while done < n_tiles:
                        cnt = min(32, n_tiles - done)
                        cg = spool.tile([128, 32, 24], dt.float32, tag="cgt",
                                        name="cg")
                        gather(cg[:, 0:cnt, :], u_tab[t], npp,
                               ct[:, done * 8:(done + cnt) * 8], cnt * 128, 24)
                        nc.vector.tensor_tensor(
                            out=u_tot[:, done:done + cnt, :],
                            in0=u_tot[:, done:done + cnt, :],
                            in1=cg[:, 0:cnt, :], op=OP.add)
                        done += cnt

                # epilogue
                dadj = spool.tile([128, n_tiles, HEADS], dt.float32, tag="dadj",
                                  name="dadj")
                pn = npad_t[:].ap[0][0]
                nc.vector.tensor_tensor(
                    out=dadj[:], in0=u_tot[:, :, 16:24],
                    in1=vap(npad_t[:], 0, [[pn, 128], [1, n_tiles], [0, HEADS]]),
                    op=OP.subtract)
                nc.vector.tensor_scalar_add(out=dadj[:], in0=dadj[:], scalar1=1e-9)
                rden = spool.tile([128, n_tiles, HEADS], dt.float32, tag="rden",
                                  name="rden")
                nc.vector.reciprocal(out=rden[:], in_=dadj[:])
                agg = wpool.tile([128, n_tiles, HIDDEN], dt.float32, tag="wgt",
                                 name="agg")
                pr_ = rden[:].ap[0][0]
                pa = agg[:].ap[0][0]
                nc.vector.tensor_tensor(
                    out=vap(agg[:], 0,
                            [[pa, 128], [HIDDEN, n_tiles], [HEAD_DIM, HEADS],
                             [1, HEAD_DIM]]),
                    in0=vap(u_tot[:], 0,
                            [[u_tot[:].ap[0][0], 128], [24, n_tiles],
                             [HEAD_DIM, HEADS], [1, HEAD_DIM]]),
                    in1=vap(rden[:], 0,
                            [[pr_, 128], [HEADS, n_tiles], [1, HEADS],
                             [0, HEAD_DIM]]),
                    op=OP.mult)
                hnew = wpool.tile([128, n_tiles, HIDDEN], dt.float32, tag="prod",
                                  name="hnew")
                for k in range(n_tiles):
                    tp = ppool.tile([HIDDEN, 128], dt.float32, tag="tp", name="atp")
                    nc.tensor.transpose(out=tp[:], in_=agg[:, k, :],
                                        identity=ident[:])
                    aT = spool.tile([HIDDEN, 128], dt.float32, tag="hT", name="aT")
                    nc.vector.tensor_copy(out=aT[:], in_=tp[:])
                    op_ = ppool.tile([128, HIDDEN], dt.float32, tag="mm", name="op_")
                    nc.tensor.matmul(out=op_[:], lhsT=aT[:],
                                     rhs=wmat[("wo", layer)][:],
                                     start=True, stop=True)
                    nc.vector.tensor_tensor(out=hnew[:, k, :], in0=op_[:],
                                            in1=h_all[:, k, :], op=OP.add)
                mu = spool.tile([128, n_tiles, 1], dt.float32, tag="mu", name="mu")
                nc.vector.tensor_reduce(out=mu[:], in_=hnew[:], axis=AX.X, op=OP.add)
                nc.vector.tensor_scalar_mul(out=mu[:], in0=mu[:], scalar1=1.0 / 16)
                cent = wpool.tile([128, n_tiles, HIDDEN], dt.float32, tag="wgt",
                                  name="cent")
                nc.vector.tensor_tensor(
                    out=cent[:], in0=hnew[:],
                    in1=vap(mu[:], 0, [[mu[:].ap[0][0], 128], [1, n_tiles],
                                       [0, HIDDEN]]),
                    op=OP.subtract)
                sq = wpool.tile([128, n_tiles, HIDDEN], dt.float32, tag="prod",
                                name="sq")
                nc.vector.tensor_tensor(out=sq[:], in0=cent[:], in1=cent[:],
                                        op=OP.mult)
                var = spool.tile([128, n_tiles, 1], dt.float32, tag="var", name="var")
                nc.vector.tensor_reduce(out=var[:], in_=sq[:], axis=AX.X, op=OP.add)
                nc.vector.tensor_scalar_mul(out=var[:], in0=var[:], scalar1=1.0 / 16)
                nc.vector.tensor_scalar_add(out=var[:], in0=var[:], scalar1=1e-5)
                rs = spool.tile([128, n_tiles, 1], dt.float32, tag="rs", name="rs")
                nc.vector.reciprocal(out=rs[:], in_=var[:])
                nc.scalar.activation(out=rs[:], in_=rs[:], func=ACTF.Sqrt)
                nc.vector.tensor_tensor(
                    out=h_all[:], in0=cent[:],
                    in1=vap(rs[:], 0, [[rs[:].ap[0][0], 128], [1, n_tiles],
                                       [0, HIDDEN]]),
                    op=OP.mult)

            # final head
            nc.vector.memset(acc4[:], 0.0)
            for k in range(n_tiles):
                tp = ppool.tile([HIDDEN, 128], dt.float32, tag="tp", name="ftp")
                nc.tensor.transpose(out=tp[:], in_=h_all[:, k, :], identity=ident[:])
                hT = spool.tile([HIDDEN, 128], dt.float32, tag="hT", name="fhT")
                nc.vector.tensor_copy(out=hT[:], in_=tp[:])
                gp = ppool.tile([128, DIM], dt.float32, tag="mm", name="gp")
                nc.tensor.matmul(out=gp[:], lhsT=hT[:], rhs=wout_t[:],
                                 start=True, stop=True)
                nc.vector.tensor_tensor(out=acc4[:], in0=acc4[:], in1=gp[:],
                                        op=OP.add)
            onesk = cpool.tile([128, 1], dt.float32, name="onesk")
            nc.vector.memset(onesk[:], 1.0 / cfg["n_nodes"])
            pooled_p = ppool.tile([1, 4], dt.float32, tag="mm", name="pooled_p")
            nc.tensor.matmul(out=pooled_p[:], lhsT=onesk[:], rhs=acc4[:],
                             start=True, stop=True)
            pooled_s = spool.tile([1, 4], dt.float32, tag="p4", name="pooled_s")
            nc.vector.tensor_copy(out=pooled_s[:], in_=pooled_p[:])
            nc.sync.dma_start(out=ar_in[:], in_=pooled_s[:])
            nc.gpsimd.collective_compute(
                "AllReduce", mybir.AluOpType.add,
                replica_groups=[list(range(cfg["n_cores"]))],
                ins=[ar_in[:]], outs=[ar_out[:]])
            pooled = spool.tile([1, 4], dt.float32, tag="p4b", name="pooled")
            nc.sync.dma_start(out=pooled[:], in_=ar_out[:])
            fcw_t = spool.tile([1, 12], dt.float32, tag="fcw", name="fcw_t")
            nc.sync.dma_start(out=fcw_t[:], in_=fcw_d[:])
            fcb_t = spool.tile([1, 3], dt.float32, tag="fcb", name="fcb_t")
            nc.sync.dma_start(out=fcb_t[:], in_=fcb_d[:])
            pr2 = spool.tile([1, 12], dt.float32, tag="pr2", name="pr2")
            nc.vector.tensor_tensor(
                out=pr2[:],
                in0=vap(pooled[:], 0, [[pooled[:].ap[0][0], 1], [0, 3], [1, 4]]),
                in1=vap(fcw_t[:], 0, [[fcw_t[:].ap[0][0], 1], [4, 3], [1, 4]]),
                op=OP.mult)
            y3 = spool.tile([1, 3], dt.float32, tag="y3", name="y3")
            nc.vector.tensor_reduce(
                out=y3[:],
                in_=vap(pr2[:], 0, [[pr2[:].ap[0][0], 1], [4, 3], [1, 4]]),
                axis=AX.X, op=OP.add)
            nc.vector.tensor_tensor(out=y3[:], in0=y3[:], in1=fcb_t[:], op=OP.add)
            nc.sync.dma_start(out=y_out[:], in_=y3[:])

    _split_excess_waits(nc, max_waits=1)
    lower_extended_insts(nc)
    return nc


def kernel(x, edge_index, W_in, Wq, Wk, Wv, Wo, W_out, fc_w, fc_b):
    x = np.asarray(x, dtype=np.float32)
    edge_index = np.asarray(edge_index)
    cfg = _cfg(x.shape[0])

    key = ("nc", x.shape[0], edge_index.shape[1])
    if key not in _RUN_CACHE:
        schedule, plans = _plan(edge_index, cfg)
        meta = (plans[0]["gidx"].shape[1], plans[0]["qidx"].shape[1],
                plans[0]["chunk_meta"])
        nc = _build(meta, cfg)
        _RUN_CACHE[key] = (nc, plans)
    nc, plans = _RUN_CACHE[key]

    wts = _permute_weights(
        np.asarray(W_in, np.float32), np.asarray(Wq, np.float32),
        np.asarray(Wk, np.float32), np.asarray(Wv, np.float32),
        np.asarray(Wo, np.float32), np.asarray(W_out, np.float32),
        np.asarray(fc_w, np.float32), np.asarray(fc_b, np.float32))

    nps, pcs, npp = cfg["n_per_sub"], cfg["per_core_sub"], cfg["npp"]
    old = np.arange(cfg["n_nodes"])
    c_of = (old % nps) // pcs
    l_of = (old // nps) * pcs + (old % pcs)
    in_maps = []
    for c in range(cfg["n_cores"]):
        xl = np.zeros((npp, DIM), dtype=np.float32)
        m = c_of == c
        xl[l_of[m]] = x[m]
        p = plans[c]
        in_maps.append(dict(
            x_in=xl, gidx=p["gidx"], qidx=p["qidx"], cidx=p["cidx"],
            npad=p["npad"], w_in=wts["W_in"], wq=wts["Wq"], wk=wts["Wk"],
            wv=wts["Wv"], wo=wts["Wo"], wout=wts["W_out"], fcw=wts["fcw"],
            fcb=wts["fcb"]))

    from concourse.bass_utils import run_bass_kernel_spmd
    trace = bool(os.environ.get("GNN_TRACE"))
    if trace:
        _install_profhook()
    res = run_bass_kernel_spmd(nc, in_maps, core_ids=list(range(cfg["n_cores"])),
                               trace=trace)
    if trace:
        _RUN_CACHE["last_result"] = res
    return np.asarray(res.results[0]["y"]).reshape(3).astype(np.float32)



# revision 25
# speedup vs baseline: 1.2406x; 1.1330x over previous
"""GNN message-passing (SE3-style graph attention) kernel for 8 Trainium2 cores.

Edge-parallel strategy:
- Nodes relabeled into 4 "subtables" x 8 cores so per-edge kv-gather indices
  fit int16 (dma_gather requirement). Each core owns 12500 dst nodes.
- Per layer: on-device q/k/v projections -> 4 AllGathers build global kv
  tables -> 4 passes over src subtables, each processing edges in node-major
  degree-sorted chunks: dma_gather kv rows, affine q broadcast, DVE
  scores/exp/messages, affine segment reduction into per-pass accumulators.
- Pass accumulators (degree-rank order) recombined into identity order by
  small dma_gathers, then divide / Wo project / residual / LayerNorm.
- Final: W_out, mean-pool via PE ones-matmul, AllReduce, FC head.
"""

import os
import sys
import types
import numpy as np

HEADS = 8
HEAD_DIM = 2
HIDDEN = 16
DIM = 4
DEPTH = 2
N_SUB = 4
KV_COLS = 64          # table row = 64 fp32 = 256B
SENT = 32767          # zeroed sentinel row in each kv subtable
SD_CAP = 128          # max S*D per chunk
S_CAP = 8

_RUN_CACHE = {}


# --------------------------------------------------------------------------
# harness shims (self-contained copies)
# --------------------------------------------------------------------------
def _split_excess_waits(nc, max_waits=1):
    """Walrus build allows 1 sync-wait per instruction; move extras to NOPs."""
    import concourse.mybir as mybir
    n = [0]
    for blk in nc.m.functions[0].blocks:
        new_insts = []
        for inst in blk.instructions:
            si = inst.sync_info
            if si is not None and len(si.on_wait) > max_waits:
                waits = list(si.on_wait)
                extra, keep = waits[:-max_waits], waits[-max_waits:]
                for i in range(0, len(extra), max_waits):
                    n[0] += 1
                    nop = mybir.InstNoOp(
                        name=f"IWS-{n[0]}", engine=inst.engine, ins=[], outs=[],
                        sync_info=mybir.SyncInfo(on_wait=extra[i:i + max_waits],
                                                 on_update=[]))
                    try:
                        nc.register_instruction(nop, overwrite=True)
                    except Exception:
                        pass
                    new_insts.append(nop)
                si.on_wait = keep
            new_insts.append(inst)
        blk.instructions[:] = new_insts


def _install_profhook():
    if 'antenv.axon_hooks' in sys.modules:
        return
    try:
        import antenv
        from trn_agent_boot.trn_boot import _ntff_profile_via_ctypes
        hook = _ntff_profile_via_ctypes('/opt/axon/libaxon_pjrt.so')
        mod = types.ModuleType('antenv.axon_hooks')
        state = {'hook': hook}
        mod.set_axon_ntff_profile_hook = lambda h: state.__setitem__('hook', h)
        mod.get_axon_ntff_profile_hook = lambda: state['hook']
        sys.modules['antenv.axon_hooks'] = mod
        antenv.axon_hooks = mod
    except Exception:
        pass


# --------------------------------------------------------------------------
# host-side planning
# --------------------------------------------------------------------------
def _cfg(n_nodes):
    n_cores = 8
    n_per_sub = n_nodes // N_SUB
    per_core_sub = n_per_sub // n_cores
    npc = N_SUB * per_core_sub
    npp = ((npc + 127) // 128) * 128
    return dict(n_nodes=n_nodes, n_cores=n_cores, n_per_sub=n_per_sub,
                per_core_sub=per_core_sub, npc=npc, npp=npp,
                n_tiles=npp // 128)


def _pack_gidx(idx_flat):
    """Gather feed position i lives at tile[i%16, i//16]; replicate x8 cores."""
    n = idx_flat.shape[0]
    assert n % 16 == 0
    tile16 = np.ascontiguousarray(
        idx_flat.reshape(n // 16, 16).T.astype(np.int16))
    return np.tile(tile16, (8, 1))


def _plan(edge_index, cfg):
    src = np.asarray(edge_index[0], dtype=np.int64)
    dst = np.asarray(edge_index[1], dtype=np.int64)
    nps, pcs = cfg["n_per_sub"], cfg["per_core_sub"]
    npp, n_cores = cfg["npp"], cfg["n_cores"]

    e_core = (dst % nps) // pcs
    e_l = (dst // nps) * pcs + (dst % pcs)
    e_t = src // nps
    e_row = src % nps

    passes = []
    for t in range(N_SUB):
        per_core = []
        for c in range(n_cores):
            m = (e_core == c) & (e_t == t)
            lt, rowt = e_l[m], e_row[m]
            deg = np.bincount(lt, minlength=npp).astype(np.int64)
            order = np.argsort(-deg, kind="stable")
            rank_of = np.empty(npp, dtype=np.int64)
            rank_of[order] = np.arange(npp)
            eorder = np.argsort(rank_of[lt], kind="stable")
            per_core.append(dict(deg=deg, order=order, rank_of=rank_of,
                                 lt=lt[eorder], rowt=rowt[eorder]))
        passes.append(per_core)

    schedule = []
    for t in range(N_SUB):
        chunks = []
        r0 = 0
        degs_sorted = [passes[t][c]["deg"][passes[t][c]["order"]]
                       for c in range(n_cores)]
        while r0 < npp:
            D = int(max(int(d[r0]) for d in degs_sorted))
            if D == 0:
                break
            S = max(1, min(S_CAP, SD_CAP // D, (npp - r0) // 128))
            chunks.append((r0, S, D))
            r0 += 128 * S
        schedule.append(chunks)

    plans = []
    for c in range(n_cores):
        gidx_cols, qidx_list, chunk_meta = [], [], []
        npad = np.zeros(npp, dtype=np.float64)
        gcol0 = 0
        for t in range(N_SUB):
            pc = passes[t][c]
            deg, order, rank_of = pc["deg"], pc["order"], pc["rank_of"]
            lt, rowt = pc["lt"], pc["rowt"]
            offs = np.zeros(npp + 1, dtype=np.int64)
            offs[1:] = np.cumsum(deg[order])
            ranks_e = rank_of[lt]
            j_e = np.arange(lt.shape[0]) - offs[ranks_e]
            # one pass-wide q index block: full rank order
            qidx_list.append(_pack_gidx(order[0:npp]))
            for ci, (r0, S, D) in enumerate(schedule[t]):
                nrows = 128 * S * D
                idx_flat = np.full(nrows, SENT, dtype=np.int64)
                em = (ranks_e >= r0) & (ranks_e < r0 + 128 * S)
                q = ranks_e[em] - r0
                pos = ((q // 128) * D + j_e[em]) * 128 + (q % 128)
                idx_flat[pos] = rowt[em]
                gidx_cols.append(_pack_gidx(idx_flat))
                ch_nodes = order[r0:r0 + 128 * S]
                npad[ch_nodes] += D - deg[ch_nodes]
                chunk_meta.append((t, r0, S, D, gcol0, nrows // 16))
                gcol0 += nrows // 16
        gidx = (np.concatenate(gidx_cols, axis=1) if gidx_cols
                else np.zeros((128, 16), np.int16))
        qidx = (np.concatenate(qidx_list, axis=1) if qidx_list
                else np.zeros((128, 8), np.int16))
        cidx = np.stack([_pack_gidx(passes[t][c]["rank_of"][:npp])
                         for t in range(N_SUB)])
        npad_t = np.ascontiguousarray(
            npad.reshape(cfg["n_tiles"], 128).T.astype(np.float32))
        plans.append(dict(gidx=gidx, qidx=qidx, cidx=cidx, npad=npad_t,
                          chunk_meta=chunk_meta))
    return schedule, plans


def _permute_weights(W_in, Wq, Wk, Wv, Wo, W_out, fc_w, fc_b):
    perm = np.array([h * HEAD_DIM + d for d in range(HEAD_DIM)
                     for h in range(HEADS)], dtype=np.int64)
    return dict(W_in=np.ascontiguousarray(W_in),
                Wq=np.ascontiguousarray(Wq[:, :, perm]),
                Wk=np.ascontiguousarray(Wk[:, :, perm]),
                Wv=np.ascontiguousarray(Wv),
                Wo=np.ascontiguousarray(Wo),
                W_out=np.ascontiguousarray(W_out),
                fcw=np.ascontiguousarray(fc_w.T).reshape(1, 12).astype(np.float32),
                fcb=fc_b.reshape(1, 3).astype(np.float32))


# --------------------------------------------------------------------------
# device program
# --------------------------------------------------------------------------
def _build(meta, cfg):
    import concourse.bass as bass
    import concourse.mybir as mybir
    import concourse.tile as tile
    from concourse import library_config
    from concourse.masks import make_identity
    from concourse.library_overlay import lower_extended_insts

    dt = mybir.dt
    AX = mybir.AxisListType
    OP = mybir.AluOpType
    ACTF = mybir.ActivationFunctionType
    npp, n_tiles, npc = cfg["npp"], cfg["n_tiles"], cfg["npc"]
    pcs = cfg["per_core_sub"]
    g_cols_total, q_cols_total, chunk_meta = meta

    nc = bass.Bass(num_devices=cfg["n_cores"], num_swdge_queues=4)

    x_in = nc.dram_tensor("x_in", [npp, DIM], dt.float32, kind="ExternalInput")
    gidx_d = nc.dram_tensor("gidx", [128, g_cols_total], dt.int16, kind="ExternalInput")
    qidx_d = nc.dram_tensor("qidx", [128, q_cols_total], dt.int16, kind="ExternalInput")
    cidx_d = nc.dram_tensor("cidx", [N_SUB, 128, npp // 16], dt.int16, kind="ExternalInput")
    npad_d = nc.dram_tensor("npad", [128, n_tiles], dt.float32, kind="ExternalInput")
    w_in_d = nc.dram_tensor("w_in", [DIM, HIDDEN], dt.float32, kind="ExternalInput")
    wq_d = nc.dram_tensor("wq", [DEPTH, HIDDEN, HIDDEN], dt.float32, kind="ExternalInput")
    wk_d = nc.dram_tensor("wk", [DEPTH, HIDDEN, HIDDEN], dt.float32, kind="ExternalInput")
    wv_d = nc.dram_tensor("wv", [DEPTH, HIDDEN, HIDDEN], dt.float32, kind="ExternalInput")
    wo_d = nc.dram_tensor("wo", [DEPTH, HIDDEN, HIDDEN], dt.float32, kind="ExternalInput")
    wout_d = nc.dram_tensor("wout", [HIDDEN, DIM], dt.float32, kind="ExternalInput")
    fcw_d = nc.dram_tensor("fcw", [1, 12], dt.float32, kind="ExternalInput")
    fcb_d = nc.dram_tensor("fcb", [1, 3], dt.float32, kind="ExternalInput")
    y_out = nc.dram_tensor("y", [1, 3], dt.float32, kind="ExternalOutput")

    kv_tab = [nc.dram_tensor(f"kvtab{t}", [32768, 128], dt.bfloat16,
                             kind="Internal", addr_space="Shared")
              for t in range(N_SUB)]
    q_tab = nc.dram_tensor("qtab", [npp, 128], dt.bfloat16, kind="Internal")
    u_tab = [nc.dram_tensor(f"utab{t}", [npp, KV_COLS], dt.float32, kind="Internal")
             for t in range(N_SUB)]
    stage_t = [nc.dram_tensor(f"stage{t}", [npc // N_SUB, 128], dt.bfloat16,
                              kind="Internal") for t in range(N_SUB)]
    ar_in = nc.dram_tensor("ar_in", [1, 4], dt.float32, kind="Internal")
    ar_out = nc.dram_tensor("ar_out", [1, 4], dt.float32, kind="Internal",
                            addr_space="Shared")

    nc.gpsimd.load_library(library_config.attnmlp)
    gq = [0]
    _nregs = {}

    def gather(out_ap, in_tensor, nrows, idx_ap, num_idxs, elem_size,
               pitch=KV_COLS):
        """Raw InstDMAGatherAnt reading elem_size elems from 256B-pitch rows.

        Bypasses bass's elem_size_bytes%256 assert: the ucode only requires
        the row *pitch* (elem_step bytes) to be a 256B multiple."""
        if num_idxs not in _nregs:
            _nregs[num_idxs] = nc.gpsimd.to_reg(num_idxs)
        g = nc.gpsimd
        in_ap = bass.AP(in_tensor, 0, [[pitch, nrows], [1, elem_size]])
        _in_ap = g.lower_ap_dma(in_ap, for_custom_bir_dma=True)
        _idxs_ap = g.lower_ap(idx_ap)
        _out_ap = g.lower_ap(out_ap)
        g.add_instruction(mybir.InstDMAGatherAnt(
            name=nc.get_next_instruction_name(),
            ins=[*_in_ap, _idxs_ap, g.lower_val_access(_nregs[num_idxs])],
            outs=[_out_ap],
            transpose=False, num_idxs=num_idxs, elem_size=elem_size,
            stride_bytes_256=1, gen_mode=0, single_packet=False,
            queue_num=gq[0] % 4,
            sbuf_tokens_per_rank=0, sbuf_free_dim_per_rank=0,
            sbuf_free_dim_pad_per_rank=0, sbuf_byte_offset=0))
        gq[0] += 1

    def vap(base_ap, extra_off, dims):
        return bass.AP(base_ap.tensor, base_ap.offset + extra_off, dims)

    with tile.TileContext(nc) as tc:
        with (
            tc.tile_pool(name="const", bufs=1) as cpool,
            tc.tile_pool(name="res", bufs=1) as rpool,
            tc.tile_pool(name="work", bufs=2) as wpool,
            tc.tile_pool(name="gath", bufs=4) as gpool,
            tc.tile_pool(name="small", bufs=2) as spool,
            tc.tile_pool(name="psum", bufs=4, space="PSUM") as ppool,
        ):
            ident = cpool.tile([128, 128], dt.float32, name="ident")
            make_identity(nc, ident[:])
            w_in_t = cpool.tile([DIM, HIDDEN], dt.float32, name="w_in_t")
            nc.sync.dma_start(out=w_in_t[:], in_=w_in_d[:])
            wmat = {}
            for nm, dd in (("wq", wq_d), ("wk", wk_d), ("wv", wv_d), ("wo", wo_d)):
                for l in range(DEPTH):
                    w = cpool.tile([HIDDEN, HIDDEN], dt.float32, name=f"{nm}{l}")
                    nc.sync.dma_start(out=w[:], in_=dd[l])
                    wmat[(nm, l)] = w
            wout_t = cpool.tile([HIDDEN, DIM], dt.float32, name="wout_t")
            nc.sync.dma_start(out=wout_t[:], in_=wout_d[:])
            npad_t = cpool.tile([128, n_tiles], dt.float32, name="npad_t")
            nc.sync.dma_start(out=npad_t[:], in_=npad_d[:])

            zrow = cpool.tile([1, 128], dt.bfloat16, name="zrow")
            nc.vector.memset(zrow[:], 0.0)
            for t in range(N_SUB):
                nc.sync.dma_start(out=kv_tab[t][SENT:SENT + 1, :], in_=zrow[:])

            h_all = rpool.tile([128, n_tiles, HIDDEN], dt.float32, name="h_all")
            u_tot = rpool.tile([128, n_tiles, 24], dt.float32, name="u_tot")
            u_accs = [rpool.tile([128, n_tiles, 24], dt.float32, name=f"u_acc{i}")
                      for i in range(N_SUB)]
            acc4 = rpool.tile([128, 4], dt.float32, name="acc4")

            # stage 0: h0 = x @ W_in
            xs = wpool.tile([128, n_tiles, DIM], dt.float32, tag="gt", name="xs")
            nc.sync.dma_start(out=xs[:],
                              in_=x_in[:].rearrange("(a p) f -> p a f", p=128))
            for k in range(n_tiles):
                tp = ppool.tile([DIM, 128], dt.float32, tag="tp", name="tp")
                nc.tensor.transpose(out=tp[:], in_=xs[:, k, :], identity=ident[:])
                tps = spool.tile([HIDDEN, 128], dt.float32, tag="hT", name="tps")
                nc.vector.tensor_copy(out=tps[0:DIM, :], in_=tp[:])
                hp = ppool.tile([128, HIDDEN], dt.float32, tag="mm", name="hp")
                nc.tensor.matmul(out=hp[:], lhsT=tps[0:DIM, :], rhs=w_in_t[:],
                                 start=True, stop=True)
                nc.vector.tensor_copy(out=h_all[:, k, :], in_=hp[:])

            scale = float(1.0 / np.sqrt(HEAD_DIM))

            def issue_ag(t):
                nc.gpsimd.collective_compute(
                    "AllGather", mybir.AluOpType.bypass,
                    replica_groups=[list(range(cfg["n_cores"]))],
                    ins=[stage_t[t][:]],
                    outs=[kv_tab[t][0:cfg["n_cores"] * pcs, :]])

            for layer in range(DEPTH):
                # q/k/v projections; stage written per-subtable so each
                # AllGather depends only on its own slab
                for k in range(n_tiles):
                    tp = ppool.tile([HIDDEN, 128], dt.float32, tag="tp", name="htp")
                    nc.tensor.transpose(out=tp[:], in_=h_all[:, k, :],
                                        identity=ident[:])
                    hT = spool.tile([HIDDEN, 128], dt.float32, tag="hT", name="hT")
                    nc.vector.tensor_copy(out=hT[:], in_=tp[:])
                    qp = ppool.tile([128, HIDDEN], dt.float32, tag="mm", name="qp")
                    nc.tensor.matmul(out=qp[:], lhsT=hT[:], rhs=wmat[("wq", layer)][:],
                                     start=True, stop=True)
                    qs = spool.tile([128, HIDDEN], dt.bfloat16, tag="qs", name="qs")
                    nc.vector.tensor_copy(out=qs[:], in_=qp[:])
                    nc.sync.dma_start(out=q_tab[k * 128:(k + 1) * 128, 0:HIDDEN],
                                      in_=qs[:])
                    kp = ppool.tile([128, HIDDEN], dt.float32, tag="mm", name="kp")
                    nc.tensor.matmul(out=kp[:], lhsT=hT[:], rhs=wmat[("wk", layer)][:],
                                     start=True, stop=True)
                    kvs = spool.tile([128, 32], dt.bfloat16, tag="kvs", name="kvs")
                    nc.vector.tensor_copy(out=kvs[:, 0:16], in_=kp[:])
                    vp = ppool.tile([128, HIDDEN], dt.float32, tag="mm", name="vp")
                    nc.tensor.matmul(out=vp[:], lhsT=hT[:], rhs=wmat[("wv", layer)][:],
                                     start=True, stop=True)
                    nc.vector.tensor_copy(out=kvs[:, 16:32], in_=vp[:])
                    lo, hi = k * 128, min((k + 1) * 128, npc)
                    r = lo
                    while r < hi:
                        t = r // pcs
                        e = min(hi, (t + 1) * pcs)
                        nc.sync.dma_start(
                            out=stage_t[t][r - t * pcs:e - t * pcs, 0:32],
                            in_=kvs[r - lo:e - lo, :])
                        r = e
                issue_ag(0)
                issue_ag(1)

                for t in range(N_SUB):
                    u_acc = u_accs[t]
                    nc.vector.memset(u_acc[:], 0.0)
                    # pass-wide q gather: q_tab permuted into rank order
                    qt = spool.tile([128, npp // 16], dt.int16, tag="qt", name="qt")
                    nc.sync.dma_start(
                        out=qt[:], in_=qidx_d[:, t * (npp // 16):(t + 1) * (npp // 16)])
                    qg = wpool.tile([128, n_tiles, HIDDEN], dt.bfloat16,
                                    tag="qgp", name="qgp")
                    done = 0
                    while done < n_tiles:
                        cnt = min(32, n_tiles - done)
                        gather(qg[:, done:done + cnt, :], q_tab, npp,
                               qt[:, done * 8:(done + cnt) * 8], cnt * 128, HIDDEN,
                               pitch=128)
                        done += cnt
                    qgp = qg[:]
                    pq = qgp.ap[0][0]
                    for (tt, r0, S, D, gc0, gcols) in chunk_meta:
                        if tt != t:
                            continue
                        sl = r0 // 128
                        gt = wpool.tile([128, 1024], dt.int16, tag="gt", name="gt")
                        nc.sync.dma_start(out=gt[:, 0:gcols],
                                          in_=gidx_d[:, gc0:gc0 + gcols])
                        kvg = gpool.tile([128, SD_CAP, 32], dt.bfloat16,
                                         tag="kvg", name="kvg")
                        nrow = S * D
                        done = 0
                        while done < nrow:
                            cnt = min(32, nrow - done)
                            gather(kvg[:, done:done + cnt, :], kv_tab[t], 32768,
                                   gt[:, done * 8:(done + cnt) * 8], cnt * 128, 32,
                                   pitch=128)
                            done += cnt
                        kvga = kvg[:]
                        qga = vap(qgp, sl * HIDDEN,
                                  [[pq, 128], [HIDDEN, S], [1, HIDDEN]])
                        pk = kvga.ap[0][0]
                        prod = wpool.tile([128, S * HIDDEN, D],
                                          dt.bfloat16, tag="prod", name="prod")
                        pp = prod[:].ap[0][0]
                        nc.vector.tensor_tensor(
                            out=vap(prod[:], 0,
                                    [[pp, 128], [HIDDEN * D, S], [D, HIDDEN], [1, D]]),
                            in0=vap(qga, 0,
                                    [[pq, 128], [HIDDEN, S], [1, HIDDEN], [0, D]]),
                            in1=vap(kvga, 0,
                                    [[pk, 128], [D * 32, S], [1, HIDDEN],
                                     [32, D]]),
                            op=OP.mult)
                        wgt = wpool.tile([128, S * HEADS, D],
                                         dt.bfloat16, tag="wgt", name="wgt")
                        pw = wgt[:].ap[0][0]
                        nc.vector.tensor_tensor(
                            out=vap(wgt[:], 0,
                                    [[pw, 128], [HEADS * D, S], [D, HEADS], [1, D]]),
                            in0=vap(prod[:], 0,
                                    [[pp, 128], [HIDDEN * D, S], [D, HEADS], [1, D]]),
                            in1=vap(prod[:], HEADS * D,
                                    [[pp, 128], [HIDDEN * D, S], [D, HEADS], [1, D]]),
                            op=OP.add)
                        nc.scalar.activation(
                            out=vap(wgt[:], 0, [[pw, 128], [1, S * HEADS * D]]),
                            in_=vap(wgt[:], 0, [[pw, 128], [1, S * HEADS * D]]),
                            func=ACTF.Exp, scale=scale)
                        nc.vector.tensor_reduce(
                            out=u_acc[:, sl:sl + S, 16:24],
                            in_=vap(wgt[:], 0,
                                    [[pw, 128], [D, S * HEADS], [1, D]]),
                            axis=AX.X, op=OP.add)
                        msg = wpool.tile([128, S * HIDDEN, D],
                                         dt.bfloat16, tag="prod", name="msg")
                        pm = msg[:].ap[0][0]
                        nc.vector.tensor_tensor(
                            out=vap(msg[:], 0,
                                    [[pm, 128], [HIDDEN * D, S], [HEAD_DIM * D, HEADS],
                                     [D, HEAD_DIM], [1, D]]),
                            in0=vap(wgt[:], 0,
                                    [[pw, 128], [HEADS * D, S], [D, HEADS],
                                     [0, HEAD_DIM], [1, D]]),
                            in1=vap(kvga, 16,
                                    [[pk, 128], [D * 32, S], [HEAD_DIM, HEADS],
                                     [1, HEAD_DIM], [32, D]]),
                            op=OP.mult)
                        nc.vector.tensor_reduce(
                            out=u_acc[:, sl:sl + S, 0:16],
                            in_=vap(msg[:], 0,
                                    [[pm, 128], [D, S * HIDDEN], [1, D]]),
                            axis=AX.X, op=OP.add)
                    # u_acc (rank order) -> DRAM; recombination deferred to
                    # layer end so pass t+1 gathers are never blocked
                    nc.sync.dma_start(
                        out=bass.AP(u_tab[t], 0,
                                    [[KV_COLS, 128], [128 * KV_COLS, n_tiles],
                                     [1, 24]]),
                        in_=u_acc[:])
                    if t + 2 < N_SUB:
                        issue_ag(t + 2)

                # recombine all passes into identity order
                nc.vector.memset(u_tot[:], 0.0)
                for t in range(N_SUB):
                    ct = spool.tile([128, npp // 16], dt.int16, tag="ct", name="ct")
                    nc.sync.dma_start(out=ct[:], in_=cidx_d[t])
                    done = 0
                    while done < n_tiles:
                        cnt = min(32, n_tiles - done)
                        cg = spool.tile([128, 32, 24], dt.float32, tag="cgt",
                                        name="cg")
                        gather(cg[:, 0:cnt, :], u_tab[t], npp,
                               ct[:, done * 8:(done + cnt) * 8], cnt * 128, 24)
                        nc.vector.tensor_tensor(
                            out=u_tot[:, done:done + cnt, :],
                            in0=u_tot[:, done:done + cnt, :],
                            in1=cg[:, 0:cnt, :], op=OP.add)
                        done += cnt

                # epilogue
                dadj = spool.tile([128, n_tiles, HEADS], dt.float32, tag="dadj",
                                  name="dadj")
                pn = npad_t[:].ap[0][0]
                nc.vector.tensor_tensor(
                    out=dadj[:], in0=u_tot[:, :, 16:24],
                    in1=vap(npad_t[:], 0, [[pn, 128], [1, n_tiles], [0, HEADS]]),
                    op=OP.subtract)
                nc.vector.tensor_scalar_add(out=dadj[:], in0=dadj[:], scalar1=1e-9)
                rden = spool.tile([128, n_tiles, HEADS], dt.float32, tag="rden",
                                  name="rden")
                nc.vector.reciprocal(out=rden[:], in_=dadj[:])
                agg = wpool.tile([128, n_tiles, HIDDEN], dt.float32, tag="wgt",
                                 name="agg")
                pr_ = rden[:].ap[0][0]
                pa = agg[:].ap[0][0]
                nc.vector.tensor_tensor(
                    out=vap(agg[:], 0,
                            [[pa, 128], [HIDDEN, n_tiles], [HEAD_DIM, HEADS],
                             [1, HEAD_DIM]]),
                    in0=vap(u_tot[:], 0,
                            [[u_tot[:].ap[0][0], 128], [24, n_tiles],
                             [HEAD_DIM, HEADS], [1, HEAD_DIM]]),
                    in1=vap(rden[:], 0,
                            [[pr_, 128], [HEADS, n_tiles], [1, HEADS],
                             [0, HEAD_DIM]]),
                    op=OP.mult)
                hnew = wpool.tile([128, n_tiles, HIDDEN], dt.float32, tag="prod",
                                  name="hnew")
                for k in range(n_tiles):
                    tp = ppool.tile([HIDDEN, 128], dt.float32, tag="tp", name="atp")
                    nc.tensor.transpose(out=tp[:], in_=agg[:, k, :],
                                        identity=ident[:])
                    aT = spool.tile([HIDDEN, 128], dt.float32, tag="hT", name="aT")
                    nc.vector.tensor_copy(out=aT[:], in_=tp[:])
                    op_ = ppool.tile([128, HIDDEN], dt.float32, tag="mm", name="op_")
                    nc.tensor.matmul(out=op_[:], lhsT=aT[:],
                                     rhs=wmat[("wo", layer)][:],
                                     start=True, stop=True)
                    nc.vector.tensor_tensor(out=hnew[:, k, :], in0=op_[:],
                                            in1=h_all[:, k, :], op=OP.add)
                mu = spool.tile([128, n_tiles, 1], dt.float32, tag="mu", name="mu")
                nc.vector.tensor_reduce(out=mu[:], in_=hnew[:], axis=AX.X, op=OP.add)
                nc.vector.tensor_scalar_mul(out=mu[:], in0=mu[:], scalar1=1.0 / 16)
                cent = wpool.tile([128, n_tiles, HIDDEN], dt.float32, tag="wgt",
                                  name="cent")
                nc.vector.tensor_tensor(
                    out=cent[:], in0=hnew[:],
                    in1=vap(mu[:], 0, [[mu[:].ap[0][0], 128], [1, n_tiles],
                                       [0, HIDDEN]]),
                    op=OP.subtract)
                sq = wpool.tile([128, n_tiles, HIDDEN], dt.float32, tag="prod",
                                name="sq")
                nc.vector.tensor_tensor(out=sq[:], in0=cent[:], in1=cent[:],
                                        op=OP.mult)
                var = spool.tile([128, n_tiles, 1], dt.float32, tag="var", name="var")
                nc.vector.tensor_reduce(out=var[:], in_=sq[:], axis=AX.X, op=OP.add)
                nc.vector.tensor_scalar_mul(out=var[:], in0=var[:], scalar1=1.0 / 16)
                nc.vector.tensor_scalar_add(out=var[:], in0=var[:], scalar1=1e-5)
                rs = spool.tile([128, n_tiles, 1], dt.float32, tag="rs", name="rs")
                nc.vector.reciprocal(out=rs[:], in_=var[:])
                nc.scalar.activation(out=rs[:], in_=rs[:], func=ACTF.Sqrt)
                nc.vector.tensor_tensor(
                    out=h_all[:], in0=cent[:],
                    in1=vap(rs[:], 0, [[rs[:].ap[0][0], 128], [1, n_tiles],
                                       [0, HIDDEN]]),
                    op=OP.mult)

            # final head
            nc.vector.memset(acc4[:], 0.0)
            for k in range(n_tiles):
                tp = ppool.tile([HIDDEN, 128], dt.float32, tag="tp", name="ftp")
                nc.tensor.transpose(out=tp[:], in_=h_all[:, k, :], identity=ident[:])
                hT = spool.tile([HIDDEN, 128], dt.float32, tag="hT", name="fhT")
                nc.vector.tensor_copy(out=hT[:], in_=tp[:])
                gp = ppool.tile([128, DIM], dt.float32, tag="mm", name="gp")
                nc.tensor.matmul(out=gp[:], lhsT=hT[:], rhs=wout_t[:],
                                 start=True, stop=True)
                nc.vector.tensor_tensor(out=acc4[:], in0=acc4[:], in1=gp[:],
                                        op=OP.add)
            onesk = cpool.tile([128, 1], dt.float32, name="onesk")
            nc.vector.memset(onesk[:], 1.0 / cfg["n_nodes"])
            pooled_p = ppool.tile([1, 4], dt.float32, tag="mm", name="pooled_p")
            nc.tensor.matmul(out=pooled_p[:], lhsT=onesk[:], rhs=acc4[:],
                             start=True, stop=True)
            pooled_s = spool.tile([1, 4], dt.float32, tag="p4", name="pooled_s")
            nc.vector.tensor_copy(out=pooled_s[:], in_=pooled_p[:])
            nc.sync.dma_start(out=ar_in[:], in_=pooled_s[:])
            nc.gpsimd.collective_compute(
                "AllReduce", mybir.AluOpType.add,
                replica_groups=[list(range(cfg["n_cores"]))],
                ins=[ar_in[:]], outs=[ar_out[:]])
            pooled = spool.tile([1, 4], dt.float32, tag="p4b", name="pooled")
            nc.sync.dma_start(out=pooled[:], in_=ar_out[:])
            fcw_t = spool.tile([1, 12], dt.float32, tag="fcw", name="fcw_t")
            nc.sync.dma_start(out=fcw_t[:], in_=fcw_d[:])
            fcb_t = spool.tile([1, 3], dt.float32, tag="fcb", name="fcb_t")
            nc.sync.dma_start(out=fcb_t[:], in_=fcb_d[:])
            pr2 = spool.tile([1, 12], dt.float32, tag="pr2", name="pr2")
            nc.vector.tensor_tensor(
                out=pr2[:],
                in0=vap(pooled[:], 0, [[pooled[:].ap[0][0], 1], [0, 3], [1, 4]]),
                in1=vap(fcw_t[:], 0, [[fcw_t[:].ap[0][0], 1], [4, 3], [1, 4]]),
                op=OP.mult)
            y3 = spool.tile([1, 3], dt.float32, tag="y3", name="y3")
            nc.vector.tensor_reduce(
                out=y3[:],
                in_=vap(pr2[:], 0, [[pr2[:].ap[0][0], 1], [4, 3], [1, 4]]),
                axis=AX.X, op=OP.add)
            nc.vector.tensor_tensor(out=y3[:], in0=y3[:], in1=fcb_t[:], op=OP.add)
            nc.sync.dma_start(out=y_out[:], in_=y3[:])

    _split_excess_waits(nc, max_waits=1)
    lower_extended_insts(nc)
    return nc


def kernel(x, edge_index, W_in, Wq, Wk, Wv, Wo, W_out, fc_w, fc_b):
    x = np.asarray(x, dtype=np.float32)
    edge_index = np.asarray(edge_index)
    cfg = _cfg(x.shape[0])

    key = ("nc", x.shape[0], edge_index.shape[1])
    if key not in _RUN_CACHE:
        schedule, plans = _plan(edge_index, cfg)
        meta = (plans[0]["gidx"].shape[1], plans[0]["qidx"].shape[1],
                plans[0]["chunk_meta"])
        nc = _build(meta, cfg)
        _RUN_CACHE[key] = (nc, plans)
    nc, plans = _RUN_CACHE[key]

    wts = _permute_weights(
        np.asarray(W_in, np.float32), np.asarray(Wq, np.float32),
        np.asarray(Wk, np.float32), np.asarray(Wv, np.float32),
        np.asarray(Wo, np.float32), np.asarray(W_out, np.float32),
        np.asarray(fc_w, np.float32), np.asarray(fc_b, np.float32))

    nps, pcs, npp = cfg["n_per_sub"], cfg["per_core_sub"], cfg["npp"]
    old = np.arange(cfg["n_nodes"])
    c_of = (old % nps) // pcs
    l_of = (old // nps) * pcs + (old % pcs)
    in_maps = []
    for c in range(cfg["n_cores"]):
        xl = np.zeros((npp, DIM), dtype=np.float32)
        m = c_of == c
        xl[l_of[m]] = x[m]
        p = plans[c]
        in_maps.append(dict(
            x_in=xl, gidx=p["gidx"], qidx=p["qidx"], cidx=p["cidx"],
            npad=p["npad"], w_in=wts["W_in"], wq=wts["Wq"], wk=wts["Wk"],
            wv=wts["Wv"], wo=wts["Wo"], wout=wts["W_out"], fcw=wts["fcw"],
            fcb=wts["fcb"]))

    from concourse.bass_utils import run_bass_kernel_spmd
    trace = bool(os.environ.get("GNN_TRACE"))
    if trace:
        _install_profhook()
    res = run_bass_kernel_spmd(nc, in_maps, core_ids=list(range(cfg["n_cores"])),
                               trace=trace)
    if trace:
        _RUN_CACHE["last_result"] = res
    return np.asarray(res.results[0]["y"]).reshape(3).astype(np.float32)



# revision 26
# speedup vs baseline: 1.3125x; 1.0580x over previous
"""GNN message-passing (SE3-style graph attention) kernel for 8 Trainium2 cores.

Edge-parallel strategy:
- Nodes relabeled into 4 "subtables" x 8 cores so per-edge kv-gather indices
  fit int16 (dma_gather requirement). Each core owns 12500 dst nodes.
- Per layer: on-device q/k/v projections -> 4 AllGathers build global kv
  tables -> 4 passes over src subtables, each processing edges in node-major
  degree-sorted chunks: dma_gather kv rows, affine q broadcast, DVE
  scores/exp/messages, affine segment reduction into per-pass accumulators.
- Pass accumulators (degree-rank order) recombined into identity order by
  small dma_gathers, then divide / Wo project / residual / LayerNorm.
- Final: W_out, mean-pool via PE ones-matmul, AllReduce, FC head.
"""

import os
import sys
import types
import numpy as np

HEADS = 8
HEAD_DIM = 2
HIDDEN = 16
DIM = 4
DEPTH = 2
N_SUB = 4
KV_COLS = 64          # table row = 64 fp32 = 256B
SENT = 32767          # zeroed sentinel row in each kv subtable
SD_CAP = 128          # max S*D per chunk
S_CAP = 8

_RUN_CACHE = {}


# --------------------------------------------------------------------------
# harness shims (self-contained copies)
# --------------------------------------------------------------------------
def _split_excess_waits(nc, max_waits=1):
    """Walrus build allows 1 sync-wait per instruction; move extras to NOPs."""
    import concourse.mybir as mybir
    n = [0]
    for blk in nc.m.functions[0].blocks:
        new_insts = []
        for inst in blk.instructions:
            si = inst.sync_info
            if si is not None and len(si.on_wait) > max_waits:
                waits = list(si.on_wait)
                extra, keep = waits[:-max_waits], waits[-max_waits:]
                for i in range(0, len(extra), max_waits):
                    n[0] += 1
                    nop = mybir.InstNoOp(
                        name=f"IWS-{n[0]}", engine=inst.engine, ins=[], outs=[],
                        sync_info=mybir.SyncInfo(on_wait=extra[i:i + max_waits],
                                                 on_update=[]))
                    try:
                        nc.register_instruction(nop, overwrite=True)
                    except Exception:
                        pass
                    new_insts.append(nop)
                si.on_wait = keep
            new_insts.append(inst)
        blk.instructions[:] = new_insts


def _install_profhook():
    if 'antenv.axon_hooks' in sys.modules:
        return
    try:
        import antenv
        from trn_agent_boot.trn_boot import _ntff_profile_via_ctypes
        hook = _ntff_profile_via_ctypes('/opt/axon/libaxon_pjrt.so')
        mod = types.ModuleType('antenv.axon_hooks')
        state = {'hook': hook}
        mod.set_axon_ntff_profile_hook = lambda h: state.__setitem__('hook', h)
        mod.get_axon_ntff_profile_hook = lambda: state['hook']
        sys.modules['antenv.axon_hooks'] = mod
        antenv.axon_hooks = mod
    except Exception:
        pass


# --------------------------------------------------------------------------
# host-side planning
# --------------------------------------------------------------------------
def _cfg(n_nodes):
    n_cores = 8
    n_per_sub = n_nodes // N_SUB
    per_core_sub = n_per_sub // n_cores
    npc = N_SUB * per_core_sub
    npp = ((npc + 127) // 128) * 128
    return dict(n_nodes=n_nodes, n_cores=n_cores, n_per_sub=n_per_sub,
                per_core_sub=per_core_sub, npc=npc, npp=npp,
                n_tiles=npp // 128)


def _pack_gidx(idx_flat):
    """Gather feed position i lives at tile[i%16, i//16]; replicate x8 cores."""
    n = idx_flat.shape[0]
    assert n % 16 == 0
    tile16 = np.ascontiguousarray(
        idx_flat.reshape(n // 16, 16).T.astype(np.int16))
    return np.tile(tile16, (8, 1))


def _plan(edge_index, cfg):
    src = np.asarray(edge_index[0], dtype=np.int64)
    dst = np.asarray(edge_index[1], dtype=np.int64)
    nps, pcs = cfg["n_per_sub"], cfg["per_core_sub"]
    npp, n_cores = cfg["npp"], cfg["n_cores"]

    e_core = (dst % nps) // pcs
    e_l = (dst // nps) * pcs + (dst % pcs)
    e_t = src // nps
    e_row = src % nps

    passes = []
    for t in range(N_SUB):
        per_core = []
        for c in range(n_cores):
            m = (e_core == c) & (e_t == t)
            lt, rowt = e_l[m], e_row[m]
            deg = np.bincount(lt, minlength=npp).astype(np.int64)
            order = np.argsort(-deg, kind="stable")
            rank_of = np.empty(npp, dtype=np.int64)
            rank_of[order] = np.arange(npp)
            eorder = np.argsort(rank_of[lt], kind="stable")
            per_core.append(dict(deg=deg, order=order, rank_of=rank_of,
                                 lt=lt[eorder], rowt=rowt[eorder]))
        passes.append(per_core)

    schedule = []
    for t in range(N_SUB):
        chunks = []
        r0 = 0
        degs_sorted = [passes[t][c]["deg"][passes[t][c]["order"]]
                       for c in range(n_cores)]
        while r0 < npp:
            D = int(max(int(d[r0]) for d in degs_sorted))
            if D == 0:
                break
            S = max(1, min(S_CAP, SD_CAP // D, (npp - r0) // 128))
            chunks.append((r0, S, D))
            r0 += 128 * S
        schedule.append(chunks)

    plans = []
    for c in range(n_cores):
        gidx_cols, qidx_list, chunk_meta = [], [], []
        npad = np.zeros(npp, dtype=np.float64)
        gcol0 = 0
        for t in range(N_SUB):
            pc = passes[t][c]
            deg, order, rank_of = pc["deg"], pc["order"], pc["rank_of"]
            lt, rowt = pc["lt"], pc["rowt"]
            offs = np.zeros(npp + 1, dtype=np.int64)
            offs[1:] = np.cumsum(deg[order])
            ranks_e = rank_of[lt]
            j_e = np.arange(lt.shape[0]) - offs[ranks_e]
            # one pass-wide q index block: full rank order
            qidx_list.append(_pack_gidx(order[0:npp]))
            for ci, (r0, S, D) in enumerate(schedule[t]):
                nrows = 128 * S * D
                idx_flat = np.full(nrows, SENT, dtype=np.int64)
                em = (ranks_e >= r0) & (ranks_e < r0 + 128 * S)
                q = ranks_e[em] - r0
                pos = ((q // 128) * D + j_e[em]) * 128 + (q % 128)
                idx_flat[pos] = rowt[em]
                gidx_cols.append(_pack_gidx(idx_flat))
                ch_nodes = order[r0:r0 + 128 * S]
                npad[ch_nodes] += D - deg[ch_nodes]
                chunk_meta.append((t, r0, S, D, gcol0, nrows // 16))
                gcol0 += nrows // 16
        gidx = (np.concatenate(gidx_cols, axis=1) if gidx_cols
                else np.zeros((128, 16), np.int16))
        qidx = (np.concatenate(qidx_list, axis=1) if qidx_list
                else np.zeros((128, 8), np.int16))
        cidx = np.stack([_pack_gidx(passes[t][c]["rank_of"][:npp])
                         for t in range(N_SUB)])
        npad_t = np.ascontiguousarray(
            npad.reshape(cfg["n_tiles"], 128).T.astype(np.float32))
        plans.append(dict(gidx=gidx, qidx=qidx, cidx=cidx, npad=npad_t,
                          chunk_meta=chunk_meta))
    return schedule, plans


def _permute_weights(W_in, Wq, Wk, Wv, Wo, W_out, fc_w, fc_b):
    perm = np.array([h * HEAD_DIM + d for d in range(HEAD_DIM)
                     for h in range(HEADS)], dtype=np.int64)
    return dict(W_in=np.ascontiguousarray(W_in),
                Wq=np.ascontiguousarray(Wq[:, :, perm]),
                Wk=np.ascontiguousarray(Wk[:, :, perm]),
                Wv=np.ascontiguousarray(Wv),
                Wo=np.ascontiguousarray(Wo),
                W_out=np.ascontiguousarray(W_out),
                fcw=np.ascontiguousarray(fc_w.T).reshape(1, 12).astype(np.float32),
                fcb=fc_b.reshape(1, 3).astype(np.float32))


# --------------------------------------------------------------------------
# device program
# --------------------------------------------------------------------------
def _build(meta, cfg):
    import concourse.bass as bass
    import concourse.mybir as mybir
    import concourse.tile as tile
    from concourse import library_config
    from concourse.masks import make_identity
    from concourse.library_overlay import lower_extended_insts

    dt = mybir.dt
    AX = mybir.AxisListType
    OP = mybir.AluOpType
    ACTF = mybir.ActivationFunctionType
    npp, n_tiles, npc = cfg["npp"], cfg["n_tiles"], cfg["npc"]
    pcs = cfg["per_core_sub"]
    g_cols_total, q_cols_total, chunk_meta = meta

    nc = bass.Bass(num_devices=cfg["n_cores"], num_swdge_queues=4)

    x_in = nc.dram_tensor("x_in", [npp, DIM], dt.float32, kind="ExternalInput")
    gidx_d = nc.dram_tensor("gidx", [128, g_cols_total], dt.int16, kind="ExternalInput")
    qidx_d = nc.dram_tensor("qidx", [128, q_cols_total], dt.int16, kind="ExternalInput")
    cidx_d = nc.dram_tensor("cidx", [N_SUB, 128, npp // 16], dt.int16, kind="ExternalInput")
    npad_d = nc.dram_tensor("npad", [128, n_tiles], dt.float32, kind="ExternalInput")
    w_in_d = nc.dram_tensor("w_in", [DIM, HIDDEN], dt.float32, kind="ExternalInput")
    wq_d = nc.dram_tensor("wq", [DEPTH, HIDDEN, HIDDEN], dt.float32, kind="ExternalInput")
    wk_d = nc.dram_tensor("wk", [DEPTH, HIDDEN, HIDDEN], dt.float32, kind="ExternalInput")
    wv_d = nc.dram_tensor("wv", [DEPTH, HIDDEN, HIDDEN], dt.float32, kind="ExternalInput")
    wo_d = nc.dram_tensor("wo", [DEPTH, HIDDEN, HIDDEN], dt.float32, kind="ExternalInput")
    wout_d = nc.dram_tensor("wout", [HIDDEN, DIM], dt.float32, kind="ExternalInput")
    fcw_d = nc.dram_tensor("fcw", [1, 12], dt.float32, kind="ExternalInput")
    fcb_d = nc.dram_tensor("fcb", [1, 3], dt.float32, kind="ExternalInput")
    y_out = nc.dram_tensor("y", [1, 3], dt.float32, kind="ExternalOutput")

    kv_tab = [nc.dram_tensor(f"kvtab{t}", [32768, 128], dt.bfloat16,
                             kind="Internal", addr_space="Shared")
              for t in range(N_SUB)]
    q_tab = nc.dram_tensor("qtab", [npp, 128], dt.bfloat16, kind="Internal")
    u_tab = [nc.dram_tensor(f"utab{t}", [npp, KV_COLS], dt.float32, kind="Internal")
             for t in range(N_SUB)]
    stage_t = [nc.dram_tensor(f"stage{t}", [npc // N_SUB, 128], dt.bfloat16,
                              kind="Internal") for t in range(N_SUB)]
    ar_in = nc.dram_tensor("ar_in", [1, 4], dt.float32, kind="Internal")
    ar_out = nc.dram_tensor("ar_out", [1, 4], dt.float32, kind="Internal",
                            addr_space="Shared")

    nc.gpsimd.load_library(library_config.attnmlp)
    gq = [0]
    _nregs = {}

    def gather(out_ap, in_tensor, nrows, idx_ap, num_idxs, elem_size,
               pitch=KV_COLS):
        """Raw InstDMAGatherAnt reading elem_size elems from 256B-pitch rows.

        Bypasses bass's elem_size_bytes%256 assert: the ucode only requires
        the row *pitch* (elem_step bytes) to be a 256B multiple."""
        if num_idxs not in _nregs:
            _nregs[num_idxs] = nc.gpsimd.to_reg(num_idxs)
        g = nc.gpsimd
        in_ap = bass.AP(in_tensor, 0, [[pitch, nrows], [1, elem_size]])
        _in_ap = g.lower_ap_dma(in_ap, for_custom_bir_dma=True)
        _idxs_ap = g.lower_ap(idx_ap)
        _out_ap = g.lower_ap(out_ap)
        g.add_instruction(mybir.InstDMAGatherAnt(
            name=nc.get_next_instruction_name(),
            ins=[*_in_ap, _idxs_ap, g.lower_val_access(_nregs[num_idxs])],
            outs=[_out_ap],
            transpose=False, num_idxs=num_idxs, elem_size=elem_size,
            stride_bytes_256=1, gen_mode=0, single_packet=False,
            queue_num=gq[0] % 4,
            sbuf_tokens_per_rank=0, sbuf_free_dim_per_rank=0,
            sbuf_free_dim_pad_per_rank=0, sbuf_byte_offset=0))
        gq[0] += 1

    def vap(base_ap, extra_off, dims):
        return bass.AP(base_ap.tensor, base_ap.offset + extra_off, dims)

    with tile.TileContext(nc) as tc:
        with (
            tc.tile_pool(name="const", bufs=1) as cpool,
            tc.tile_pool(name="res", bufs=1) as rpool,
            tc.tile_pool(name="work", bufs=3) as wpool,
            tc.tile_pool(name="gath", bufs=6) as gpool,
            tc.tile_pool(name="small", bufs=3) as spool,
            tc.tile_pool(name="psum", bufs=4, space="PSUM") as ppool,
        ):
            ident = cpool.tile([128, 128], dt.float32, name="ident")
            make_identity(nc, ident[:])
            w_in_t = cpool.tile([DIM, HIDDEN], dt.float32, name="w_in_t")
            nc.sync.dma_start(out=w_in_t[:], in_=w_in_d[:])
            wmat = {}
            for nm, dd in (("wq", wq_d), ("wk", wk_d), ("wv", wv_d), ("wo", wo_d)):
                for l in range(DEPTH):
                    w = cpool.tile([HIDDEN, HIDDEN], dt.float32, name=f"{nm}{l}")
                    nc.sync.dma_start(out=w[:], in_=dd[l])
                    wmat[(nm, l)] = w
            wout_t = cpool.tile([HIDDEN, DIM], dt.float32, name="wout_t")
            nc.sync.dma_start(out=wout_t[:], in_=wout_d[:])
            npad_t = cpool.tile([128, n_tiles], dt.float32, name="npad_t")
            nc.sync.dma_start(out=npad_t[:], in_=npad_d[:])

            zrow = cpool.tile([1, 128], dt.bfloat16, name="zrow")
            nc.vector.memset(zrow[:], 0.0)
            for t in range(N_SUB):
                nc.sync.dma_start(out=kv_tab[t][SENT:SENT + 1, :], in_=zrow[:])

            h_all = rpool.tile([128, n_tiles, HIDDEN], dt.float32, name="h_all")
            u_tot = rpool.tile([128, n_tiles, 24], dt.float32, name="u_tot")
            u_accs = [rpool.tile([128, n_tiles, 24], dt.float32, name=f"u_acc{i}")
                      for i in range(N_SUB)]
            acc4 = rpool.tile([128, 4], dt.float32, name="acc4")

            # stage 0: h0 = x @ W_in
            xs = wpool.tile([128, n_tiles, DIM], dt.float32, tag="gt", name="xs")
            nc.sync.dma_start(out=xs[:],
                              in_=x_in[:].rearrange("(a p) f -> p a f", p=128))
            for k in range(n_tiles):
                tp = ppool.tile([DIM, 128], dt.float32, tag="tp", name="tp")
                nc.tensor.transpose(out=tp[:], in_=xs[:, k, :], identity=ident[:])
                tps = spool.tile([HIDDEN, 128], dt.float32, tag="hT", name="tps")
                nc.vector.tensor_copy(out=tps[0:DIM, :], in_=tp[:])
                hp = ppool.tile([128, HIDDEN], dt.float32, tag="mm", name="hp")
                nc.tensor.matmul(out=hp[:], lhsT=tps[0:DIM, :], rhs=w_in_t[:],
                                 start=True, stop=True)
                nc.vector.tensor_copy(out=h_all[:, k, :], in_=hp[:])

            scale = float(1.0 / np.sqrt(HEAD_DIM))

            def issue_ag(t):
                nc.gpsimd.collective_compute(
                    "AllGather", mybir.AluOpType.bypass,
                    replica_groups=[list(range(cfg["n_cores"]))],
                    ins=[stage_t[t][:]],
                    outs=[kv_tab[t][0:cfg["n_cores"] * pcs, :]])

            for layer in range(DEPTH):
                # q/k/v projections; stage written per-subtable so each
                # AllGather depends only on its own slab
                for k in range(n_tiles):
                    tp = ppool.tile([HIDDEN, 128], dt.float32, tag="tp", name="htp")
                    nc.tensor.transpose(out=tp[:], in_=h_all[:, k, :],
                                        identity=ident[:])
                    hT = spool.tile([HIDDEN, 128], dt.float32, tag="hT", name="hT")
                    nc.vector.tensor_copy(out=hT[:], in_=tp[:])
                    qp = ppool.tile([128, HIDDEN], dt.float32, tag="mm", name="qp")
                    nc.tensor.matmul(out=qp[:], lhsT=hT[:], rhs=wmat[("wq", layer)][:],
                                     start=True, stop=True)
                    qs = spool.tile([128, HIDDEN], dt.bfloat16, tag="qs", name="qs")
                    nc.vector.tensor_copy(out=qs[:], in_=qp[:])
                    nc.sync.dma_start(out=q_tab[k * 128:(k + 1) * 128, 0:HIDDEN],
                                      in_=qs[:])
                    kp = ppool.tile([128, HIDDEN], dt.float32, tag="mm", name="kp")
                    nc.tensor.matmul(out=kp[:], lhsT=hT[:], rhs=wmat[("wk", layer)][:],
                                     start=True, stop=True)
                    kvs = spool.tile([128, 32], dt.bfloat16, tag="kvs", name="kvs")
                    nc.vector.tensor_copy(out=kvs[:, 0:16], in_=kp[:])
                    vp = ppool.tile([128, HIDDEN], dt.float32, tag="mm", name="vp")
                    nc.tensor.matmul(out=vp[:], lhsT=hT[:], rhs=wmat[("wv", layer)][:],
                                     start=True, stop=True)
                    nc.vector.tensor_copy(out=kvs[:, 16:32], in_=vp[:])
                    lo, hi = k * 128, min((k + 1) * 128, npc)
                    r = lo
                    while r < hi:
                        t = r // pcs
                        e = min(hi, (t + 1) * pcs)
                        nc.sync.dma_start(
                            out=stage_t[t][r - t * pcs:e - t * pcs, 0:32],
                            in_=kvs[r - lo:e - lo, :])
                        r = e
                issue_ag(0)
                issue_ag(1)

                for t in range(N_SUB):
                    u_acc = u_accs[t]
                    nc.vector.memset(u_acc[:], 0.0)
                    # pass-wide q gather: q_tab permuted into rank order
                    qt = spool.tile([128, npp // 16], dt.int16, tag="qt", name="qt")
                    nc.sync.dma_start(
                        out=qt[:], in_=qidx_d[:, t * (npp // 16):(t + 1) * (npp // 16)])
                    qg = wpool.tile([128, n_tiles, HIDDEN], dt.bfloat16,
                                    tag="qgp", name="qgp")
                    done = 0
                    while done < n_tiles:
                        cnt = min(32, n_tiles - done)
                        gather(qg[:, done:done + cnt, :], q_tab, npp,
                               qt[:, done * 8:(done + cnt) * 8], cnt * 128, HIDDEN,
                               pitch=128)
                        done += cnt
                    qgp = qg[:]
                    pq = qgp.ap[0][0]
                    for (tt, r0, S, D, gc0, gcols) in chunk_meta:
                        if tt != t:
                            continue
                        sl = r0 // 128
                        gt = wpool.tile([128, 1024], dt.int16, tag="gt", name="gt")
                        nc.sync.dma_start(out=gt[:, 0:gcols],
                                          in_=gidx_d[:, gc0:gc0 + gcols])
                        kvg = gpool.tile([128, SD_CAP, 32], dt.bfloat16,
                                         tag="kvg", name="kvg")
                        nrow = S * D
                        done = 0
                        while done < nrow:
                            cnt = min(32, nrow - done)
                            gather(kvg[:, done:done + cnt, :], kv_tab[t], 32768,
                                   gt[:, done * 8:(done + cnt) * 8], cnt * 128, 32,
                                   pitch=128)
                            done += cnt
                        kvga = kvg[:]
                        qga = vap(qgp, sl * HIDDEN,
                                  [[pq, 128], [HIDDEN, S], [1, HIDDEN]])
                        pk = kvga.ap[0][0]
                        prod = wpool.tile([128, S * HIDDEN, D],
                                          dt.bfloat16, tag="prod", name="prod")
                        pp = prod[:].ap[0][0]
                        nc.vector.tensor_tensor(
                            out=vap(prod[:], 0,
                                    [[pp, 128], [HIDDEN * D, S], [D, HIDDEN], [1, D]]),
                            in0=vap(qga, 0,
                                    [[pq, 128], [HIDDEN, S], [1, HIDDEN], [0, D]]),
                            in1=vap(kvga, 0,
                                    [[pk, 128], [D * 32, S], [1, HIDDEN],
                                     [32, D]]),
                            op=OP.mult)
                        wgt = wpool.tile([128, S * HEADS, D],
                                         dt.bfloat16, tag="wgt", name="wgt")
                        pw = wgt[:].ap[0][0]
                        nc.vector.tensor_tensor(
                            out=vap(wgt[:], 0,
                                    [[pw, 128], [HEADS * D, S], [D, HEADS], [1, D]]),
                            in0=vap(prod[:], 0,
                                    [[pp, 128], [HIDDEN * D, S], [D, HEADS], [1, D]]),
                            in1=vap(prod[:], HEADS * D,
                                    [[pp, 128], [HIDDEN * D, S], [D, HEADS], [1, D]]),
                            op=OP.add)
                        nc.scalar.activation(
                            out=vap(wgt[:], 0, [[pw, 128], [1, S * HEADS * D]]),
                            in_=vap(wgt[:], 0, [[pw, 128], [1, S * HEADS * D]]),
                            func=ACTF.Exp, scale=scale)
                        nc.vector.tensor_reduce(
                            out=u_acc[:, sl:sl + S, 16:24],
                            in_=vap(wgt[:], 0,
                                    [[pw, 128], [D, S * HEADS], [1, D]]),
                            axis=AX.X, op=OP.add)
                        msg = wpool.tile([128, S * HIDDEN, D],
                                         dt.bfloat16, tag="prod", name="msg")
                        pm = msg[:].ap[0][0]
                        nc.vector.tensor_tensor(
                            out=vap(msg[:], 0,
                                    [[pm, 128], [HIDDEN * D, S], [HEAD_DIM * D, HEADS],
                                     [D, HEAD_DIM], [1, D]]),
                            in0=vap(wgt[:], 0,
                                    [[pw, 128], [HEADS * D, S], [D, HEADS],
                                     [0, HEAD_DIM], [1, D]]),
                            in1=vap(kvga, 16,
                                    [[pk, 128], [D * 32, S], [HEAD_DIM, HEADS],
                                     [1, HEAD_DIM], [32, D]]),
                            op=OP.mult)
                        nc.vector.tensor_reduce(
                            out=u_acc[:, sl:sl + S, 0:16],
                            in_=vap(msg[:], 0,
                                    [[pm, 128], [D, S * HIDDEN], [1, D]]),
                            axis=AX.X, op=OP.add)
                    # u_acc (rank order) -> DRAM; recombination deferred to
                    # layer end so pass t+1 gathers are never blocked
                    nc.sync.dma_start(
                        out=bass.AP(u_tab[t], 0,
                                    [[KV_COLS, 128], [128 * KV_COLS, n_tiles],
                                     [1, 24]]),
                        in_=u_acc[:])
                    if t + 2 < N_SUB:
                        issue_ag(t + 2)

                # recombine all passes into identity order
                nc.vector.memset(u_tot[:], 0.0)
                for t in range(N_SUB):
                    ct = spool.tile([128, npp // 16], dt.int16, tag="ct", name="ct")
                    nc.sync.dma_start(out=ct[:], in_=cidx_d[t])
                    done = 0
                    while done < n_tiles:
                        cnt = min(32, n_tiles - done)
                        cg = spool.tile([128, 32, 24], dt.float32, tag="cgt",
                                        name="cg")
                        gather(cg[:, 0:cnt, :], u_tab[t], npp,
                               ct[:, done * 8:(done + cnt) * 8], cnt * 128, 24)
                        nc.vector.tensor_tensor(
                            out=u_tot[:, done:done + cnt, :],
                            in0=u_tot[:, done:done + cnt, :],
                            in1=cg[:, 0:cnt, :], op=OP.add)
                        done += cnt

                # epilogue
                dadj = spool.tile([128, n_tiles, HEADS], dt.float32, tag="dadj",
                                  name="dadj")
                pn = npad_t[:].ap[0][0]
                nc.vector.tensor_tensor(
                    out=dadj[:], in0=u_tot[:, :, 16:24],
                    in1=vap(npad_t[:], 0, [[pn, 128], [1, n_tiles], [0, HEADS]]),
                    op=OP.subtract)
                nc.vector.tensor_scalar_add(out=dadj[:], in0=dadj[:], scalar1=1e-9)
                rden = spool.tile([128, n_tiles, HEADS], dt.float32, tag="rden",
                                  name="rden")
                nc.vector.reciprocal(out=rden[:], in_=dadj[:])
                agg = wpool.tile([128, n_tiles, HIDDEN], dt.float32, tag="wgt",
                                 name="agg")
                pr_ = rden[:].ap[0][0]
                pa = agg[:].ap[0][0]
                nc.vector.tensor_tensor(
                    out=vap(agg[:], 0,
                            [[pa, 128], [HIDDEN, n_tiles], [HEAD_DIM, HEADS],
                             [1, HEAD_DIM]]),
                    in0=vap(u_tot[:], 0,
                            [[u_tot[:].ap[0][0], 128], [24, n_tiles],
                             [HEAD_DIM, HEADS], [1, HEAD_DIM]]),
                    in1=vap(rden[:], 0,
                            [[pr_, 128], [HEADS, n_tiles], [1, HEADS],
                             [0, HEAD_DIM]]),
                    op=OP.mult)
                hnew = wpool.tile([128, n_tiles, HIDDEN], dt.float32, tag="prod",
                                  name="hnew")
                for k in range(n_tiles):
                    tp = ppool.tile([HIDDEN, 128], dt.float32, tag="tp", name="atp")
                    nc.tensor.transpose(out=tp[:], in_=agg[:, k, :],
                                        identity=ident[:])
                    aT = spool.tile([HIDDEN, 128], dt.float32, tag="hT", name="aT")
                    nc.vector.tensor_copy(out=aT[:], in_=tp[:])
                    op_ = ppool.tile([128, HIDDEN], dt.float32, tag="mm", name="op_")
                    nc.tensor.matmul(out=op_[:], lhsT=aT[:],
                                     rhs=wmat[("wo", layer)][:],
                                     start=True, stop=True)
                    nc.vector.tensor_tensor(out=hnew[:, k, :], in0=op_[:],
                                            in1=h_all[:, k, :], op=OP.add)
                mu = spool.tile([128, n_tiles, 1], dt.float32, tag="mu", name="mu")
                nc.vector.tensor_reduce(out=mu[:], in_=hnew[:], axis=AX.X, op=OP.add)
                nc.vector.tensor_scalar_mul(out=mu[:], in0=mu[:], scalar1=1.0 / 16)
                cent = wpool.tile([128, n_tiles, HIDDEN], dt.float32, tag="wgt",
                                  name="cent")
                nc.vector.tensor_tensor(
                    out=cent[:], in0=hnew[:],
                    in1=vap(mu[:], 0, [[mu[:].ap[0][0], 128], [1, n_tiles],
                                       [0, HIDDEN]]),
                    op=OP.subtract)
                sq = wpool.tile([128, n_tiles, HIDDEN], dt.float32, tag="prod",
                                name="sq")
                nc.vector.tensor_tensor(out=sq[:], in0=cent[:], in1=cent[:],
                                        op=OP.mult)
                var = spool.tile([128, n_tiles, 1], dt.float32, tag="var", name="var")
                nc.vector.tensor_reduce(out=var[:], in_=sq[:], axis=AX.X, op=OP.add)
                nc.vector.tensor_scalar_mul(out=var[:], in0=var[:], scalar1=1.0 / 16)
                nc.vector.tensor_scalar_add(out=var[:], in0=var[:], scalar1=1e-5)
                rs = spool.tile([128, n_tiles, 1], dt.float32, tag="rs", name="rs")
                nc.vector.reciprocal(out=rs[:], in_=var[:])
                nc.scalar.activation(out=rs[:], in_=rs[:], func=ACTF.Sqrt)
                nc.vector.tensor_tensor(
                    out=h_all[:], in0=cent[:],
                    in1=vap(rs[:], 0, [[rs[:].ap[0][0], 128], [1, n_tiles],
                                       [0, HIDDEN]]),
                    op=OP.mult)

            # final head
            nc.vector.memset(acc4[:], 0.0)
            for k in range(n_tiles):
                tp = ppool.tile([HIDDEN, 128], dt.float32, tag="tp", name="ftp")
                nc.tensor.transpose(out=tp[:], in_=h_all[:, k, :], identity=ident[:])
                hT = spool.tile([HIDDEN, 128], dt.float32, tag="hT", name="fhT")
                nc.vector.tensor_copy(out=hT[:], in_=tp[:])
                gp = ppool.tile([128, DIM], dt.float32, tag="mm", name="gp")
                nc.tensor.matmul(out=gp[:], lhsT=hT[:], rhs=wout_t[:],
                                 start=True, stop=True)
                nc.vector.tensor_tensor(out=acc4[:], in0=acc4[:], in1=gp[:],
                                        op=OP.add)
            onesk = cpool.tile([128, 1], dt.float32, name="onesk")
            nc.vector.memset(onesk[:], 1.0 / cfg["n_nodes"])
            pooled_p = ppool.tile([1, 4], dt.float32, tag="mm", name="pooled_p")
            nc.tensor.matmul(out=pooled_p[:], lhsT=onesk[:], rhs=acc4[:],
                             start=True, stop=True)
            pooled_s = spool.tile([1, 4], dt.float32, tag="p4", name="pooled_s")
            nc.vector.tensor_copy(out=pooled_s[:], in_=pooled_p[:])
            nc.sync.dma_start(out=ar_in[:], in_=pooled_s[:])
            nc.gpsimd.collective_compute(
                "AllReduce", mybir.AluOpType.add,
                replica_groups=[list(range(cfg["n_cores"]))],
                ins=[ar_in[:]], outs=[ar_out[:]])
            pooled = spool.tile([1, 4], dt.float32, tag="p4b", name="pooled")
            nc.sync.dma_start(out=pooled[:], in_=ar_out[:])
            fcw_t = spool.tile([1, 12], dt.float32, tag="fcw", name="fcw_t")
            nc.sync.dma_start(out=fcw_t[:], in_=fcw_d[:])
            fcb_t = spool.tile([1, 3], dt.float32, tag="fcb", name="fcb_t")
            nc.sync.dma_start(out=fcb_t[:], in_=fcb_d[:])
            pr2 = spool.tile([1, 12], dt.float32, tag="pr2", name="pr2")
            nc.vector.tensor_tensor(
                out=pr2[:],
                in0=vap(pooled[:], 0, [[pooled[:].ap[0][0], 1], [0, 3], [1, 4]]),
                in1=vap(fcw_t[:], 0, [[fcw_t[:].ap[0][0], 1], [4, 3], [1, 4]]),
                op=OP.mult)
            y3 = spool.tile([1, 3], dt.float32, tag="y3", name="y3")
            nc.vector.tensor_reduce(
                out=y3[:],
                in_=vap(pr2[:], 0, [[pr2[:].ap[0][0], 1], [4, 3], [1, 4]]),
                axis=AX.X, op=OP.add)
            nc.vector.tensor_tensor(out=y3[:], in0=y3[:], in1=fcb_t[:], op=OP.add)
            nc.sync.dma_start(out=y_out[:], in_=y3[:])

    _split_excess_waits(nc, max_waits=1)
    lower_extended_insts(nc)
    return nc


def kernel(x, edge_index, W_in, Wq, Wk, Wv, Wo, W_out, fc_w, fc_b):
    x = np.asarray(x, dtype=np.float32)
    edge_index = np.asarray(edge_index)
    cfg = _cfg(x.shape[0])

    key = ("nc", x.shape[0], edge_index.shape[1])
    if key not in _RUN_CACHE:
        schedule, plans = _plan(edge_index, cfg)
        meta = (plans[0]["gidx"].shape[1], plans[0]["qidx"].shape[1],
                plans[0]["chunk_meta"])
        nc = _build(meta, cfg)
        _RUN_CACHE[key] = (nc, plans)
    nc, plans = _RUN_CACHE[key]

    wts = _permute_weights(
        np.asarray(W_in, np.float32), np.asarray(Wq, np.float32),
        np.asarray(Wk, np.float32), np.asarray(Wv, np.float32),
        np.asarray(Wo, np.float32), np.asarray(W_out, np.float32),
        np.asarray(fc_w, np.float32), np.asarray(fc_b, np.float32))

    nps, pcs, npp = cfg["n_per_sub"], cfg["per_core_sub"], cfg["npp"]
    old = np.arange(cfg["n_nodes"])
    c_of = (old % nps) // pcs
    l_of = (old // nps) * pcs + (old % pcs)
    in_maps = []
    for c in range(cfg["n_cores"]):
        xl = np.zeros((npp, DIM), dtype=np.float32)
        m = c_of == c
        xl[l_of[m]] = x[m]
        p = plans[c]
        in_maps.append(dict(
            x_in=xl, gidx=p["gidx"], qidx=p["qidx"], cidx=p["cidx"],
            npad=p["npad"], w_in=wts["W_in"], wq=wts["Wq"], wk=wts["Wk"],
            wv=wts["Wv"], wo=wts["Wo"], wout=wts["W_out"], fcw=wts["fcw"],
            fcb=wts["fcb"]))

    from concourse.bass_utils import run_bass_kernel_spmd
    trace = bool(os.environ.get("GNN_TRACE"))
    if trace:
        _install_profhook()
    res = run_bass_kernel_spmd(nc, in_maps, core_ids=list(range(cfg["n_cores"])),
                               trace=trace)
    if trace:
        _RUN_CACHE["last_result"] = res
    return np.asarray(res.results[0]["y"]).reshape(3).astype(np.float32)



# revision 35
# speedup vs baseline: 1.4025x; 1.0686x over previous
"""GNN message-passing (SE3-style graph attention) kernel for 8 Trainium2 cores.

Edge-parallel strategy:
- Nodes relabeled into 4 "subtables" x 8 cores so per-edge kv-gather indices
  fit int16 (dma_gather requirement). Each core owns 12500 dst nodes.
- Per layer: on-device q/k/v projections -> 4 AllGathers build global kv
  tables -> 4 passes over src subtables, each processing edges in node-major
  degree-sorted chunks: dma_gather kv rows, affine q broadcast, DVE
  scores/exp/messages, affine segment reduction into per-pass accumulators.
- Pass accumulators (degree-rank order) recombined into identity order by
  small dma_gathers, then divide / Wo project / residual / LayerNorm.
- Final: W_out, mean-pool via PE ones-matmul, AllReduce, FC head.
"""

import os
import sys
import types
import numpy as np

HEADS = 8
HEAD_DIM = 2
HIDDEN = 16
DIM = 4
DEPTH = 2
N_SUB = 4
KV_COLS = 64          # table row = 64 fp32 = 256B
SENT = 32767          # zeroed sentinel row in each kv subtable
SD_CAP = 128          # max S*D per chunk
S_CAP = 8

_RUN_CACHE = {}


# --------------------------------------------------------------------------
# harness shims (self-contained copies)
# --------------------------------------------------------------------------
def _split_excess_waits(nc, max_waits=1):
    """Walrus build allows 1 sync-wait per instruction; move extras to NOPs."""
    import concourse.mybir as mybir
    n = [0]
    for blk in nc.m.functions[0].blocks:
        new_insts = []
        for inst in blk.instructions:
            si = inst.sync_info
            if si is not None and len(si.on_wait) > max_waits:
                waits = list(si.on_wait)
                extra, keep = waits[:-max_waits], waits[-max_waits:]
                for i in range(0, len(extra), max_waits):
                    n[0] += 1
                    nop = mybir.InstNoOp(
                        name=f"IWS-{n[0]}", engine=inst.engine, ins=[], outs=[],
                        sync_info=mybir.SyncInfo(on_wait=extra[i:i + max_waits],
                                                 on_update=[]))
                    try:
                        nc.register_instruction(nop, overwrite=True)
                    except Exception:
                        pass
                    new_insts.append(nop)
                si.on_wait = keep
            new_insts.append(inst)
        blk.instructions[:] = new_insts


def _install_profhook():
    if 'antenv.axon_hooks' in sys.modules:
        return
    try:
        import antenv
        from trn_agent_boot.trn_boot import _ntff_profile_via_ctypes
        hook = _ntff_profile_via_ctypes('/opt/axon/libaxon_pjrt.so')
        mod = types.ModuleType('antenv.axon_hooks')
        state = {'hook': hook}
        mod.set_axon_ntff_profile_hook = lambda h: state.__setitem__('hook', h)
        mod.get_axon_ntff_profile_hook = lambda: state['hook']
        sys.modules['antenv.axon_hooks'] = mod
        antenv.axon_hooks = mod
    except Exception:
        pass


# --------------------------------------------------------------------------
# host-side planning
# --------------------------------------------------------------------------
def _cfg(n_nodes):
    n_cores = 8
    n_per_sub = n_nodes // N_SUB
    per_core_sub = n_per_sub // n_cores
    npc = N_SUB * per_core_sub
    npp = ((npc + 127) // 128) * 128
    return dict(n_nodes=n_nodes, n_cores=n_cores, n_per_sub=n_per_sub,
                per_core_sub=per_core_sub, npc=npc, npp=npp,
                n_tiles=npp // 128)


def _pack_gidx(idx_flat):
    """Gather feed position i lives at tile[i%16, i//16]; replicate x8 cores."""
    n = idx_flat.shape[0]
    assert n % 16 == 0
    tile16 = np.ascontiguousarray(
        idx_flat.reshape(n // 16, 16).T.astype(np.int16))
    return np.tile(tile16, (8, 1))


def _plan(edge_index, cfg):
    src = np.asarray(edge_index[0], dtype=np.int64)
    dst = np.asarray(edge_index[1], dtype=np.int64)
    nps, pcs = cfg["n_per_sub"], cfg["per_core_sub"]
    npp, n_cores = cfg["npp"], cfg["n_cores"]

    e_core = (dst % nps) // pcs
    e_l = (dst // nps) * pcs + (dst % pcs)
    e_t = src // nps
    e_row = src % nps

    passes = []
    for t in range(N_SUB):
        per_core = []
        for c in range(n_cores):
            m = (e_core == c) & (e_t == t)
            lt, rowt = e_l[m], e_row[m]
            deg = np.bincount(lt, minlength=npp).astype(np.int64)
            order = np.argsort(-deg, kind="stable")
            rank_of = np.empty(npp, dtype=np.int64)
            rank_of[order] = np.arange(npp)
            eorder = np.argsort(rank_of[lt], kind="stable")
            per_core.append(dict(deg=deg, order=order, rank_of=rank_of,
                                 lt=lt[eorder], rowt=rowt[eorder]))
        passes.append(per_core)

    schedule = []
    for t in range(N_SUB):
        chunks = []
        r0 = 0
        degs_sorted = [passes[t][c]["deg"][passes[t][c]["order"]]
                       for c in range(n_cores)]
        while r0 < npp:
            D = int(max(int(d[r0]) for d in degs_sorted))
            if D == 0:
                break
            S = max(1, min(S_CAP, SD_CAP // D, (npp - r0) // 128))
            chunks.append((r0, S, D))
            r0 += 128 * S
        schedule.append(chunks)

    plans = []
    for c in range(n_cores):
        gidx_cols, qidx_list, chunk_meta = [], [], []
        npad = np.zeros(npp, dtype=np.float64)
        gcol0 = 0
        for t in range(N_SUB):
            pc = passes[t][c]
            deg, order, rank_of = pc["deg"], pc["order"], pc["rank_of"]
            lt, rowt = pc["lt"], pc["rowt"]
            offs = np.zeros(npp + 1, dtype=np.int64)
            offs[1:] = np.cumsum(deg[order])
            ranks_e = rank_of[lt]
            j_e = np.arange(lt.shape[0]) - offs[ranks_e]
            # one pass-wide q index block: full rank order
            qidx_list.append(_pack_gidx(order[0:npp]))
            for ci, (r0, S, D) in enumerate(schedule[t]):
                nrows = 128 * S * D
                idx_flat = np.full(nrows, SENT, dtype=np.int64)
                em = (ranks_e >= r0) & (ranks_e < r0 + 128 * S)
                q = ranks_e[em] - r0
                pos = ((q // 128) * D + j_e[em]) * 128 + (q % 128)
                idx_flat[pos] = rowt[em]
                gidx_cols.append(_pack_gidx(idx_flat))
                ch_nodes = order[r0:r0 + 128 * S]
                npad[ch_nodes] += D - deg[ch_nodes]
                chunk_meta.append((t, r0, S, D, gcol0, nrows // 16))
                gcol0 += nrows // 16
        gidx = (np.concatenate(gidx_cols, axis=1) if gidx_cols
                else np.zeros((128, 16), np.int16))
        qidx = (np.concatenate(qidx_list, axis=1) if qidx_list
                else np.zeros((128, 8), np.int16))
        cidx = np.stack([_pack_gidx(passes[t][c]["rank_of"][:npp])
                         for t in range(N_SUB)])
        npad_t = np.ascontiguousarray(
            npad.reshape(cfg["n_tiles"], 128).T.astype(np.float32))
        plans.append(dict(gidx=gidx, qidx=qidx, cidx=cidx, npad=npad_t,
                          chunk_meta=chunk_meta))
    return schedule, plans


def _bd4(w):
    """[a, b] -> [4a, 4b] block-diagonal (4 copies)."""
    a, b = w.shape
    out = np.zeros((4 * a, 4 * b), np.float32)
    for i in range(4):
        out[a * i:a * (i + 1), b * i:b * (i + 1)] = w
    return np.ascontiguousarray(out)


def _permute_weights(W_in, Wq, Wk, Wv, Wo, W_out, fc_w, fc_b):
    perm = np.array([h * HEAD_DIM + d for d in range(HEAD_DIM)
                     for h in range(HEADS)], dtype=np.int64)
    Wqp = Wq[:, :, perm]
    Wkp = Wk[:, :, perm]
    return dict(W_in=np.ascontiguousarray(W_in),
                Wq=np.ascontiguousarray(Wqp),
                Wk=np.ascontiguousarray(Wkp),
                Wv=np.ascontiguousarray(Wv),
                Wo=np.ascontiguousarray(Wo),
                W_out=np.ascontiguousarray(W_out),
                Wq4=np.stack([_bd4(Wqp[l]) for l in range(DEPTH)]),
                Wk4=np.stack([_bd4(Wkp[l]) for l in range(DEPTH)]),
                Wv4=np.stack([_bd4(Wv[l]) for l in range(DEPTH)]),
                Wo4=np.stack([_bd4(Wo[l]) for l in range(DEPTH)]),
                fcw=np.ascontiguousarray(fc_w.T).reshape(1, 12).astype(np.float32),
                fcb=fc_b.reshape(1, 3).astype(np.float32))


# --------------------------------------------------------------------------
# device program
# --------------------------------------------------------------------------
def _build(meta, cfg):
    import concourse.bass as bass
    import concourse.mybir as mybir
    import concourse.tile as tile
    from concourse import library_config
    from concourse.masks import make_identity
    from concourse.library_overlay import lower_extended_insts

    dt = mybir.dt
    AX = mybir.AxisListType
    OP = mybir.AluOpType
    ACTF = mybir.ActivationFunctionType
    npp, n_tiles, npc = cfg["npp"], cfg["n_tiles"], cfg["npc"]
    pcs = cfg["per_core_sub"]
    g_cols_total, q_cols_total, chunk_meta = meta

    nc = bass.Bass(num_devices=cfg["n_cores"], num_swdge_queues=4)

    x_in = nc.dram_tensor("x_in", [npp, DIM], dt.float32, kind="ExternalInput")
    gidx_d = nc.dram_tensor("gidx", [128, g_cols_total], dt.int16, kind="ExternalInput")
    qidx_d = nc.dram_tensor("qidx", [128, q_cols_total], dt.int16, kind="ExternalInput")
    cidx_d = nc.dram_tensor("cidx", [N_SUB, 128, npp // 16], dt.int16, kind="ExternalInput")
    npad_d = nc.dram_tensor("npad", [128, n_tiles], dt.float32, kind="ExternalInput")
    w_in_d = nc.dram_tensor("w_in", [DIM, HIDDEN], dt.float32, kind="ExternalInput")
    wq_d = nc.dram_tensor("wq", [DEPTH, HIDDEN, HIDDEN], dt.float32, kind="ExternalInput")
    wk_d = nc.dram_tensor("wk", [DEPTH, HIDDEN, HIDDEN], dt.float32, kind="ExternalInput")
    wv_d = nc.dram_tensor("wv", [DEPTH, HIDDEN, HIDDEN], dt.float32, kind="ExternalInput")
    wo_d = nc.dram_tensor("wo", [DEPTH, HIDDEN, HIDDEN], dt.float32, kind="ExternalInput")
    wq4_d = nc.dram_tensor("wq4", [DEPTH, 64, 64], dt.float32, kind="ExternalInput")
    wk4_d = nc.dram_tensor("wk4", [DEPTH, 64, 64], dt.float32, kind="ExternalInput")
    wv4_d = nc.dram_tensor("wv4", [DEPTH, 64, 64], dt.float32, kind="ExternalInput")
    wo4_d = nc.dram_tensor("wo4", [DEPTH, 64, 64], dt.float32, kind="ExternalInput")
    wout_d = nc.dram_tensor("wout", [HIDDEN, DIM], dt.float32, kind="ExternalInput")
    fcw_d = nc.dram_tensor("fcw", [1, 12], dt.float32, kind="ExternalInput")
    fcb_d = nc.dram_tensor("fcb", [1, 3], dt.float32, kind="ExternalInput")
    y_out = nc.dram_tensor("y", [1, 3], dt.float32, kind="ExternalOutput")

    kv_tab = [nc.dram_tensor(f"kvtab{t}", [32768, 128], dt.bfloat16,
                             kind="Internal", addr_space="Shared")
              for t in range(N_SUB)]
    q_tab = nc.dram_tensor("qtab", [npp, 128], dt.bfloat16, kind="Internal")
    u_tab = [nc.dram_tensor(f"utab{t}", [npp, KV_COLS], dt.float32, kind="Internal")
             for t in range(N_SUB)]
    stage_t = [nc.dram_tensor(f"stage{t}", [npc // N_SUB, 128], dt.bfloat16,
                              kind="Internal") for t in range(N_SUB)]
    ar_in = nc.dram_tensor("ar_in", [1, 4], dt.float32, kind="Internal")
    ar_out = nc.dram_tensor("ar_out", [1, 4], dt.float32, kind="Internal",
                            addr_space="Shared")

    nc.gpsimd.load_library(library_config.attnmlp)
    gq = [0]
    _nregs = {}

    def gather(out_ap, in_tensor, nrows, idx_ap, num_idxs, elem_size,
               pitch=KV_COLS):
        """Raw InstDMAGatherAnt reading elem_size elems from 256B-pitch rows.

        Bypasses bass's elem_size_bytes%256 assert: the ucode only requires
        the row *pitch* (elem_step bytes) to be a 256B multiple."""
        if num_idxs not in _nregs:
            _nregs[num_idxs] = nc.gpsimd.to_reg(num_idxs)
        g = nc.gpsimd
        in_ap = bass.AP(in_tensor, 0, [[pitch, nrows], [1, elem_size]])
        _in_ap = g.lower_ap_dma(in_ap, for_custom_bir_dma=True)
        _idxs_ap = g.lower_ap(idx_ap)
        _out_ap = g.lower_ap(out_ap)
        g.add_instruction(mybir.InstDMAGatherAnt(
            name=nc.get_next_instruction_name(),
            ins=[*_in_ap, _idxs_ap, g.lower_val_access(_nregs[num_idxs])],
            outs=[_out_ap],
            transpose=False, num_idxs=num_idxs, elem_size=elem_size,
            stride_bytes_256=1, gen_mode=0, single_packet=False,
            queue_num=gq[0] % 4,
            sbuf_tokens_per_rank=0, sbuf_free_dim_per_rank=0,
            sbuf_free_dim_pad_per_rank=0, sbuf_byte_offset=0))
        gq[0] += 1

    def vap(base_ap, extra_off, dims):
        return bass.AP(base_ap.tensor, base_ap.offset + extra_off, dims)

    with tile.TileContext(nc) as tc:
        with (
            tc.tile_pool(name="const", bufs=1) as cpool,
            tc.tile_pool(name="res", bufs=1) as rpool,
            tc.tile_pool(name="work", bufs=3) as wpool,
            tc.tile_pool(name="gath", bufs=4) as gpool,
            tc.tile_pool(name="small", bufs=3) as spool,
            tc.tile_pool(name="psum", bufs=4, space="PSUM") as ppool,
        ):
            ident = cpool.tile([128, 128], dt.float32, name="ident")
            make_identity(nc, ident[:])
            w_in_t = cpool.tile([DIM, HIDDEN], dt.float32, name="w_in_t")
            nc.sync.dma_start(out=w_in_t[:], in_=w_in_d[:])
            wmat = {}
            for nm, dd in (("wq", wq_d), ("wk", wk_d), ("wv", wv_d), ("wo", wo_d)):
                for l in range(DEPTH):
                    w = cpool.tile([HIDDEN, HIDDEN], dt.float32, name=f"{nm}{l}")
                    nc.sync.dma_start(out=w[:], in_=dd[l])
                    wmat[(nm, l)] = w
            for nm, dd in (("wq4", wq4_d), ("wk4", wk4_d), ("wv4", wv4_d),
                           ("wo4", wo4_d)):
                for l in range(DEPTH):
                    w = cpool.tile([64, 64], dt.float32, name=f"{nm}{l}")
                    nc.sync.dma_start(out=w[:], in_=dd[l])
                    wmat[(nm, l)] = w
            wout_t = cpool.tile([HIDDEN, DIM], dt.float32, name="wout_t")
            nc.sync.dma_start(out=wout_t[:], in_=wout_d[:])
            npad_t = cpool.tile([128, n_tiles], dt.float32, name="npad_t")
            nc.sync.dma_start(out=npad_t[:], in_=npad_d[:])

            zrow = cpool.tile([1, 128], dt.bfloat16, name="zrow")
            nc.vector.memset(zrow[:], 0.0)
            for t in range(N_SUB):
                nc.sync.dma_start(out=kv_tab[t][SENT:SENT + 1, :], in_=zrow[:])

            h_all = rpool.tile([128, n_tiles, HIDDEN], dt.float32, name="h_all")
            u_tot = rpool.tile([128, n_tiles, 24], dt.float32, name="u_tot")
            u_accs = [rpool.tile([128, n_tiles, 24], dt.float32, name=f"u_acc{i}")
                      for i in range(N_SUB)]
            acc4 = rpool.tile([128, 4], dt.float32, name="acc4")

            # stage 0: h0 = x @ W_in
            xs = wpool.tile([128, n_tiles, DIM], dt.float32, tag="gt", name="xs")
            nc.sync.dma_start(out=xs[:],
                              in_=x_in[:].rearrange("(a p) f -> p a f", p=128))
            for k in range(n_tiles):
                tp = ppool.tile([DIM, 128], dt.float32, tag="tp", name="tp")
                nc.tensor.transpose(out=tp[:], in_=xs[:, k, :], identity=ident[:])
                tps = spool.tile([HIDDEN, 128], dt.float32, tag="hT", name="tps")
                nc.vector.tensor_copy(out=tps[0:DIM, :], in_=tp[:])
                hp = ppool.tile([128, HIDDEN], dt.float32, tag="mm", name="hp")
                nc.tensor.matmul(out=hp[:], lhsT=tps[0:DIM, :], rhs=w_in_t[:],
                                 start=True, stop=True)
                nc.vector.tensor_copy(out=h_all[:, k, :], in_=hp[:])

            scale = float(1.0 / np.sqrt(HEAD_DIM))

            def issue_ag(t):
                nc.gpsimd.collective_compute(
                    "AllGather", mybir.AluOpType.bypass,
                    replica_groups=[list(range(cfg["n_cores"]))],
                    ins=[stage_t[t][:]],
                    outs=[kv_tab[t][0:cfg["n_cores"] * pcs, :]])

            for layer in range(DEPTH):
                # q/k/v projections, 4 tiles per group via block-diagonal
                # weights; stage written per-subtable so each AllGather
                # depends only on its own slab
                for g0 in range(0, n_tiles, 4):
                    nt = min(4, n_tiles - g0)
                    w = 16 * nt
                    tp = ppool.tile([64, 128], dt.float32, tag="tp", name="htp")
                    nc.tensor.transpose(
                        out=tp[0:w, :],
                        in_=h_all[:, g0:g0 + nt, :].rearrange("p a b -> p (a b)"),
                        identity=ident[:])
                    hT = spool.tile([64, 128], dt.float32, tag="hT", name="hT")
                    nc.vector.tensor_copy(out=hT[0:w, :], in_=tp[0:w, :])
                    qp = ppool.tile([128, 64], dt.float32, tag="mm", name="qp")
                    nc.tensor.matmul(out=qp[:, 0:w], lhsT=hT[0:w, :],
                                     rhs=wmat[("wq4", layer)][0:w, 0:w],
                                     start=True, stop=True)
                    qs = spool.tile([128, 4, HIDDEN], dt.bfloat16, tag="qs",
                                    name="qs")
                    nc.vector.tensor_copy(
                        out=qs[:, 0:nt, :].rearrange("p a b -> p (a b)"),
                        in_=qp[:, 0:w])
                    nc.sync.dma_start(
                        out=bass.AP(q_tab, g0 * 128 * 128,
                                    [[128, 128], [128 * 128, nt], [1, HIDDEN]]),
                        in_=qs[:, 0:nt, :])
                    kp = ppool.tile([128, 64], dt.float32, tag="mm", name="kp")
                    nc.tensor.matmul(out=kp[:, 0:w], lhsT=hT[0:w, :],
                                     rhs=wmat[("wk4", layer)][0:w, 0:w],
                                     start=True, stop=True)
                    kvs = spool.tile([128, 4, 32], dt.bfloat16, tag="kvs",
                                     name="kvs")
                    pkv = kvs[:].ap[0][0]
                    nc.vector.tensor_copy(
                        out=vap(kvs[:], 0,
                                [[pkv, 128], [32, nt], [1, HIDDEN]]),
                        in_=kp[:, 0:w])
                    vp = ppool.tile([128, 64], dt.float32, tag="mm", name="vp")
                    nc.tensor.matmul(out=vp[:, 0:w], lhsT=hT[0:w, :],
                                     rhs=wmat[("wv4", layer)][0:w, 0:w],
                                     start=True, stop=True)
                    nc.vector.tensor_copy(
                        out=vap(kvs[:], 16,
                                [[pkv, 128], [32, nt], [1, HIDDEN]]),
                        in_=vp[:, 0:w])
                    # stage rows, split at tile and subtable boundaries
                    lo, hi = g0 * 128, min((g0 + nt) * 128, npc)
                    r = lo
                    while r < hi:
                        t = r // pcs
                        tile_i = (r - lo) // 128
                        e = min(hi, (t + 1) * pcs, lo + (tile_i + 1) * 128)
                        pa = (r - lo) % 128
                        nc.sync.dma_start(
                            out=stage_t[t][r - t * pcs:e - t * pcs, 0:32],
                            in_=kvs[pa:pa + (e - r), tile_i, :])
                        r = e
                issue_ag(0)
                issue_ag(1)

                for t in range(N_SUB):
                    u_acc = u_accs[t]
                    nc.vector.memset(u_acc[:], 0.0)
                    # pass-wide q gather: q_tab permuted into rank order
                    qt = spool.tile([128, npp // 16], dt.int16, tag="qt", name="qt")
                    nc.sync.dma_start(
                        out=qt[:], in_=qidx_d[:, t * (npp // 16):(t + 1) * (npp // 16)])
                    qg = wpool.tile([128, n_tiles, HIDDEN], dt.bfloat16,
                                    tag="qgp", name="qgp")
                    done = 0
                    while done < n_tiles:
                        cnt = min(32, n_tiles - done)
                        gather(qg[:, done:done + cnt, :], q_tab, npp,
                               qt[:, done * 8:(done + cnt) * 8], cnt * 128, HIDDEN,
                               pitch=128)
                        done += cnt
                    qgp = qg[:]
                    pq = qgp.ap[0][0]
                    for (tt, r0, S, D, gc0, gcols) in chunk_meta:
                        if tt != t:
                            continue
                        sl = r0 // 128
                        gt = wpool.tile([128, 1024], dt.int16, tag="gt", name="gt")
                        nc.sync.dma_start(out=gt[:, 0:gcols],
                                          in_=gidx_d[:, gc0:gc0 + gcols])
                        kvg = gpool.tile([128, SD_CAP, 32], dt.bfloat16,
                                         tag="kvg", name="kvg")
                        nrow = S * D
                        done = 0
                        while done < nrow:
                            cnt = min(32, nrow - done)
                            gather(kvg[:, done:done + cnt, :], kv_tab[t], 32768,
                                   gt[:, done * 8:(done + cnt) * 8], cnt * 128, 32,
                                   pitch=128)
                            done += cnt
                        kvga = kvg[:]
                        qga = vap(qgp, sl * HIDDEN,
                                  [[pq, 128], [HIDDEN, S], [1, HIDDEN]])
                        pk = kvga.ap[0][0]
                        prod = wpool.tile([128, S * HIDDEN, D],
                                          dt.bfloat16, tag="prod", name="prod")
                        pp = prod[:].ap[0][0]
                        nc.vector.tensor_tensor(
                            out=vap(prod[:], 0,
                                    [[pp, 128], [HIDDEN * D, S], [D, HIDDEN], [1, D]]),
                            in0=vap(qga, 0,
                                    [[pq, 128], [HIDDEN, S], [1, HIDDEN], [0, D]]),
                            in1=vap(kvga, 0,
                                    [[pk, 128], [D * 32, S], [1, HIDDEN],
                                     [32, D]]),
                            op=OP.mult)
                        wgt = wpool.tile([128, S * HEADS, D],
                                         dt.bfloat16, tag="wgt", name="wgt")
                        pw = wgt[:].ap[0][0]
                        nc.vector.tensor_tensor(
                            out=vap(wgt[:], 0,
                                    [[pw, 128], [HEADS * D, S], [D, HEADS], [1, D]]),
                            in0=vap(prod[:], 0,
                                    [[pp, 128], [HIDDEN * D, S], [D, HEADS], [1, D]]),
                            in1=vap(prod[:], HEADS * D,
                                    [[pp, 128], [HIDDEN * D, S], [D, HEADS], [1, D]]),
                            op=OP.add)
                        nc.scalar.activation(
                            out=vap(wgt[:], 0, [[pw, 128], [1, S * HEADS * D]]),
                            in_=vap(wgt[:], 0, [[pw, 128], [1, S * HEADS * D]]),
                            func=ACTF.Exp, scale=scale)
                        nc.vector.tensor_reduce(
                            out=u_acc[:, sl:sl + S, 16:24],
                            in_=vap(wgt[:], 0,
                                    [[pw, 128], [D, S * HEADS], [1, D]]),
                            axis=AX.X, op=OP.add)
                        msg = wpool.tile([128, S * HIDDEN, D],
                                         dt.bfloat16, tag="prod", name="msg")
                        pm = msg[:].ap[0][0]
                        nc.vector.tensor_tensor(
                            out=vap(msg[:], 0,
                                    [[pm, 128], [HIDDEN * D, S], [HEAD_DIM * D, HEADS],
                                     [D, HEAD_DIM], [1, D]]),
                            in0=vap(wgt[:], 0,
                                    [[pw, 128], [HEADS * D, S], [D, HEADS],
                                     [0, HEAD_DIM], [1, D]]),
                            in1=vap(kvga, 16,
                                    [[pk, 128], [D * 32, S], [HEAD_DIM, HEADS],
                                     [1, HEAD_DIM], [32, D]]),
                            op=OP.mult)
                        nc.vector.tensor_reduce(
                            out=u_acc[:, sl:sl + S, 0:16],
                            in_=vap(msg[:], 0,
                                    [[pm, 128], [D, S * HIDDEN], [1, D]]),
                            axis=AX.X, op=OP.add)
                    # u_acc (rank order) -> DRAM; recombination deferred to
                    # layer end so pass t+1 gathers are never blocked
                    nc.sync.dma_start(
                        out=bass.AP(u_tab[t], 0,
                                    [[KV_COLS, 128], [128 * KV_COLS, n_tiles],
                                     [1, 24]]),
                        in_=u_acc[:])
                    if t + 2 < N_SUB:
                        issue_ag(t + 2)

                # recombine all passes into identity order
                nc.vector.memset(u_tot[:], 0.0)
                for t in range(N_SUB):
                    ct = spool.tile([128, npp // 16], dt.int16, tag="ct", name="ct")
                    nc.sync.dma_start(out=ct[:], in_=cidx_d[t])
                    done = 0
                    while done < n_tiles:
                        cnt = min(32, n_tiles - done)
                        cg = spool.tile([128, 32, 24], dt.float32, tag="cgt",
                                        name="cg")
                        gather(cg[:, 0:cnt, :], u_tab[t], npp,
                               ct[:, done * 8:(done + cnt) * 8], cnt * 128, 24)
                        nc.vector.tensor_tensor(
                            out=u_tot[:, done:done + cnt, :],
                            in0=u_tot[:, done:done + cnt, :],
                            in1=cg[:, 0:cnt, :], op=OP.add)
                        done += cnt

                # epilogue
                dadj = spool.tile([128, n_tiles, HEADS], dt.float32, tag="dadj",
                                  name="dadj")
                pn = npad_t[:].ap[0][0]
                nc.vector.tensor_tensor(
                    out=dadj[:], in0=u_tot[:, :, 16:24],
                    in1=vap(npad_t[:], 0, [[pn, 128], [1, n_tiles], [0, HEADS]]),
                    op=OP.subtract)
                nc.vector.tensor_scalar_add(out=dadj[:], in0=dadj[:], scalar1=1e-9)
                rden = spool.tile([128, n_tiles, HEADS], dt.float32, tag="rden",
                                  name="rden")
                nc.vector.reciprocal(out=rden[:], in_=dadj[:])
                agg = wpool.tile([128, n_tiles, HIDDEN], dt.float32, tag="wgt",
                                 name="agg")
                pr_ = rden[:].ap[0][0]
                pa = agg[:].ap[0][0]
                nc.vector.tensor_tensor(
                    out=vap(agg[:], 0,
                            [[pa, 128], [HIDDEN, n_tiles], [HEAD_DIM, HEADS],
                             [1, HEAD_DIM]]),
                    in0=vap(u_tot[:], 0,
                            [[u_tot[:].ap[0][0], 128], [24, n_tiles],
                             [HEAD_DIM, HEADS], [1, HEAD_DIM]]),
                    in1=vap(rden[:], 0,
                            [[pr_, 128], [HEADS, n_tiles], [1, HEADS],
                             [0, HEAD_DIM]]),
                    op=OP.mult)
                hnew = wpool.tile([128, n_tiles, HIDDEN], dt.float32, tag="hnw",
                                  name="hnew")
                for g0 in range(0, n_tiles, 4):
                    nt = min(4, n_tiles - g0)
                    w = 16 * nt
                    tp = ppool.tile([64, 128], dt.float32, tag="tp", name="atp")
                    nc.tensor.transpose(
                        out=tp[0:w, :],
                        in_=agg[:, g0:g0 + nt, :].rearrange("p a b -> p (a b)"),
                        identity=ident[:])
                    aT = spool.tile([64, 128], dt.float32, tag="hT", name="aT")
                    nc.vector.tensor_copy(out=aT[0:w, :], in_=tp[0:w, :])
                    op_ = ppool.tile([128, 64], dt.float32, tag="mm", name="op_")
                    nc.tensor.matmul(out=op_[:, 0:w], lhsT=aT[0:w, :],
                                     rhs=wmat[("wo4", layer)][0:w, 0:w],
                                     start=True, stop=True)
                    nc.vector.tensor_tensor(
                        out=hnew[:, g0:g0 + nt, :].rearrange("p a b -> p (a b)"),
                        in0=op_[:, 0:w],
                        in1=h_all[:, g0:g0 + nt, :].rearrange("p a b -> p (a b)"),
                        op=OP.add)
                mu = spool.tile([128, n_tiles, 1], dt.float32, tag="mu", name="mu")
                nc.vector.tensor_reduce(out=mu[:], in_=hnew[:], axis=AX.X, op=OP.add)
                nc.vector.tensor_scalar_mul(out=mu[:], in0=mu[:], scalar1=1.0 / 16)
                cent = wpool.tile([128, n_tiles, HIDDEN], dt.float32, tag="wgt",
                                  name="cent")
                nc.vector.tensor_tensor(
                    out=cent[:], in0=hnew[:],
                    in1=vap(mu[:], 0, [[mu[:].ap[0][0], 128], [1, n_tiles],
                                       [0, HIDDEN]]),
                    op=OP.subtract)
                sq = wpool.tile([128, n_tiles, HIDDEN], dt.float32, tag="prod",
                                name="sq")
                nc.vector.tensor_tensor(out=sq[:], in0=cent[:], in1=cent[:],
                                        op=OP.mult)
                var = spool.tile([128, n_tiles, 1], dt.float32, tag="var", name="var")
                nc.vector.tensor_reduce(out=var[:], in_=sq[:], axis=AX.X, op=OP.add)
                nc.vector.tensor_scalar_mul(out=var[:], in0=var[:], scalar1=1.0 / 16)
                nc.vector.tensor_scalar_add(out=var[:], in0=var[:], scalar1=1e-5)
                rs = spool.tile([128, n_tiles, 1], dt.float32, tag="rs", name="rs")
                nc.vector.reciprocal(out=rs[:], in_=var[:])
                nc.scalar.activation(out=rs[:], in_=rs[:], func=ACTF.Sqrt)
                nc.vector.tensor_tensor(
                    out=h_all[:], in0=cent[:],
                    in1=vap(rs[:], 0, [[rs[:].ap[0][0], 128], [1, n_tiles],
                                       [0, HIDDEN]]),
                    op=OP.mult)

            # final head
            nc.vector.memset(acc4[:], 0.0)
            for k in range(n_tiles):
                tp = ppool.tile([HIDDEN, 128], dt.float32, tag="tp", name="ftp")
                nc.tensor.transpose(out=tp[:], in_=h_all[:, k, :], identity=ident[:])
                hT = spool.tile([HIDDEN, 128], dt.float32, tag="hT", name="fhT")
                nc.vector.tensor_copy(out=hT[:], in_=tp[:])
                gp = ppool.tile([128, DIM], dt.float32, tag="mm", name="gp")
                nc.tensor.matmul(out=gp[:], lhsT=hT[:], rhs=wout_t[:],
                                 start=True, stop=True)
                nc.vector.tensor_tensor(out=acc4[:], in0=acc4[:], in1=gp[:],
                                        op=OP.add)
            onesk = cpool.tile([128, 1], dt.float32, name="onesk")
            nc.vector.memset(onesk[:], 1.0 / cfg["n_nodes"])
            pooled_p = ppool.tile([1, 4], dt.float32, tag="mm", name="pooled_p")
            nc.tensor.matmul(out=pooled_p[:], lhsT=onesk[:], rhs=acc4[:],
                             start=True, stop=True)
            pooled_s = spool.tile([1, 4], dt.float32, tag="p4", name="pooled_s")
            nc.vector.tensor_copy(out=pooled_s[:], in_=pooled_p[:])
            nc.sync.dma_start(out=ar_in[:], in_=pooled_s[:])
            nc.gpsimd.collective_compute(
                "AllReduce", mybir.AluOpType.add,
                replica_groups=[list(range(cfg["n_cores"]))],
                ins=[ar_in[:]], outs=[ar_out[:]])
            pooled = spool.tile([1, 4], dt.float32, tag="p4b", name="pooled")
            nc.sync.dma_start(out=pooled[:], in_=ar_out[:])
            fcw_t = spool.tile([1, 12], dt.float32, tag="fcw", name="fcw_t")
            nc.sync.dma_start(out=fcw_t[:], in_=fcw_d[:])
            fcb_t = spool.tile([1, 3], dt.float32, tag="fcb", name="fcb_t")
            nc.sync.dma_start(out=fcb_t[:], in_=fcb_d[:])
            pr2 = spool.tile([1, 12], dt.float32, tag="pr2", name="pr2")
            nc.vector.tensor_tensor(
                out=pr2[:],
                in0=vap(pooled[:], 0, [[pooled[:].ap[0][0], 1], [0, 3], [1, 4]]),
                in1=vap(fcw_t[:], 0, [[fcw_t[:].ap[0][0], 1], [4, 3], [1, 4]]),
                op=OP.mult)
            y3 = spool.tile([1, 3], dt.float32, tag="y3", name="y3")
            nc.vector.tensor_reduce(
                out=y3[:],
                in_=vap(pr2[:], 0, [[pr2[:].ap[0][0], 1], [4, 3], [1, 4]]),
                axis=AX.X, op=OP.add)
            nc.vector.tensor_tensor(out=y3[:], in0=y3[:], in1=fcb_t[:], op=OP.add)
            nc.sync.dma_start(out=y_out[:], in_=y3[:])

    _split_excess_waits(nc, max_waits=1)
    lower_extended_insts(nc)
    return nc


def kernel(x, edge_index, W_in, Wq, Wk, Wv, Wo, W_out, fc_w, fc_b):
    x = np.asarray(x, dtype=np.float32)
    edge_index = np.asarray(edge_index)
    cfg = _cfg(x.shape[0])

    key = ("nc", x.shape[0], edge_index.shape[1])
    if key not in _RUN_CACHE:
        schedule, plans = _plan(edge_index, cfg)
        meta = (plans[0]["gidx"].shape[1], plans[0]["qidx"].shape[1],
                plans[0]["chunk_meta"])
        nc = _build(meta, cfg)
        _RUN_CACHE[key] = (nc, plans)
    nc, plans = _RUN_CACHE[key]

    wts = _permute_weights(
        np.asarray(W_in, np.float32), np.asarray(Wq, np.float32),
        np.asarray(Wk, np.float32), np.asarray(Wv, np.float32),
        np.asarray(Wo, np.float32), np.asarray(W_out, np.float32),
        np.asarray(fc_w, np.float32), np.asarray(fc_b, np.float32))

    nps, pcs, npp = cfg["n_per_sub"], cfg["per_core_sub"], cfg["npp"]
    old = np.arange(cfg["n_nodes"])
    c_of = (old % nps) // pcs
    l_of = (old // nps) * pcs + (old % pcs)
    in_maps = []
    for c in range(cfg["n_cores"]):
        xl = np.zeros((npp, DIM), dtype=np.float32)
        m = c_of == c
        xl[l_of[m]] = x[m]
        p = plans[c]
        in_maps.append(dict(
            x_in=xl, gidx=p["gidx"], qidx=p["qidx"], cidx=p["cidx"],
            npad=p["npad"], w_in=wts["W_in"], wq=wts["Wq"], wk=wts["Wk"],
            wv=wts["Wv"], wo=wts["Wo"], wq4=wts["Wq4"], wk4=wts["Wk4"],
            wv4=wts["Wv4"], wo4=wts["Wo4"], wout=wts["W_out"], fcw=wts["fcw"],
            fcb=wts["fcb"]))

    from concourse.bass_utils import run_bass_kernel_spmd
    trace = bool(os.environ.get("GNN_TRACE"))
    if trace:
        _install_profhook()
    res = run_bass_kernel_spmd(nc, in_maps, core_ids=list(range(cfg["n_cores"])),
                               trace=trace)
    if trace:
        _RUN_CACHE["last_result"] = res
    return np.asarray(res.results[0]["y"]).reshape(3).astype(np.float32)

